# revision 40
# baseline (speedup 1.0000x reference)
"""Causal attention head (S=8192, De=dim=256) on 8 trn2 NeuronCores.

Math (reference):
    Q = Wq @ x.T; K = Wk @ x.T; V = Wv @ x.T
    S = (Q.T @ K) / sqrt(256); causal mask (upper tri -> -inf)
    out = softmax(S, axis=1) @ V.T          # [8192, 256]

Sharding: core c owns rows c::8 (stride-8 interleave) -> every core's
row block has a near-identical causal prefix profile, so the SPMD kernel
is identical across cores; all per-core variation is input data.

Per-core kernel, fp8e4 DoubleRow edition. The PE's fp8 DoubleRow mode
contracts 2x128 partitions per pass at 0.5 cycles/row (4x bf16 for a
256-deep contraction), so the large matmuls run in fp8; the few-key
early causal rows (chunk 0 = local rows 0..255) keep an exact bf16
score+PV path since score noise doesn't average out there.
  - K: col tiles 0..15 bf16-generated (kept as k16 for chunk-0 ST,
    gpsimd-quantized into k8), tiles 16..63 fp8 DoubleRow into k8.
  - V: tiles 0..15 bf16 into vt16 (chunk-0 PV) + gpsimd-converted into
    vt8; tiles 16..63 fp8 DoubleRow straight into vt8.
  - Q bf16 -> q8 (all rows) and q16 (chunk-0 rows).
  - scores: S.T tile = K_j.T @ Q_r as ONE DoubleRow inst per 128-col
    tile for chunks 1-3, bf16 two-inst contraction for chunk 0.
  - exp on ACT: fp8 out with exponent bias -2 for chunks 1-3 (score
    outliers reach ~95, exp(s/16) would overflow fp8e4's 240 max; the
    bias cancels in the row-sum normalization), bf16 out for chunk 0.
  - causal band masks: 0/1 multiply over just the 128-row mixed window
    per tile; gpsimd for early band groups, DVE (2x bf16) otherwise.
  - PV: chunk 0 in bf16 (exact V), chunks 1-3 DoubleRow over col-tile
    pairs, moving dim split (0:128 | 128:257) to stay under the 512
    moving-row ISA limit; ones column yields row sums for free.
  - K/V generation is emitted as "filler" groups interleaved into the
    attention chunks (just-in-time via ensure_*), so their PSUM->SBUF
    copies overlap the exp-bound phase; the softmax pipeline is 6 deep
    and carried across chunk boundaries.
No softmax max-subtraction needed in fp32: |scores/16| <= ~6.
"""

import sys

sys.path.insert(0, "/opt/trn_rl_repo")

from contextlib import ExitStack

import ml_dtypes
import numpy as np

import concourse.bass as bass
import concourse.mybir as mybir
import concourse.tile as tile
from concourse import bacc
from concourse.bass_utils import run_bass_kernel_spmd

BF16 = mybir.dt.bfloat16
F8 = mybir.dt.float8e4
F32 = mybir.dt.float32
NPBF16 = ml_dtypes.bfloat16
NPF8 = ml_dtypes.float8_e4m3
DR = mybir.MatmulPerfMode.DoubleRow

S, DIM, DE = 8192, 256, 256
NCORES = 8
RPC = S // NCORES          # 1024 rows per core
RCHUNK = 256               # rows per S.T matmul (moving free dim)
NRC = RPC // RCHUNK        # 4 row chunks per core
CT = 128                   # col tile (PE partition)
NCT = S // CT              # 64 col tiles total
BAND = 2048 // CT          # 16 col tiles per causal band of a row chunk
VW = DE + 1                # V.T chunk width incl. ones column
BCOLS = 2048               # cols covered by the bf16 K/V path (tiles < 16)

_cached = {}


def _build_nc(repeat=0, debug=False):
    nc = bacc.Bacc("TRN2", target_bir_lowering=False, debug=False,
                   num_devices=NCORES)
    x8d = nc.dram_tensor("x8", [128, 2 * S], F8, kind="ExternalInput")
    xTd = nc.dram_tensor("xT", [2, 128, BCOLS], BF16, kind="ExternalInput")
    xqT = nc.dram_tensor("xqT", [2, 128, RPC], BF16, kind="ExternalInput")
    wqT = nc.dram_tensor("wqT", [2, 128, DIM], BF16, kind="ExternalInput")
    wkT = nc.dram_tensor("wkT", [2, 128, DIM], BF16, kind="ExternalInput")
    wvT = nc.dram_tensor("wvT", [2, 128, DE], BF16, kind="ExternalInput")
    wk8d = nc.dram_tensor("wk8", [128, 2 * DIM], F8, kind="ExternalInput")
    wv8d = nc.dram_tensor("wv8", [128, 2 * DE], F8, kind="ExternalInput")
    m16d = nc.dram_tensor("m16", [128, BAND * 128], BF16, kind="ExternalInput")
    m8d = nc.dram_tensor("m8", [128, BAND * 128], F8, kind="ExternalInput")
    outd = nc.dram_tensor("out", [RPC, DE], F32, kind="ExternalOutput")
    dbg = None
    if debug:
        dbg = {
            "k8o": nc.dram_tensor("k8o", [128, 2 * S], F8, kind="ExternalOutput"),
            "q8o": nc.dram_tensor("q8o", [128, 2 * RPC], F8, kind="ExternalOutput"),
            "vt8o": nc.dram_tensor("vt8o", [128, NCT * VW], F8, kind="ExternalOutput"),
            "m8o": nc.dram_tensor("m8o", [128, BAND * 128], F8, kind="ExternalOutput"),
            "ptA": nc.dram_tensor("ptA", [128, 1024], F8, kind="ExternalOutput"),
            "ptB": nc.dram_tensor("ptB", [128, 1024], F8, kind="ExternalOutput"),
            "ptC": nc.dram_tensor("ptC", [128, 1024], F8, kind="ExternalOutput"),
            "pv0": nc.dram_tensor("pv0", [128, VW], F32, kind="ExternalOutput"),
            "pv1": nc.dram_tensor("pv1", [128, VW], F32, kind="ExternalOutput"),
        }

    with tile.TileContext(nc) as tc, ExitStack() as ctx:
        const = ctx.enter_context(tc.tile_pool(name="const", bufs=1))
        ps_st = ctx.enter_context(tc.tile_pool(name="ps_st", bufs=3, space="PSUM"))
        ps_pv = ctx.enter_context(tc.tile_pool(name="ps_pv", bufs=1, space="PSUM"))
        pt_pool = ctx.enter_context(tc.tile_pool(name="pt", bufs=9))
        ep_pool = ctx.enter_context(tc.tile_pool(name="ep", bufs=4))

        def body(_iv=None):
            _emit(nc, tc, const, ps_st, ps_pv, pt_pool, ep_pool,
                  x8d, xTd, xqT, wqT, wkT, wvT, wk8d, wv8d, m16d, m8d, outd,
                  dbg)

        if repeat:
            with tc.For_i(0, repeat, 1) as _iv:
                body(_iv)
        else:
            body()

    nc.compile()
    return nc


def _emit(nc, tc, const, ps_st, ps_pv, pt_pool, ep_pool,
          x8d, xTd, xqT, wqT, wkT, wvT, wk8d, wv8d, m16d, m8d, outd,
          dbg=None):
    # ---- constants / staged inputs in SBUF ----
    x8_sb = const.tile([128, 2 * S], F8, tag="x8")
    xt_sb = [const.tile([128, BCOLS], BF16, tag=f"xt{i}", name=f"xt{i}") for i in range(2)]
    xq_sb = [const.tile([128, RPC], BF16, tag=f"xq{i}", name=f"xq{i}") for i in range(2)]
    wq_sb = const.tile([128, 2 * DIM], BF16, tag="wq")
    wk_sb = const.tile([128, 2 * DIM], BF16, tag="wk")
    wv_sb = const.tile([128, 2 * DE], BF16, tag="wv")
    wk8_sb = const.tile([128, 2 * DIM], F8, tag="wk8")
    wv8_sb = const.tile([128, 2 * DE], F8, tag="wv8")
    m16_sb = const.tile([128, BAND * 128], BF16, tag="m16")
    m8_sb = const.tile([128, BAND * 128], F8, tag="m8")
    k8_sb = const.tile([128, 2 * S], F8, tag="k8")
    q8_sb = const.tile([128, 2 * RPC], F8, tag="q8")
    k16_sb = [const.tile([128, BCOLS], BF16, tag=f"k16_{i}", name=f"k16_{i}")
              for i in range(2)]
    q16_sb = [const.tile([128, RCHUNK], BF16, tag=f"q16_{i}", name=f"q16_{i}")
              for i in range(2)]
    vt8_sb = const.tile([128, NCT * VW], F8, tag="vt8")
    vt16_sb = const.tile([128, BAND * VW], BF16, tag="vt16")

    x8_3 = x8_sb[:].rearrange("p (i c) -> p i c", i=2)     # [128,2,S]
    wk8_3 = wk8_sb[:].rearrange("p (i d) -> p i d", i=2)   # [128,2,256]
    wv8_3 = wv8_sb[:].rearrange("p (i e) -> p i e", i=2)   # [128,2,256]
    k8_3 = k8_sb[:].rearrange("p (i c) -> p i c", i=2)     # [128,2,S]
    q8_3 = q8_sb[:].rearrange("p (i r) -> p i r", i=2)     # [128,2,RPC]

    # input staging: weights ride the otherwise-idle DVE queue, x.T (bf16)
    # in fine chunks on ACT (K-bf16 consumes it first), x8 immediately on
    # SP in consumption order, xq+masks via gpsimd SWDGE.
    for i in range(2):
        nc.gpsimd.dma_start(xq_sb[i][:], xqT[i, :, :])
        nc.gpsimd.dma_start(wq_sb[:, i * DIM:(i + 1) * DIM], wqT[i, :, :])
    for i in range(2):
        nc.gpsimd.dma_start(wk_sb[:, i * DIM:(i + 1) * DIM], wkT[i, :, :])
        nc.gpsimd.dma_start(wv_sb[:, i * DE:(i + 1) * DE], wvT[i, :, :])
    nc.gpsimd.dma_start(wv8_sb[:], wv8d[:, :])
    nc.gpsimd.dma_start(wk8_sb[:], wk8d[:, :])
    nc.gpsimd.dma_start(m8_sb[:], m8d[:, :])
    nc.gpsimd.dma_start(m16_sb[:], m16d[:, :])
    for o in range(0, BCOLS, 1024):
        for i in range(2):
            nc.scalar.dma_start(xt_sb[i][:, o:o + 1024], xTd[i, :, o:o + 1024])
    XCH = 2048
    for o in range(0, S, XCH):
        for i in range(2):
            nc.sync.dma_start(x8_sb[:, i * S + o: i * S + o + XCH],
                              x8d[:, i * S + o: i * S + o + XCH])
    # per-partition bias vector for the fp8 exp (see below)
    nbias = const.tile([128, 1], F32, tag="nbias")
    nc.vector.memset(nbias[:], -2.0)
    # ones columns for V.T: only col 256 of each VW-chunk needs the 1.0
    # (on DVE: strided single-element writes, trivially cheap)
    nc.vector.memset(
        vt8_sb[:].rearrange("p (c w) -> p c w", w=VW)[:, :, DE:DE + 1], 1.0)
    nc.vector.memset(
        vt16_sb[:].rearrange("p (c w) -> p c w", w=VW)[:, :, DE:DE + 1], 1.0)

    # round-robin of PSUM->SBUF copy engines; ACT also runs all exps so it
    # gets a lighter share.
    cp_state = [0]

    def cp(dst, src):
        cp_state[0] += 1
        # the first ~12 copies happen before the exp stream ramps up, so
        # ACT can share them evenly; after that ACT is exp-bound and only
        # takes every 8th
        if (cp_state[0] % 2 == 0) if cp_state[0] <= 18 else (cp_state[0] % 32 == 0):
            nc.scalar.copy(dst, src)
        else:
            nc.vector.tensor_copy(dst, src)

    # ---- Q = Wq @ xq.T -> quantized q8 ----
    for d in range(2):
        ps = ps_st.tile([128, 1024], F32, tag="st", name="psq")
        for n in range(0, RPC, 512):
            for kd in range(2):
                nc.tensor.matmul(
                    ps[:, n:n + 512],
                    wq_sb[:, kd * DIM + d * 128: kd * DIM + d * 128 + 128],
                    xq_sb[kd][:, n:n + 512],
                    start=(kd == 0), stop=(kd == 1),
                )
        cp(q8_sb[:, d * RPC:(d + 1) * RPC], ps[:])
        cp(q16_sb[d][:], ps[:, 0:RCHUNK])

    # ---- filler closures, interleaved into the attention chunks so their
    # PSUM->SBUF copies (DVE/ACT) overlap the exp-bound phase: bf16 V.T
    # tiles 0..15 into vt16 (chunk-0 PV), fp8 DoubleRow vt8 (all 64
    # tiles), and fp8 K tiles 16..63.
    def _k16_group(d, p):
        # K tiles 0..15 in bf16 (x.T cols < 2048), kept in bf16 for
        # chunk-0's exact ST and gpsimd-quantized to k8 for the fp8 chunks
        def go():
            ps = ps_st.tile([128, 1024], F32, tag="st", name="psk")
            for nn in range(0, 1024, 512):
                for kd in range(2):
                    nc.tensor.matmul(
                        ps[:, nn:nn + 512],
                        wk_sb[:, kd * DIM + d * 128: kd * DIM + d * 128 + 128],
                        xt_sb[kd][:, p + nn:p + nn + 512],
                        start=(kd == 0), stop=(kd == 1),
                    )
            cp(k16_sb[d][:, p:p + 1024], ps[:])
            nc.gpsimd.tensor_copy(k8_sb[:, d * S + p:d * S + p + 1024],
                                  k16_sb[d][:, p:p + 1024])
        return go

    def _v16_group(g4):
        def go():
            ps = ps_st.tile([128, 1024], F32, tag="st", name="psv")
            for jj in range(4):
                j = g4 + jj
                for kd in range(2):
                    nc.tensor.matmul(
                        ps[:, jj * DE:(jj + 1) * DE],
                        xt_sb[kd][:, j * CT:(j + 1) * CT],
                        wv_sb[:, kd * DE:(kd + 1) * DE],
                        start=(kd == 0), stop=(kd == 1),
                    )
            vt_view = vt16_sb[:, g4 * VW:(g4 + 4) * VW].rearrange(
                "p (c w) -> p c w", w=VW)[:, :, 0:DE]
            cp(vt_view, ps[:].rearrange("p (c w) -> p c w", w=DE))
            nc.gpsimd.tensor_copy(vt8_sb[:, g4 * VW:(g4 + 4) * VW],
                                  vt16_sb[:, g4 * VW:(g4 + 4) * VW])
        return go

    def _v8_group(g4):
        def go():
            ps = ps_st.tile([128, 1024], F32, tag="st", name="psv8")
            for jj in range(4):
                j = g4 + jj
                nc.tensor.matmul(
                    ps[:, jj * DE:(jj + 1) * DE],
                    x8_3[:, :, j * CT:(j + 1) * CT],
                    wv8_3,
                    start=True, stop=True, perf_mode=DR,
                )
            vt_view = vt8_sb[:, g4 * VW:(g4 + 4) * VW].rearrange(
                "p (c w) -> p c w", w=VW)[:, :, 0:DE]
            cp(vt_view, ps[:].rearrange("p (c w) -> p c w", w=DE))
        return go

    def _k8_group(d, c0):
        def go():
            ps = ps_st.tile([128, 1024], F32, tag="st", name="psk8")
            for nn in range(0, 1024, 256):
                nc.tensor.matmul(
                    ps[:, nn:nn + 256],
                    wk8_3[:, :, d * 128:(d + 1) * 128],
                    x8_3[:, :, c0 + nn:c0 + nn + 256],
                    start=True, stop=True, perf_mode=DR,
                )
            cp(k8_sb[:, d * S + c0:d * S + c0 + 1024], ps[:])
        return go

    k16_fill = [(p, _k16_group(d, p))
                for p in range(0, BCOLS, 1024) for d in range(2)]
    v16_fill = [(g4, _v16_group(g4)) for g4 in range(0, BAND, 4)]
    v8_fill = [(g4, _v8_group(g4)) for g4 in range(BAND, NCT, 4)]
    k8_fill = [(c0, _k8_group(d, c0))
               for c0 in range(BCOLS, S, 1024) for d in range(2)]

    def ensure_k16(upto_col):
        while k16_fill and k16_fill[0][0] < upto_col:
            k16_fill.pop(0)[1]()

    def ensure_v16(upto_tile):
        while v16_fill and v16_fill[0][0] < upto_tile:
            v16_fill.pop(0)[1]()

    def ensure_v8(upto_tile):
        while v8_fill and v8_fill[0][0] < upto_tile:
            v8_fill.pop(0)[1]()

    def ensure_k8(upto_col):
        while k8_fill and k8_fill[0][0] < upto_col:
            k8_fill.pop(0)[1]()

    # proportional pacing: spread the filler groups over the 40 ST groups
    # so their PSUM->SBUF copies never pile up on DVE; vt16 first (chunk-0
    # PV drains earliest), then vt8 slightly ahead of k8 (PV trails ST)
    n_fill = (len(k16_fill) + len(v16_fill) + len(v8_fill)
              + len(k8_fill))
    pace = [0]

    def pace_fillers():
        pace[0] += 1
        target = n_fill * pace[0] // 40
        while (len(k16_fill) + len(v16_fill) + len(v8_fill)
               + len(k8_fill)) > n_fill - target:
            if k16_fill:
                k16_fill.pop(0)[1]()
            elif v16_fill:
                v16_fill.pop(0)[1]()
            elif v8_fill and (len(v8_fill) * 2 >= len(k8_fill) or not k8_fill):
                v8_fill.pop(0)[1]()
            elif k8_fill:
                k8_fill.pop(0)[1]()
            else:
                break

    # ---- attention: per row chunk, stream causal col tiles ----
    # col tiles in groups of G: one 2-bank PSUM tile holds G S.T tiles side
    # by side -> a single ACT exp (and a single band mask multiply) covers
    # the whole group, amortizing ACT overhead.
    #
    # software pipeline, depth 4, carried ACROSS chunk boundaries: PV for
    # group g is emitted after the ST matmuls of group g+4, and a chunk's
    # last PVs (plus its epilogue) drain while the next chunk's ST/exp
    # stream is already running.
    G = 4
    from collections import deque
    pending = deque()  # (emit_pv_fn, tail_fn_or_None)

    def drain_one():
        fn, tail = pending.popleft()
        fn()
        if tail is not None:
            tail()

    for r in range(NRC):
        fp8 = r > 0
        ncols = BAND * (r + 1)
        # h=0 (rows 0..127) accumulation ends at col tile 16r+7 (later
        # tiles are fully masked there); h=1 runs to the last tile.
        last_j = {0: min(BAND * r + 7, ncols - 1), 1: ncols - 1}
        pvbox = []

        def get_pv(pvbox=pvbox):
            # lazy: allocated at the first PV drain, which happens after
            # the previous chunk's epilogue has been emitted (bufs=1 ring)
            if not pvbox:
                pvbox.append([
                    ps_pv.tile([128, VW], F32, tag=f"pv{h}", name=f"pv{h}")
                    for h in range(2)
                ])
            return pvbox[0]

        if fp8:
            # DoubleRow PV over col-tile pairs; moving dim split at 128.
            def emit_pv(pt, g, u0, get_pv=get_pv, last_j=last_j):
                ensure_v16(min(g + G, BAND))
                ensure_v8(g + G)
                pv = get_pv()
                pt3 = pt[:].rearrange("p (c u) -> p c u", u=RCHUNK)
                for t in range(0, G, 2):
                    j = g + t
                    lhs = pt3[:, t:t + 2]  # [128, 2, 256] both tiles
                    rhs = vt8_sb[:, j * VW:(j + 2) * VW].rearrange(
                        "p (c w) -> p c w", w=VW)
                    for h in ((1,) if u0 else (0, 1)):
                        # PSUM start=1 marks the whole 2KB zero region
                        # pending-zero; each write to a pending byte zeroes
                        # then writes. So ONLY the very first inst starts:
                        # the w1 split's first write rides the same mark. A
                        # second start would re-mark w0's bytes and wipe its
                        # pair-0 contribution on the next accumulation.
                        first = (j == 0)
                        last = (j + 1 == last_j[h])
                        nc.tensor.matmul(
                            pv[h][:, 0:128],
                            lhs[:, :, h * 128:h * 128 + 128],
                            rhs[:, :, 0:128],
                            start=first, stop=False, perf_mode=DR,
                            skip_group_check=True,
                        )
                        nc.tensor.matmul(
                            pv[h][:, 128:VW],
                            lhs[:, :, h * 128:h * 128 + 128],
                            rhs[:, :, 128:VW],
                            start=False, stop=last, perf_mode=DR,
                            skip_group_check=True,
                        )
        else:
            def emit_pv(pt, g, u0, get_pv=get_pv, last_j=last_j):
                ensure_v16(g + G)
                pv = get_pv()
                for t in range(G):
                    j = g + t
                    for h in ((1,) if u0 else (0, 1)):
                        nc.tensor.matmul(
                            pv[h][:],
                            pt[:, t * RCHUNK + h * 128: t * RCHUNK + h * 128 + 128],
                            vt16_sb[:, j * VW:(j + 1) * VW],
                            start=(j == 0), stop=(j == last_j[h]),
                        )

        def epilogue(get_pv=get_pv, r=r):
            pv = get_pv()
            if dbg is not None and r == 1:
                for h in range(2):
                    tl = const.tile([128, VW], F32, tag=f"pv{h}_sb",
                                    name=f"pv{h}_sb")
                    dbg[f"pv{h}_sb"] = tl
                    nc.scalar.copy(tl[:], pv[h][:])
            for h in range(2):
                linv = ep_pool.tile([128, 1], F32, tag="linv")
                nc.vector.reciprocal(linv[:], pv[h][:, DE:DE + 1])
                osb = ep_pool.tile([128, DE], F32, tag="osb")
                nc.vector.tensor_scalar_mul(osb[:], pv[h][:, 0:DE], linv[:])
                rows = r * RCHUNK + h * 128
                nc.sync.dma_start(outd[rows:rows + 128, :], osb[:])

        for g in range(0, ncols, G):
            gb = g - BAND * r
            if r == 0:
                ensure_k16(CT * (g + G))
            else:
                ensure_k8(CT * (g + G))
            # when every col tile in the group has j' >= 8, rows 0..127
            # of the chunk are entirely non-causal: compute only the
            # high 128 rows (u0=128) and skip the h=0 PV matmuls.
            u0 = 128 if gb >= 8 else 0
            st = ps_st.tile([128, G * RCHUNK], F32, tag="st")
            for t in range(G):
                j = g + t
                if fp8:
                    nc.tensor.matmul(
                        st[:, t * RCHUNK + u0:(t + 1) * RCHUNK],
                        k8_3[:, :, j * CT:(j + 1) * CT],
                        q8_3[:, :, r * RCHUNK + u0:(r + 1) * RCHUNK],
                        start=True, stop=True, perf_mode=DR,
                    )
                else:
                    # chunk 0 holds the few-key causal rows, most sensitive
                    # to score noise: exact bf16 K/Q there
                    for kd in range(2):
                        nc.tensor.matmul(
                            st[:, t * RCHUNK + u0:(t + 1) * RCHUNK],
                            k16_sb[kd][:, j * CT:(j + 1) * CT],
                            q16_sb[kd][:, u0:RCHUNK],
                            start=(kd == 0), stop=(kd == 1),
                        )
            pace_fillers()
            # ramp the pipeline down toward the end of the last chunk so
            # the tail drain after the final ST group is short
            depth = 6 if not (r == 3 and g >= ncols - 4 * G) else 3
            while len(pending) >= depth:
                drain_one()
            pdt = F8 if fp8 else BF16
            msk = m8_sb if fp8 else m16_sb
            pt = pt_pool.tile([128, G * RCHUNK], pdt, tag="pt8" if fp8 else "pt16")
            # fp8 chunks store P' = exp(s/16 - 2): score outliers reach
            # ~16*6 (heavy |q||k| tails), and exp would overflow fp8e4's
            # 240 max -> inf -> NaN after the 0-mask. The constant bias
            # cancels exactly in the row-sum normalization.
            bias = nbias[:] if fp8 else 0.0
            if u0:
                st_v = st[:].rearrange("p (c w) -> p c w", w=RCHUNK)[:, :, u0:]
                pt_v = pt[:].rearrange("p (c w) -> p c w", w=RCHUNK)[:, :, u0:]
                nc.scalar.activation(
                    pt_v, st_v, mybir.ActivationFunctionType.Exp,
                    scale=0.0625, bias=bias,
                )
            else:
                nc.scalar.activation(
                    pt[:], st[:], mybir.ActivationFunctionType.Exp,
                    scale=0.0625, bias=bias,
                )
            if gb >= 0:
                # diagonal band: only the 128-row window starting at u0 of
                # each tile mixes causal/non-causal entries (rows below are
                # never read thanks to the u0 skip, rows above are fully
                # causal), so the 0/1 mask covers just that window.
                pt_w = pt[:].rearrange(
                    "p (c w) -> p c w", w=RCHUNK)[:, :, u0:u0 + 128]
                mk_w = msk[:, gb * 128:(gb + G) * 128].rearrange(
                    "p (c w) -> p c w", w=128)
                _mask_mul(nc, r, gb, pt_w, mk_w)
            if dbg is not None and r == 1 and g in (0, 16, 24):
                key = {0: "ptA_sb", 16: "ptB_sb", 24: "ptC_sb"}[g]
                tl = const.tile([128, 1024], F8, tag=key, name=key)
                dbg[key] = tl
                nc.gpsimd.tensor_copy(tl[:], pt[:])
            is_last = (g + G >= ncols)
            pending.append((
                lambda pt=pt, g=g, u0=u0, f=emit_pv: f(pt, g, u0),
                epilogue if is_last else None,
            ))
    while pending:
        drain_one()
    if dbg is not None:
        nc.sync.dma_start(dbg["ptA"][:, :], dbg["ptA_sb"][:])
        nc.sync.dma_start(dbg["ptB"][:, :], dbg["ptB_sb"][:])
        nc.sync.dma_start(dbg["ptC"][:, :], dbg["ptC_sb"][:])
        nc.sync.dma_start(dbg["pv0"][:, :], dbg["pv0_sb"][:])
        nc.sync.dma_start(dbg["pv1"][:, :], dbg["pv1_sb"][:])
        nc.sync.dma_start(dbg["k8o"][:, :], k8_sb[:])
        nc.sync.dma_start(dbg["q8o"][:, :], q8_sb[:])
        nc.sync.dma_start(dbg["vt8o"][:, :], vt8_sb[:])
        nc.sync.dma_start(dbg["m8o"][:, :], m8_sb[:])


def _mask_mul(nc, r, gb, pt_v, mk_v):
    # chunks 1-2 run while DVE is saturated with K/V copies -> gpsimd,
    # except the last band groups (gb >= 8) whose PVs drain into the next
    # chunk: the slow gpsimd there would stall the in-order PE at the
    # boundary. chunk 0 (bf16, 2x mode) and chunk 3 (DVE idle) -> DVE.
    if r in (1, 2, 3) and gb < 8:
        nc.gpsimd.tensor_mul(pt_v, pt_v, mk_v)
    else:
        nc.vector.tensor_mul(pt_v, pt_v, mk_v)


def _host_inputs(x, Wq, Wk, Wv):
    xT = np.ascontiguousarray(x.T)                       # [256, 8192] f32
    x8 = np.ascontiguousarray(
        xT.reshape(2, 128, S).transpose(1, 0, 2).reshape(128, 2 * S)
    ).astype(NPF8)
    xT16 = np.ascontiguousarray(xT[:, :BCOLS]).astype(NPBF16).reshape(2, 128, BCOLS)
    wqb = np.ascontiguousarray(Wq.T).astype(NPBF16).reshape(2, 128, DIM)
    wkb = np.ascontiguousarray(Wk.T).astype(NPBF16).reshape(2, 128, DIM)
    wvb = np.ascontiguousarray(Wv.T).astype(NPBF16).reshape(2, 128, DE)
    wk8 = np.ascontiguousarray(
        Wk.T.reshape(2, 128, DIM).transpose(1, 0, 2).reshape(128, 2 * DIM)
    ).astype(NPF8)
    wv8 = np.ascontiguousarray(
        Wv.T.reshape(2, 128, DE).transpose(1, 0, 2).reshape(128, 2 * DE)
    ).astype(NPF8)
    k_idx = np.arange(128)[:, None, None]
    jp = np.arange(BAND)[None, :, None]
    u = 128 * (jp >= 8) + np.arange(128)[None, None, :]
    in_maps = []
    for c in range(NCORES):
        xq = np.ascontiguousarray(x[c::NCORES].T).astype(NPBF16).reshape(2, 128, RPC)
        m = (128 * jp + k_idx <= 8 * u + c)
        m = np.ascontiguousarray(m.reshape(128, BAND * 128))
        in_maps.append({
            "x8": x8, "xT": xT16, "xqT": xq, "wqT": wqb, "wkT": wkb,
            "wvT": wvb, "wk8": wk8, "wv8": wv8,
            "m16": m.astype(NPBF16), "m8": m.astype(NPF8),
        })
    return in_maps


def kernel(x, Wq, Wk, Wv, _trace=False, _trace_kwargs=None):
    if "nc" not in _cached:
        _cached["nc"] = _build_nc()
    nc = _cached["nc"]
    in_maps = _host_inputs(
        np.asarray(x, np.float32), np.asarray(Wq, np.float32),
        np.asarray(Wk, np.float32), np.asarray(Wv, np.float32),
    )
    kw = dict(_trace_kwargs or {})
    res = run_bass_kernel_spmd(
        nc, in_maps, core_ids=list(range(NCORES)), trace=_trace, **kw
    )
    out = np.empty((S, DE), np.float32)
    for c in range(NCORES):
        out[c::NCORES] = res.results[c]["out"]
    _cached["last_results"] = res
    return out


# revision 48
# speedup vs baseline: 1.0260x; 1.0260x over previous
"""Causal attention head (S=8192, De=dim=256) on 8 trn2 NeuronCores.

Math (reference):
    Q = Wq @ x.T; K = Wk @ x.T; V = Wv @ x.T
    S = (Q.T @ K) / sqrt(256); causal mask (upper tri -> -inf)
    out = softmax(S, axis=1) @ V.T          # [8192, 256]

Sharding: core c owns rows c::8 (stride-8 interleave) -> every core's
row block has a near-identical causal prefix profile, so the SPMD kernel
is identical across cores; all per-core variation is input data.

Per-core kernel, fp8e4 DoubleRow edition. The PE's fp8 DoubleRow mode
contracts 2x128 partitions per pass at 0.5 cycles/row (4x bf16 for a
256-deep contraction), so the large matmuls run in fp8; the few-key
early causal rows (chunk 0 = local rows 0..255) keep an exact bf16
score+PV path since score noise doesn't average out there.
  - K: col tiles 0..15 bf16-generated (kept as k16 for chunk-0 ST,
    gpsimd-quantized into k8), tiles 16..63 fp8 DoubleRow into k8.
  - V: tiles 0..15 bf16 into vt16 (chunk-0 PV) + gpsimd-converted into
    vt8; tiles 16..63 fp8 DoubleRow straight into vt8.
  - Q bf16 -> q8 (all rows) and q16 (chunk-0 rows).
  - scores: S.T tile = K_j.T @ Q_r as ONE DoubleRow inst per 128-col
    tile for chunks 1-3, bf16 two-inst contraction for chunk 0.
  - exp on ACT: fp8 out with exponent bias -2 for chunks 1-3 (score
    outliers reach ~95, exp(s/16) would overflow fp8e4's 240 max; the
    bias cancels in the row-sum normalization), bf16 out for chunk 0.
  - causal band masks: 0/1 multiply over just the 128-row mixed window
    per tile; gpsimd for early band groups, DVE (2x bf16) otherwise.
  - PV: chunk 0 in bf16 (exact V), chunks 1-3 DoubleRow over col-tile
    pairs, moving dim split (0:128 | 128:257) to stay under the 512
    moving-row ISA limit; ones column yields row sums for free.
  - K/V generation is emitted as "filler" groups interleaved into the
    attention chunks (just-in-time via ensure_*), so their PSUM->SBUF
    copies overlap the exp-bound phase; the softmax pipeline is 6 deep
    and carried across chunk boundaries.
No softmax max-subtraction needed in fp32: |scores/16| <= ~6.
"""

import sys

sys.path.insert(0, "/opt/trn_rl_repo")

from contextlib import ExitStack

import ml_dtypes
import numpy as np

import concourse.bass as bass
import concourse.mybir as mybir
import concourse.tile as tile
from concourse import bacc
from concourse.bass_utils import run_bass_kernel_spmd

BF16 = mybir.dt.bfloat16
F8 = mybir.dt.float8e4
F32 = mybir.dt.float32
NPBF16 = ml_dtypes.bfloat16
NPF8 = ml_dtypes.float8_e4m3
DR = mybir.MatmulPerfMode.DoubleRow

S, DIM, DE = 8192, 256, 256
NCORES = 8
RPC = S // NCORES          # 1024 rows per core
RCHUNK = 256               # rows per S.T matmul (moving free dim)
NRC = RPC // RCHUNK        # 4 row chunks per core
CT = 128                   # col tile (PE partition)
NCT = S // CT              # 64 col tiles total
BAND = 2048 // CT          # 16 col tiles per causal band of a row chunk
VW = DE + 1                # V.T chunk width incl. ones column
BCOLS = 2048               # cols covered by the bf16 K/V path (tiles < 16)

_cached = {}


def _build_nc(repeat=0, debug=False):
    nc = bacc.Bacc("TRN2", target_bir_lowering=False, debug=False,
                   num_devices=NCORES)
    x8d = nc.dram_tensor("x8", [128, 2 * S], F8, kind="ExternalInput")
    xTd = nc.dram_tensor("xT", [2, 128, BCOLS], BF16, kind="ExternalInput")
    xqT = nc.dram_tensor("xqT", [2, 128, RPC], BF16, kind="ExternalInput")
    wqT = nc.dram_tensor("wqT", [2, 128, DIM], BF16, kind="ExternalInput")
    wkT = nc.dram_tensor("wkT", [2, 128, DIM], BF16, kind="ExternalInput")
    wvT = nc.dram_tensor("wvT", [2, 128, DE], BF16, kind="ExternalInput")
    wk8d = nc.dram_tensor("wk8", [128, 2 * DIM], F8, kind="ExternalInput")
    wv8d = nc.dram_tensor("wv8", [128, 2 * DE], F8, kind="ExternalInput")
    m16d = nc.dram_tensor("m16", [128, BAND * 128], BF16, kind="ExternalInput")
    m8d = nc.dram_tensor("m8", [128, BAND * 128], F8, kind="ExternalInput")
    outd = nc.dram_tensor("out", [RPC, DE], F32, kind="ExternalOutput")
    dbg = None
    if debug:
        dbg = {
            "k8o": nc.dram_tensor("k8o", [128, 2 * S], F8, kind="ExternalOutput"),
            "q8o": nc.dram_tensor("q8o", [128, 2 * RPC], F8, kind="ExternalOutput"),
            "vt8o": nc.dram_tensor("vt8o", [128, NCT * VW], F8, kind="ExternalOutput"),
            "m8o": nc.dram_tensor("m8o", [128, BAND * 128], F8, kind="ExternalOutput"),
            "ptA": nc.dram_tensor("ptA", [128, 1024], F8, kind="ExternalOutput"),
            "ptB": nc.dram_tensor("ptB", [128, 1024], F8, kind="ExternalOutput"),
            "ptC": nc.dram_tensor("ptC", [128, 1024], F8, kind="ExternalOutput"),
            "pv0": nc.dram_tensor("pv0", [128, VW], F32, kind="ExternalOutput"),
            "pv1": nc.dram_tensor("pv1", [128, VW], F32, kind="ExternalOutput"),
        }

    with tile.TileContext(nc) as tc, ExitStack() as ctx:
        const = ctx.enter_context(tc.tile_pool(name="const", bufs=1))
        ps_st = ctx.enter_context(tc.tile_pool(name="ps_st", bufs=3, space="PSUM"))
        ps_pv = ctx.enter_context(tc.tile_pool(name="ps_pv", bufs=1, space="PSUM"))
        pt_pool = ctx.enter_context(tc.tile_pool(name="pt", bufs=9))
        ep_pool = ctx.enter_context(tc.tile_pool(name="ep", bufs=4))

        def body(_iv=None):
            _emit(nc, tc, const, ps_st, ps_pv, pt_pool, ep_pool,
                  x8d, xTd, xqT, wqT, wkT, wvT, wk8d, wv8d, m16d, m8d, outd,
                  dbg)

        if repeat:
            with tc.For_i(0, repeat, 1) as _iv:
                body(_iv)
        else:
            body()

    nc.compile()
    return nc


def _emit(nc, tc, const, ps_st, ps_pv, pt_pool, ep_pool,
          x8d, xTd, xqT, wqT, wkT, wvT, wk8d, wv8d, m16d, m8d, outd,
          dbg=None):
    # ---- constants / staged inputs in SBUF ----
    x8_sb = const.tile([128, 2 * S], F8, tag="x8")
    xt_sb = [const.tile([128, BCOLS], BF16, tag=f"xt{i}", name=f"xt{i}") for i in range(2)]
    xq_sb = [const.tile([128, RPC], BF16, tag=f"xq{i}", name=f"xq{i}") for i in range(2)]
    wq_sb = const.tile([128, 2 * DIM], BF16, tag="wq")
    wk_sb = const.tile([128, 2 * DIM], BF16, tag="wk")
    wv_sb = const.tile([128, 2 * DE], BF16, tag="wv")
    wk8_sb = const.tile([128, 2 * DIM], F8, tag="wk8")
    wv8_sb = const.tile([128, 2 * DE], F8, tag="wv8")
    m16_sb = const.tile([128, BAND * 128], BF16, tag="m16")
    m8_sb = const.tile([128, BAND * 128], F8, tag="m8")
    k8_sb = const.tile([128, 2 * S], F8, tag="k8")
    q8_sb = const.tile([128, 2 * RPC], F8, tag="q8")
    k16_sb = [const.tile([128, BCOLS], BF16, tag=f"k16_{i}", name=f"k16_{i}")
              for i in range(2)]
    q16_sb = [const.tile([128, RCHUNK], BF16, tag=f"q16_{i}", name=f"q16_{i}")
              for i in range(2)]
    vt8_sb = const.tile([128, NCT * VW], F8, tag="vt8")
    vt16_sb = const.tile([128, BAND * VW], BF16, tag="vt16")

    x8_3 = x8_sb[:].rearrange("p (i c) -> p i c", i=2)     # [128,2,S]
    wk8_3 = wk8_sb[:].rearrange("p (i d) -> p i d", i=2)   # [128,2,256]
    wv8_3 = wv8_sb[:].rearrange("p (i e) -> p i e", i=2)   # [128,2,256]
    k8_3 = k8_sb[:].rearrange("p (i c) -> p i c", i=2)     # [128,2,S]
    q8_3 = q8_sb[:].rearrange("p (i r) -> p i r", i=2)     # [128,2,RPC]

    # input staging: weights ride the otherwise-idle DVE queue, x.T (bf16)
    # in fine chunks on ACT (K-bf16 consumes it first), x8 immediately on
    # SP in consumption order, xq+masks via gpsimd SWDGE.
    for i in range(2):
        nc.gpsimd.dma_start(xq_sb[i][:], xqT[i, :, :])
        nc.gpsimd.dma_start(wq_sb[:, i * DIM:(i + 1) * DIM], wqT[i, :, :])
    for i in range(2):
        nc.gpsimd.dma_start(wk_sb[:, i * DIM:(i + 1) * DIM], wkT[i, :, :])
        nc.gpsimd.dma_start(wv_sb[:, i * DE:(i + 1) * DE], wvT[i, :, :])
    nc.gpsimd.dma_start(wv8_sb[:], wv8d[:, :])
    nc.gpsimd.dma_start(wk8_sb[:], wk8d[:, :])
    nc.gpsimd.dma_start(m8_sb[:], m8d[:, :])
    nc.gpsimd.dma_start(m16_sb[:], m16d[:, :])
    for o in range(0, BCOLS, 1024):
        for i in range(2):
            nc.scalar.dma_start(xt_sb[i][:, o:o + 1024], xTd[i, :, o:o + 1024])
    XCH = 2048
    for o in range(0, S, XCH):
        for i in range(2):
            nc.sync.dma_start(x8_sb[:, i * S + o: i * S + o + XCH],
                              x8d[:, i * S + o: i * S + o + XCH])
    # per-partition bias vector for the fp8 exp (see below)
    nbias = const.tile([128, 1], F32, tag="nbias")
    nc.vector.memset(nbias[:], -2.0)
    # dummy activation right away: pulls the 1.3us Exp table load into the
    # initial DMA-wait window instead of the first real exp
    warm = const.tile([128, 1], F32, tag="warm")
    nc.scalar.activation(warm[:], nbias[:],
                         mybir.ActivationFunctionType.Exp)
    # ones columns for V.T: only col 256 of each VW-chunk needs the 1.0
    # (on DVE: strided single-element writes, trivially cheap)
    nc.vector.memset(
        vt8_sb[:].rearrange("p (c w) -> p c w", w=VW)[:, :, DE:DE + 1], 1.0)
    nc.vector.memset(
        vt16_sb[:].rearrange("p (c w) -> p c w", w=VW)[:, :, DE:DE + 1], 1.0)

    # round-robin of PSUM->SBUF copy engines; ACT also runs all exps so it
    # gets a lighter share.
    cp_state = [0]

    def cp(dst, src):
        cp_state[0] += 1
        # the first ~12 copies happen before the exp stream ramps up, so
        # ACT can share them evenly; after that ACT is exp-bound and only
        # takes every 8th
        if cp_state[0] <= 18 and cp_state[0] % 2 == 0:
            nc.scalar.copy(dst, src)
        else:
            nc.vector.tensor_copy(dst, src)

    # ---- Q = Wq @ xq.T -> quantized q8 ----
    for d in range(2):
        ps = ps_st.tile([128, 1024], F32, tag="st", name="psq")
        for n in range(0, RPC, 512):
            for kd in range(2):
                nc.tensor.matmul(
                    ps[:, n:n + 512],
                    wq_sb[:, kd * DIM + d * 128: kd * DIM + d * 128 + 128],
                    xq_sb[kd][:, n:n + 512],
                    start=(kd == 0), stop=(kd == 1),
                )
        cp(q8_sb[:, d * RPC:(d + 1) * RPC], ps[:])
        cp(q16_sb[d][:], ps[:, 0:RCHUNK])

    # ---- filler closures, interleaved into the attention chunks so their
    # PSUM->SBUF copies (DVE/ACT) overlap the exp-bound phase: bf16 V.T
    # tiles 0..15 into vt16 (chunk-0 PV), fp8 DoubleRow vt8 (all 64
    # tiles), and fp8 K tiles 16..63.
    def _k16_group(d, p):
        # K tiles 0..15 in bf16 (x.T cols < 2048), kept in bf16 for
        # chunk-0's exact ST and gpsimd-quantized to k8 for the fp8 chunks
        def go():
            ps = ps_st.tile([128, 1024], F32, tag="st", name="psk")
            for nn in range(0, 1024, 512):
                for kd in range(2):
                    nc.tensor.matmul(
                        ps[:, nn:nn + 512],
                        wk_sb[:, kd * DIM + d * 128: kd * DIM + d * 128 + 128],
                        xt_sb[kd][:, p + nn:p + nn + 512],
                        start=(kd == 0), stop=(kd == 1),
                    )
            cp(k16_sb[d][:, p:p + 1024], ps[:])
            nc.gpsimd.tensor_copy(k8_sb[:, d * S + p:d * S + p + 1024],
                                  k16_sb[d][:, p:p + 1024])
        return go

    def _v16_group(g4):
        def go():
            ps = ps_st.tile([128, 1024], F32, tag="st", name="psv")
            for jj in range(4):
                j = g4 + jj
                for kd in range(2):
                    nc.tensor.matmul(
                        ps[:, jj * DE:(jj + 1) * DE],
                        xt_sb[kd][:, j * CT:(j + 1) * CT],
                        wv_sb[:, kd * DE:(kd + 1) * DE],
                        start=(kd == 0), stop=(kd == 1),
                    )
            vt_view = vt16_sb[:, g4 * VW:(g4 + 4) * VW].rearrange(
                "p (c w) -> p c w", w=VW)[:, :, 0:DE]
            cp(vt_view, ps[:].rearrange("p (c w) -> p c w", w=DE))
            nc.gpsimd.tensor_copy(vt8_sb[:, g4 * VW:(g4 + 4) * VW],
                                  vt16_sb[:, g4 * VW:(g4 + 4) * VW])
        return go

    def _v8_group(g4):
        def go():
            ps = ps_st.tile([128, 1024], F32, tag="st", name="psv8")
            for jj in range(4):
                j = g4 + jj
                nc.tensor.matmul(
                    ps[:, jj * DE:(jj + 1) * DE],
                    x8_3[:, :, j * CT:(j + 1) * CT],
                    wv8_3,
                    start=True, stop=True, perf_mode=DR,
                )
            vt_view = vt8_sb[:, g4 * VW:(g4 + 4) * VW].rearrange(
                "p (c w) -> p c w", w=VW)[:, :, 0:DE]
            cp(vt_view, ps[:].rearrange("p (c w) -> p c w", w=DE))
        return go

    def _k8_group(d, c0):
        def go():
            ps = ps_st.tile([128, 1024], F32, tag="st", name="psk8")
            for nn in range(0, 1024, 256):
                nc.tensor.matmul(
                    ps[:, nn:nn + 256],
                    wk8_3[:, :, d * 128:(d + 1) * 128],
                    x8_3[:, :, c0 + nn:c0 + nn + 256],
                    start=True, stop=True, perf_mode=DR,
                )
            cp(k8_sb[:, d * S + c0:d * S + c0 + 1024], ps[:])
        return go

    k16_fill = [(p, _k16_group(d, p))
                for p in range(0, BCOLS, 1024) for d in range(2)]
    v16_fill = [(g4, _v16_group(g4)) for g4 in range(0, BAND, 4)]
    v8_fill = [(g4, _v8_group(g4)) for g4 in range(BAND, NCT, 4)]
    k8_fill = [(c0, _k8_group(d, c0))
               for c0 in range(BCOLS, S, 1024) for d in range(2)]

    def ensure_k16(upto_col):
        while k16_fill and k16_fill[0][0] < upto_col:
            k16_fill.pop(0)[1]()

    def ensure_v16(upto_tile):
        while v16_fill and v16_fill[0][0] < upto_tile:
            v16_fill.pop(0)[1]()

    def ensure_v8(upto_tile):
        while v8_fill and v8_fill[0][0] < upto_tile:
            v8_fill.pop(0)[1]()

    def ensure_k8(upto_col):
        while k8_fill and k8_fill[0][0] < upto_col:
            k8_fill.pop(0)[1]()

    # proportional pacing: spread the filler groups over the 40 ST groups
    # so their PSUM->SBUF copies never pile up on DVE; vt16 first (chunk-0
    # PV drains earliest), then vt8 slightly ahead of k8 (PV trails ST)
    n_fill = (len(k16_fill) + len(v16_fill) + len(v8_fill)
              + len(k8_fill))
    pace = [0]

    def pace_fillers():
        pace[0] += 1
        target = n_fill * pace[0] // 40
        while (len(k16_fill) + len(v16_fill) + len(v8_fill)
               + len(k8_fill)) > n_fill - target:
            if k16_fill:
                k16_fill.pop(0)[1]()
            elif v16_fill:
                v16_fill.pop(0)[1]()
            elif v8_fill and (len(v8_fill) * 2 >= len(k8_fill) or not k8_fill):
                v8_fill.pop(0)[1]()
            elif k8_fill:
                k8_fill.pop(0)[1]()
            else:
                break

    # ---- attention: per row chunk, stream causal col tiles ----
    # col tiles in groups of G: one 2-bank PSUM tile holds G S.T tiles side
    # by side -> a single ACT exp (and a single band mask multiply) covers
    # the whole group, amortizing ACT overhead.
    #
    # software pipeline, depth 4, carried ACROSS chunk boundaries: PV for
    # group g is emitted after the ST matmuls of group g+4, and a chunk's
    # last PVs (plus its epilogue) drain while the next chunk's ST/exp
    # stream is already running.
    G = 4
    from collections import deque
    pending = deque()  # (emit_pv_fn, tail_fn_or_None)

    def drain_one():
        fn, tail = pending.popleft()
        fn()
        if tail is not None:
            tail()

    for r in range(NRC):
        fp8 = r > 0
        ncols = BAND * (r + 1)
        # h=0 (rows 0..127) accumulation ends at col tile 16r+7 (later
        # tiles are fully masked there); h=1 runs to the last tile.
        last_j = {0: min(BAND * r + 7, ncols - 1), 1: ncols - 1}
        pvbox = []

        def get_pv(pvbox=pvbox):
            # lazy: allocated at the first PV drain, which happens after
            # the previous chunk's epilogue has been emitted (bufs=1 ring)
            if not pvbox:
                pvbox.append([
                    ps_pv.tile([128, VW], F32, tag=f"pv{h}", name=f"pv{h}")
                    for h in range(2)
                ])
            return pvbox[0]

        if fp8:
            # DoubleRow PV over col-tile pairs; moving dim split at 128.
            def emit_pv(pt, g, u0, get_pv=get_pv, last_j=last_j):
                ensure_v16(min(g + G, BAND))
                ensure_v8(g + 3 * G)
                pv = get_pv()
                pt3 = pt[:].rearrange("p (c u) -> p c u", u=RCHUNK)
                for t in range(0, G, 2):
                    j = g + t
                    lhs = pt3[:, t:t + 2]  # [128, 2, 256] both tiles
                    rhs = vt8_sb[:, j * VW:(j + 2) * VW].rearrange(
                        "p (c w) -> p c w", w=VW)
                    for h in ((1,) if u0 else (0, 1)):
                        # PSUM start=1 marks the whole 2KB zero region
                        # pending-zero; each write to a pending byte zeroes
                        # then writes. So ONLY the very first inst starts:
                        # the w1 split's first write rides the same mark. A
                        # second start would re-mark w0's bytes and wipe its
                        # pair-0 contribution on the next accumulation.
                        first = (j == 0)
                        last = (j + 1 == last_j[h])
                        nc.tensor.matmul(
                            pv[h][:, 0:128],
                            lhs[:, :, h * 128:h * 128 + 128],
                            rhs[:, :, 0:128],
                            start=first, stop=False, perf_mode=DR,
                            skip_group_check=True,
                        )
                        nc.tensor.matmul(
                            pv[h][:, 128:VW],
                            lhs[:, :, h * 128:h * 128 + 128],
                            rhs[:, :, 128:VW],
                            start=False, stop=last, perf_mode=DR,
                            skip_group_check=True,
                        )
        else:
            def emit_pv(pt, g, u0, get_pv=get_pv, last_j=last_j):
                ensure_v16(g + G)
                pv = get_pv()
                for t in range(G):
                    j = g + t
                    for h in ((1,) if u0 else (0, 1)):
                        nc.tensor.matmul(
                            pv[h][:],
                            pt[:, t * RCHUNK + h * 128: t * RCHUNK + h * 128 + 128],
                            vt16_sb[:, j * VW:(j + 1) * VW],
                            start=(j == 0), stop=(j == last_j[h]),
                        )

        def epilogue(get_pv=get_pv, r=r):
            pv = get_pv()
            if dbg is not None and r == 1:
                for h in range(2):
                    tl = const.tile([128, VW], F32, tag=f"pv{h}_sb",
                                    name=f"pv{h}_sb")
                    dbg[f"pv{h}_sb"] = tl
                    nc.scalar.copy(tl[:], pv[h][:])
            for h in range(2):
                linv = ep_pool.tile([128, 1], F32, tag="linv")
                nc.vector.reciprocal(linv[:], pv[h][:, DE:DE + 1])
                osb = ep_pool.tile([128, DE], F32, tag="osb")
                nc.vector.tensor_scalar_mul(osb[:], pv[h][:, 0:DE], linv[:])
                rows = r * RCHUNK + h * 128
                nc.sync.dma_start(outd[rows:rows + 128, :], osb[:])

        for g in range(0, ncols, G):
            gb = g - BAND * r
            if r == 0:
                ensure_k16(CT * (g + G))
            else:
                # prefetch margin: the filler's PSUM->SBUF copy takes ~1.2us,
                # so pull k8 coverage ~2 groups ahead of the ST that reads it
                ensure_k8(CT * (g + 3 * G))
            # when every col tile in the group has j' >= 8, rows 0..127
            # of the chunk are entirely non-causal: compute only the
            # high 128 rows (u0=128) and skip the h=0 PV matmuls.
            u0 = 128 if gb >= 8 else 0
            st = ps_st.tile([128, G * RCHUNK], F32, tag="st")
            for t in range(G):
                j = g + t
                if fp8:
                    nc.tensor.matmul(
                        st[:, t * RCHUNK + u0:(t + 1) * RCHUNK],
                        k8_3[:, :, j * CT:(j + 1) * CT],
                        q8_3[:, :, r * RCHUNK + u0:(r + 1) * RCHUNK],
                        start=True, stop=True, perf_mode=DR,
                    )
                else:
                    # chunk 0 holds the few-key causal rows, most sensitive
                    # to score noise: exact bf16 K/Q there
                    for kd in range(2):
                        nc.tensor.matmul(
                            st[:, t * RCHUNK + u0:(t + 1) * RCHUNK],
                            k16_sb[kd][:, j * CT:(j + 1) * CT],
                            q16_sb[kd][:, u0:RCHUNK],
                            start=(kd == 0), stop=(kd == 1),
                        )
            pace_fillers()
            # ramp the pipeline down toward the end of the last chunk so
            # the tail drain after the final ST group is short
            depth = 6 if not (r == 3 and g >= ncols - 4 * G) else 3
            while len(pending) >= depth:
                drain_one()
            pdt = F8 if fp8 else BF16
            msk = m8_sb if fp8 else m16_sb
            pt = pt_pool.tile([128, G * RCHUNK], pdt, tag="pt8" if fp8 else "pt16")
            # fp8 chunks store P' = exp(s/16 - 2): score outliers reach
            # ~16*6 (heavy |q||k| tails), and exp would overflow fp8e4's
            # 240 max -> inf -> NaN after the 0-mask. The constant bias
            # cancels exactly in the row-sum normalization.
            bias = nbias[:] if fp8 else 0.0
            if u0:
                st_v = st[:].rearrange("p (c w) -> p c w", w=RCHUNK)[:, :, u0:]
                pt_v = pt[:].rearrange("p (c w) -> p c w", w=RCHUNK)[:, :, u0:]
                nc.scalar.activation(
                    pt_v, st_v, mybir.ActivationFunctionType.Exp,
                    scale=0.0625, bias=bias,
                )
            else:
                nc.scalar.activation(
                    pt[:], st[:], mybir.ActivationFunctionType.Exp,
                    scale=0.0625, bias=bias,
                )
            if gb >= 0:
                # diagonal band: only the 128-row window starting at u0 of
                # each tile mixes causal/non-causal entries (rows below are
                # never read thanks to the u0 skip, rows above are fully
                # causal), so the 0/1 mask covers just that window.
                pt_w = pt[:].rearrange(
                    "p (c w) -> p c w", w=RCHUNK)[:, :, u0:u0 + 128]
                mk_w = msk[:, gb * 128:(gb + G) * 128].rearrange(
                    "p (c w) -> p c w", w=128)
                _mask_mul(nc, r, gb, pt_w, mk_w)
            if dbg is not None and r == 1 and g in (0, 16, 24):
                key = {0: "ptA_sb", 16: "ptB_sb", 24: "ptC_sb"}[g]
                tl = const.tile([128, 1024], F8, tag=key, name=key)
                dbg[key] = tl
                nc.gpsimd.tensor_copy(tl[:], pt[:])
            is_last = (g + G >= ncols)
            pending.append((
                lambda pt=pt, g=g, u0=u0, f=emit_pv: f(pt, g, u0),
                epilogue if is_last else None,
            ))
    while pending:
        drain_one()
    if dbg is not None:
        nc.sync.dma_start(dbg["ptA"][:, :], dbg["ptA_sb"][:])
        nc.sync.dma_start(dbg["ptB"][:, :], dbg["ptB_sb"][:])
        nc.sync.dma_start(dbg["ptC"][:, :], dbg["ptC_sb"][:])
        nc.sync.dma_start(dbg["pv0"][:, :], dbg["pv0_sb"][:])
        nc.sync.dma_start(dbg["pv1"][:, :], dbg["pv1_sb"][:])
        nc.sync.dma_start(dbg["k8o"][:, :], k8_sb[:])
        nc.sync.dma_start(dbg["q8o"][:, :], q8_sb[:])
        nc.sync.dma_start(dbg["vt8o"][:, :], vt8_sb[:])
        nc.sync.dma_start(dbg["m8o"][:, :], m8_sb[:])


def _mask_mul(nc, r, gb, pt_v, mk_v):
    # chunks 1-2 run while DVE is saturated with K/V copies -> gpsimd,
    # except the last band groups (gb >= 8) whose PVs drain into the next
    # chunk: the slow gpsimd there would stall the in-order PE at the
    # boundary. chunk 0 (bf16, 2x mode) and chunk 3 (DVE idle) -> DVE.
    if r in (1, 2, 3) and gb < 8:
        nc.gpsimd.tensor_mul(pt_v, pt_v, mk_v)
    else:
        nc.vector.tensor_mul(pt_v, pt_v, mk_v)


def _host_inputs(x, Wq, Wk, Wv):
    xT = np.ascontiguousarray(x.T)                       # [256, 8192] f32
    x8 = np.ascontiguousarray(
        xT.reshape(2, 128, S).transpose(1, 0, 2).reshape(128, 2 * S)
    ).astype(NPF8)
    xT16 = np.ascontiguousarray(xT[:, :BCOLS]).astype(NPBF16).reshape(2, 128, BCOLS)
    wqb = np.ascontiguousarray(Wq.T).astype(NPBF16).reshape(2, 128, DIM)
    wkb = np.ascontiguousarray(Wk.T).astype(NPBF16).reshape(2, 128, DIM)
    wvb = np.ascontiguousarray(Wv.T).astype(NPBF16).reshape(2, 128, DE)
    wk8 = np.ascontiguousarray(
        Wk.T.reshape(2, 128, DIM).transpose(1, 0, 2).reshape(128, 2 * DIM)
    ).astype(NPF8)
    wv8 = np.ascontiguousarray(
        Wv.T.reshape(2, 128, DE).transpose(1, 0, 2).reshape(128, 2 * DE)
    ).astype(NPF8)
    k_idx = np.arange(128)[:, None, None]
    jp = np.arange(BAND)[None, :, None]
    u = 128 * (jp >= 8) + np.arange(128)[None, None, :]
    in_maps = []
    for c in range(NCORES):
        xq = np.ascontiguousarray(x[c::NCORES].T).astype(NPBF16).reshape(2, 128, RPC)
        m = (128 * jp + k_idx <= 8 * u + c)
        m = np.ascontiguousarray(m.reshape(128, BAND * 128))
        in_maps.append({
            "x8": x8, "xT": xT16, "xqT": xq, "wqT": wqb, "wkT": wkb,
            "wvT": wvb, "wk8": wk8, "wv8": wv8,
            "m16": m.astype(NPBF16), "m8": m.astype(NPF8),
        })
    return in_maps


def kernel(x, Wq, Wk, Wv, _trace=False, _trace_kwargs=None):
    if "nc" not in _cached:
        _cached["nc"] = _build_nc()
    nc = _cached["nc"]
    in_maps = _host_inputs(
        np.asarray(x, np.float32), np.asarray(Wq, np.float32),
        np.asarray(Wk, np.float32), np.asarray(Wv, np.float32),
    )
    kw = dict(_trace_kwargs or {})
    res = run_bass_kernel_spmd(
        nc, in_maps, core_ids=list(range(NCORES)), trace=_trace, **kw
    )
    out = np.empty((S, DE), np.float32)
    for c in range(NCORES):
        out[c::NCORES] = res.results[c]["out"]
    _cached["last_results"] = res
    return out


# revision 53
# speedup vs baseline: 1.0530x; 1.0263x over previous
"""Causal attention head (S=8192, De=dim=256) on 8 trn2 NeuronCores.

Math (reference):
    Q = Wq @ x.T; K = Wk @ x.T; V = Wv @ x.T
    S = (Q.T @ K) / sqrt(256); causal mask (upper tri -> -inf)
    out = softmax(S, axis=1) @ V.T          # [8192, 256]

Sharding: core c owns rows c::8 (stride-8 interleave) -> every core's
row block has a near-identical causal prefix profile, so the SPMD kernel
is identical across cores; all per-core variation is input data.

Per-core kernel, fp8e4 DoubleRow edition. The PE's fp8 DoubleRow mode
contracts 2x128 partitions per pass at 0.5 cycles/row (4x bf16 for a
256-deep contraction), so the large matmuls run in fp8; the few-key
early causal rows (chunk 0 = local rows 0..255) keep an exact bf16
score+PV path since score noise doesn't average out there.
  - K: col tiles 0..15 bf16-generated (kept as k16 for chunk-0 ST,
    gpsimd-quantized into k8), tiles 16..63 fp8 DoubleRow into k8.
  - V: tiles 0..15 bf16 into vt16 (chunk-0 PV) + gpsimd-converted into
    vt8; tiles 16..63 fp8 DoubleRow straight into vt8.
  - Q bf16 -> q8 (all rows) and q16 (chunk-0 rows).
  - scores: S.T tile = K_j.T @ Q_r as ONE DoubleRow inst per 128-col
    tile for chunks 1-3, bf16 two-inst contraction for chunk 0.
  - exp on ACT: fp8 out with exponent bias -2 for chunks 1-3 (score
    outliers reach ~95, exp(s/16) would overflow fp8e4's 240 max; the
    bias cancels in the row-sum normalization), bf16 out for chunk 0.
  - causal band masks: 0/1 multiply over just the 128-row mixed window
    per tile; gpsimd for early band groups, DVE (2x bf16) otherwise.
  - PV: chunk 0 in bf16 (exact V), chunks 1-3 DoubleRow over col-tile
    pairs, moving dim split (0:128 | 128:257) to stay under the 512
    moving-row ISA limit; ones column yields row sums for free.
  - K/V generation is emitted as "filler" groups interleaved into the
    attention chunks (just-in-time via ensure_*), so their PSUM->SBUF
    copies overlap the exp-bound phase; the softmax pipeline is 6 deep
    and carried across chunk boundaries.
No softmax max-subtraction needed in fp32: |scores/16| <= ~6.
"""

import sys

sys.path.insert(0, "/opt/trn_rl_repo")

from contextlib import ExitStack

import ml_dtypes
import numpy as np

import concourse.bass as bass
import concourse.mybir as mybir
import concourse.tile as tile
from concourse import bacc
from concourse.bass_utils import run_bass_kernel_spmd

BF16 = mybir.dt.bfloat16
F8 = mybir.dt.float8e4
F32 = mybir.dt.float32
NPBF16 = ml_dtypes.bfloat16
NPF8 = ml_dtypes.float8_e4m3
DR = mybir.MatmulPerfMode.DoubleRow

S, DIM, DE = 8192, 256, 256
NCORES = 8
RPC = S // NCORES          # 1024 rows per core
RCHUNK = 256               # rows per S.T matmul (moving free dim)
NRC = RPC // RCHUNK        # 4 row chunks per core
CT = 128                   # col tile (PE partition)
NCT = S // CT              # 64 col tiles total
BAND = 2048 // CT          # 16 col tiles per causal band of a row chunk
VW = DE + 1                # V.T chunk width incl. ones column
BCOLS = 2048               # cols covered by the bf16 K/V path (tiles < 16)

_cached = {}


def _build_nc(repeat=0, debug=False):
    nc = bacc.Bacc("TRN2", target_bir_lowering=False, debug=False,
                   num_devices=NCORES)
    x8d = nc.dram_tensor("x8", [128, 2 * S], F8, kind="ExternalInput")
    xTd = nc.dram_tensor("xT", [2, 128, BCOLS], BF16, kind="ExternalInput")
    xqT = nc.dram_tensor("xqT", [2, 128, RPC], BF16, kind="ExternalInput")
    wqT = nc.dram_tensor("wqT", [2, 128, DIM], BF16, kind="ExternalInput")
    wkT = nc.dram_tensor("wkT", [2, 128, DIM], BF16, kind="ExternalInput")
    wvT = nc.dram_tensor("wvT", [2, 128, DE], BF16, kind="ExternalInput")
    wk8d = nc.dram_tensor("wk8", [128, 2 * DIM], F8, kind="ExternalInput")
    wv8d = nc.dram_tensor("wv8", [128, 2 * DE], F8, kind="ExternalInput")
    m16d = nc.dram_tensor("m16", [128, BAND * 128], BF16, kind="ExternalInput")
    m8d = nc.dram_tensor("m8", [128, BAND * 128], F8, kind="ExternalInput")
    outd = nc.dram_tensor("out", [RPC, DE], F32, kind="ExternalOutput")
    dbg = None
    if debug:
        dbg = {
            "k8o": nc.dram_tensor("k8o", [128, 2 * S], F8, kind="ExternalOutput"),
            "q8o": nc.dram_tensor("q8o", [128, 2 * RPC], F8, kind="ExternalOutput"),
            "vt8o": nc.dram_tensor("vt8o", [128, NCT * VW], F8, kind="ExternalOutput"),
            "m8o": nc.dram_tensor("m8o", [128, BAND * 128], F8, kind="ExternalOutput"),
            "ptA": nc.dram_tensor("ptA", [128, 1024], F8, kind="ExternalOutput"),
            "ptB": nc.dram_tensor("ptB", [128, 1024], F8, kind="ExternalOutput"),
            "ptC": nc.dram_tensor("ptC", [128, 1024], F8, kind="ExternalOutput"),
            "pv0": nc.dram_tensor("pv0", [128, VW], F32, kind="ExternalOutput"),
            "pv1": nc.dram_tensor("pv1", [128, VW], F32, kind="ExternalOutput"),
        }

    with tile.TileContext(nc) as tc, ExitStack() as ctx:
        const = ctx.enter_context(tc.tile_pool(name="const", bufs=1))
        ps_st = ctx.enter_context(tc.tile_pool(name="ps_st", bufs=3, space="PSUM"))
        ps_pv = ctx.enter_context(tc.tile_pool(name="ps_pv", bufs=1, space="PSUM"))
        pt_pool = ctx.enter_context(tc.tile_pool(name="pt", bufs=9))
        ep_pool = ctx.enter_context(tc.tile_pool(name="ep", bufs=4))

        def body(_iv=None):
            _emit(nc, tc, const, ps_st, ps_pv, pt_pool, ep_pool,
                  x8d, xTd, xqT, wqT, wkT, wvT, wk8d, wv8d, m16d, m8d, outd,
                  dbg)

        if repeat:
            with tc.For_i(0, repeat, 1) as _iv:
                body(_iv)
        else:
            body()

    nc.compile()
    return nc


def _emit(nc, tc, const, ps_st, ps_pv, pt_pool, ep_pool,
          x8d, xTd, xqT, wqT, wkT, wvT, wk8d, wv8d, m16d, m8d, outd,
          dbg=None):
    # ---- constants / staged inputs in SBUF ----
    x8_sb = const.tile([128, 2 * S], F8, tag="x8")
    xt_sb = [const.tile([128, BCOLS], BF16, tag=f"xt{i}", name=f"xt{i}") for i in range(2)]
    xq_sb = [const.tile([128, RPC], BF16, tag=f"xq{i}", name=f"xq{i}") for i in range(2)]
    wq_sb = const.tile([128, 2 * DIM], BF16, tag="wq")
    wk_sb = const.tile([128, 2 * DIM], BF16, tag="wk")
    wv_sb = const.tile([128, 2 * DE], BF16, tag="wv")
    wk8_sb = const.tile([128, 2 * DIM], F8, tag="wk8")
    wv8_sb = const.tile([128, 2 * DE], F8, tag="wv8")
    m16_sb = const.tile([128, BAND * 128], BF16, tag="m16")
    m8_sb = const.tile([128, BAND * 128], F8, tag="m8")
    k8_sb = const.tile([128, 2 * S], F8, tag="k8")
    q8_sb = const.tile([128, 2 * RPC], F8, tag="q8")
    k16_sb = [const.tile([128, BCOLS], BF16, tag=f"k16_{i}", name=f"k16_{i}")
              for i in range(2)]
    q16_sb = [const.tile([128, RCHUNK], BF16, tag=f"q16_{i}", name=f"q16_{i}")
              for i in range(2)]
    vt8_sb = const.tile([128, NCT * VW], F8, tag="vt8")
    vt16_sb = const.tile([128, BAND * VW], BF16, tag="vt16")

    x8_3 = x8_sb[:].rearrange("p (i c) -> p i c", i=2)     # [128,2,S]
    wk8_3 = wk8_sb[:].rearrange("p (i d) -> p i d", i=2)   # [128,2,256]
    wv8_3 = wv8_sb[:].rearrange("p (i e) -> p i e", i=2)   # [128,2,256]
    k8_3 = k8_sb[:].rearrange("p (i c) -> p i c", i=2)     # [128,2,S]
    q8_3 = q8_sb[:].rearrange("p (i r) -> p i r", i=2)     # [128,2,RPC]

    # input staging: weights ride the otherwise-idle DVE queue, x.T (bf16)
    # in fine chunks on ACT (K-bf16 consumes it first), x8 immediately on
    # SP in consumption order, xq+masks via gpsimd SWDGE.
    # xq + wq lead the SP queue: Q-gen is the serial prefix of the whole
    # kernel, and the SWDGE path starts ~2us slower than HWDGE
    for i in range(2):
        nc.sync.dma_start(xq_sb[i][:], xqT[i, :, :])
        nc.sync.dma_start(wq_sb[:, i * DIM:(i + 1) * DIM], wqT[i, :, :])
    for i in range(2):
        nc.gpsimd.dma_start(wk_sb[:, i * DIM:(i + 1) * DIM], wkT[i, :, :])
        nc.gpsimd.dma_start(wv_sb[:, i * DE:(i + 1) * DE], wvT[i, :, :])
    nc.gpsimd.dma_start(wv8_sb[:], wv8d[:, :])
    nc.gpsimd.dma_start(wk8_sb[:], wk8d[:, :])
    nc.gpsimd.dma_start(m8_sb[:], m8d[:, :])
    nc.gpsimd.dma_start(m16_sb[:], m16d[:, :])
    for o in range(0, BCOLS, 1024):
        for i in range(2):
            nc.scalar.dma_start(xt_sb[i][:, o:o + 1024], xTd[i, :, o:o + 1024])
    XCH = 2048
    for o in range(0, S, XCH):
        for i in range(2):
            nc.sync.dma_start(x8_sb[:, i * S + o: i * S + o + XCH],
                              x8d[:, i * S + o: i * S + o + XCH])
    # per-partition bias vector for the fp8 exp (see below)
    nbias = const.tile([128, 1], F32, tag="nbias")
    nc.vector.memset(nbias[:], -2.0)
    # dummy activation right away: pulls the 1.3us Exp table load into the
    # initial DMA-wait window instead of the first real exp
    warm = const.tile([128, 1], F32, tag="warm")
    nc.scalar.activation(warm[:], nbias[:],
                         mybir.ActivationFunctionType.Exp)
    # ones columns for V.T: only col 256 of each VW-chunk needs the 1.0
    # (on DVE: strided single-element writes, trivially cheap)
    nc.vector.memset(
        vt8_sb[:].rearrange("p (c w) -> p c w", w=VW)[:, :, DE:DE + 1], 1.0)
    nc.vector.memset(
        vt16_sb[:].rearrange("p (c w) -> p c w", w=VW)[:, :, DE:DE + 1], 1.0)

    # round-robin of PSUM->SBUF copy engines; ACT also runs all exps so it
    # gets a lighter share.
    cp_state = [0]

    def cp(dst, src):
        cp_state[0] += 1
        # the first ~12 copies happen before the exp stream ramps up, so
        # ACT can share them evenly; after that ACT is exp-bound and only
        # takes every 8th
        if cp_state[0] <= 18 and cp_state[0] % 2 == 0:
            nc.scalar.copy(dst, src)
        else:
            nc.vector.tensor_copy(dst, src)

    # ---- Q = Wq @ xq.T -> quantized q8 ----
    for d in range(2):
        ps = ps_st.tile([128, 1024], F32, tag="st", name="psq")
        for n in range(0, RPC, 512):
            for kd in range(2):
                nc.tensor.matmul(
                    ps[:, n:n + 512],
                    wq_sb[:, kd * DIM + d * 128: kd * DIM + d * 128 + 128],
                    xq_sb[kd][:, n:n + 512],
                    start=(kd == 0), stop=(kd == 1),
                )
        cp(q8_sb[:, d * RPC:(d + 1) * RPC], ps[:])
        cp(q16_sb[d][:], ps[:, 0:RCHUNK])

    # ---- filler closures, interleaved into the attention chunks so their
    # PSUM->SBUF copies (DVE/ACT) overlap the exp-bound phase: bf16 V.T
    # tiles 0..15 into vt16 (chunk-0 PV), fp8 DoubleRow vt8 (all 64
    # tiles), and fp8 K tiles 16..63.
    def _k16_group(d, p):
        # K tiles 0..15 in bf16 (x.T cols < 2048), kept in bf16 for
        # chunk-0's exact ST and gpsimd-quantized to k8 for the fp8 chunks
        def go():
            ps = ps_st.tile([128, 1024], F32, tag="st", name="psk")
            for nn in range(0, 1024, 512):
                for kd in range(2):
                    nc.tensor.matmul(
                        ps[:, nn:nn + 512],
                        wk_sb[:, kd * DIM + d * 128: kd * DIM + d * 128 + 128],
                        xt_sb[kd][:, p + nn:p + nn + 512],
                        start=(kd == 0), stop=(kd == 1),
                    )
            cp(k16_sb[d][:, p:p + 1024], ps[:])
            nc.gpsimd.tensor_copy(k8_sb[:, d * S + p:d * S + p + 1024],
                                  k16_sb[d][:, p:p + 1024])
        return go

    def _v16_group(g4):
        def go():
            ps = ps_st.tile([128, 1024], F32, tag="st", name="psv")
            for jj in range(4):
                j = g4 + jj
                for kd in range(2):
                    nc.tensor.matmul(
                        ps[:, jj * DE:(jj + 1) * DE],
                        xt_sb[kd][:, j * CT:(j + 1) * CT],
                        wv_sb[:, kd * DE:(kd + 1) * DE],
                        start=(kd == 0), stop=(kd == 1),
                    )
            vt_view = vt16_sb[:, g4 * VW:(g4 + 4) * VW].rearrange(
                "p (c w) -> p c w", w=VW)[:, :, 0:DE]
            cp(vt_view, ps[:].rearrange("p (c w) -> p c w", w=DE))
            nc.gpsimd.tensor_copy(vt8_sb[:, g4 * VW:(g4 + 4) * VW],
                                  vt16_sb[:, g4 * VW:(g4 + 4) * VW])
        return go

    def _v8_group(g4):
        def go():
            ps = ps_st.tile([128, 1024], F32, tag="st", name="psv8")
            for jj in range(4):
                j = g4 + jj
                nc.tensor.matmul(
                    ps[:, jj * DE:(jj + 1) * DE],
                    x8_3[:, :, j * CT:(j + 1) * CT],
                    wv8_3,
                    start=True, stop=True, perf_mode=DR,
                )
            vt_view = vt8_sb[:, g4 * VW:(g4 + 4) * VW].rearrange(
                "p (c w) -> p c w", w=VW)[:, :, 0:DE]
            cp(vt_view, ps[:].rearrange("p (c w) -> p c w", w=DE))
        return go

    def _k8_group(d, c0):
        def go():
            ps = ps_st.tile([128, 1024], F32, tag="st", name="psk8")
            for nn in range(0, 1024, 256):
                nc.tensor.matmul(
                    ps[:, nn:nn + 256],
                    wk8_3[:, :, d * 128:(d + 1) * 128],
                    x8_3[:, :, c0 + nn:c0 + nn + 256],
                    start=True, stop=True, perf_mode=DR,
                )
            cp(k8_sb[:, d * S + c0:d * S + c0 + 1024], ps[:])
        return go

    k16_fill = [(p, _k16_group(d, p))
                for p in range(0, BCOLS, 1024) for d in range(2)]
    v16_fill = [(g4, _v16_group(g4)) for g4 in range(0, BAND, 4)]
    v8_fill = [(g4, _v8_group(g4)) for g4 in range(BAND, NCT, 4)]
    k8_fill = [(c0, _k8_group(d, c0))
               for c0 in range(BCOLS, S, 1024) for d in range(2)]

    def ensure_k16(upto_col):
        while k16_fill and k16_fill[0][0] < upto_col:
            k16_fill.pop(0)[1]()

    def ensure_v16(upto_tile):
        while v16_fill and v16_fill[0][0] < upto_tile:
            v16_fill.pop(0)[1]()

    def ensure_v8(upto_tile):
        while v8_fill and v8_fill[0][0] < upto_tile:
            v8_fill.pop(0)[1]()

    def ensure_k8(upto_col):
        while k8_fill and k8_fill[0][0] < upto_col:
            k8_fill.pop(0)[1]()

    # proportional pacing: spread the filler groups over the 40 ST groups
    # so their PSUM->SBUF copies never pile up on DVE; vt16 first (chunk-0
    # PV drains earliest), then vt8 slightly ahead of k8 (PV trails ST)
    n_fill = (len(k16_fill) + len(v16_fill) + len(v8_fill)
              + len(k8_fill))
    pace = [0]

    def pace_fillers():
        pace[0] += 1
        target = n_fill * pace[0] // 40
        while (len(k16_fill) + len(v16_fill) + len(v8_fill)
               + len(k8_fill)) > n_fill - target:
            if k16_fill:
                k16_fill.pop(0)[1]()
            elif v16_fill:
                v16_fill.pop(0)[1]()
            elif v8_fill and (len(v8_fill) * 2 >= len(k8_fill) or not k8_fill):
                v8_fill.pop(0)[1]()
            elif k8_fill:
                k8_fill.pop(0)[1]()
            else:
                break

    # ---- attention: per row chunk, stream causal col tiles ----
    # col tiles in groups of G: one 2-bank PSUM tile holds G S.T tiles side
    # by side -> a single ACT exp (and a single band mask multiply) covers
    # the whole group, amortizing ACT overhead.
    #
    # software pipeline, depth 4, carried ACROSS chunk boundaries: PV for
    # group g is emitted after the ST matmuls of group g+4, and a chunk's
    # last PVs (plus its epilogue) drain while the next chunk's ST/exp
    # stream is already running.
    G = 4
    from collections import deque
    pending = deque()  # (emit_pv_fn, tail_fn_or_None)

    def drain_one():
        fn, tail = pending.popleft()
        fn()
        if tail is not None:
            tail()

    for r in range(NRC):
        fp8 = r > 0
        ncols = BAND * (r + 1)
        # h=0 (rows 0..127) accumulation ends at col tile 16r+7 (later
        # tiles are fully masked there); h=1 runs to the last tile.
        last_j = {0: min(BAND * r + 7, ncols - 1), 1: ncols - 1}
        pvbox = []

        def get_pv(pvbox=pvbox):
            # lazy: allocated at the first PV drain, which happens after
            # the previous chunk's epilogue has been emitted (bufs=1 ring)
            if not pvbox:
                pvbox.append([
                    ps_pv.tile([128, VW], F32, tag=f"pv{h}", name=f"pv{h}")
                    for h in range(2)
                ])
            return pvbox[0]

        if fp8:
            # DoubleRow PV over col-tile pairs; moving dim split at 128.
            def emit_pv(pt, g, u0, get_pv=get_pv, last_j=last_j):
                ensure_v16(min(g + G, BAND))
                ensure_v8(g + 3 * G)
                pv = get_pv()
                pt3 = pt[:].rearrange("p (c u) -> p c u", u=RCHUNK)
                for t in range(0, G, 2):
                    j = g + t
                    lhs = pt3[:, t:t + 2]  # [128, 2, 256] both tiles
                    rhs = vt8_sb[:, j * VW:(j + 2) * VW].rearrange(
                        "p (c w) -> p c w", w=VW)
                    for h in ((1,) if u0 else (0, 1)):
                        # PSUM start=1 marks the whole 2KB zero region
                        # pending-zero; each write to a pending byte zeroes
                        # then writes. So ONLY the very first inst starts:
                        # the w1 split's first write rides the same mark. A
                        # second start would re-mark w0's bytes and wipe its
                        # pair-0 contribution on the next accumulation.
                        first = (j == 0)
                        last = (j + 1 == last_j[h])
                        nc.tensor.matmul(
                            pv[h][:, 0:128],
                            lhs[:, :, h * 128:h * 128 + 128],
                            rhs[:, :, 0:128],
                            start=first, stop=False, perf_mode=DR,
                            skip_group_check=True,
                        )
                        nc.tensor.matmul(
                            pv[h][:, 128:VW],
                            lhs[:, :, h * 128:h * 128 + 128],
                            rhs[:, :, 128:VW],
                            start=False, stop=last, perf_mode=DR,
                            skip_group_check=True,
                        )
        else:
            def emit_pv(pt, g, u0, get_pv=get_pv, last_j=last_j):
                ensure_v16(g + G)
                pv = get_pv()
                for t in range(G):
                    j = g + t
                    for h in ((1,) if u0 else (0, 1)):
                        nc.tensor.matmul(
                            pv[h][:],
                            pt[:, t * RCHUNK + h * 128: t * RCHUNK + h * 128 + 128],
                            vt16_sb[:, j * VW:(j + 1) * VW],
                            start=(j == 0), stop=(j == last_j[h]),
                        )

        def epilogue(get_pv=get_pv, r=r):
            pv = get_pv()
            if dbg is not None and r == 1:
                for h in range(2):
                    tl = const.tile([128, VW], F32, tag=f"pv{h}_sb",
                                    name=f"pv{h}_sb")
                    dbg[f"pv{h}_sb"] = tl
                    nc.scalar.copy(tl[:], pv[h][:])
            for h in range(2):
                linv = ep_pool.tile([128, 1], F32, tag="linv")
                nc.vector.reciprocal(linv[:], pv[h][:, DE:DE + 1])
                osb = ep_pool.tile([128, DE], F32, tag="osb")
                nc.vector.tensor_scalar_mul(osb[:], pv[h][:, 0:DE], linv[:])
                rows = r * RCHUNK + h * 128
                nc.sync.dma_start(outd[rows:rows + 128, :], osb[:])

        for g in range(0, ncols, G):
            gb = g - BAND * r
            if r == 0:
                ensure_k16(CT * (g + G))
            else:
                # prefetch margin: the filler's PSUM->SBUF copy takes ~1.2us,
                # so pull k8 coverage ~2 groups ahead of the ST that reads it
                ensure_k8(CT * (g + 3 * G))
            # when every col tile in the group has j' >= 8, rows 0..127
            # of the chunk are entirely non-causal: compute only the
            # high 128 rows (u0=128) and skip the h=0 PV matmuls.
            u0 = 128 if gb >= 8 else 0
            st = ps_st.tile([128, G * RCHUNK], F32, tag="st")
            for t in range(G):
                j = g + t
                if fp8:
                    nc.tensor.matmul(
                        st[:, t * RCHUNK + u0:(t + 1) * RCHUNK],
                        k8_3[:, :, j * CT:(j + 1) * CT],
                        q8_3[:, :, r * RCHUNK + u0:(r + 1) * RCHUNK],
                        start=True, stop=True, perf_mode=DR,
                    )
                else:
                    # chunk 0 holds the few-key causal rows, most sensitive
                    # to score noise: exact bf16 K/Q there
                    for kd in range(2):
                        nc.tensor.matmul(
                            st[:, t * RCHUNK + u0:(t + 1) * RCHUNK],
                            k16_sb[kd][:, j * CT:(j + 1) * CT],
                            q16_sb[kd][:, u0:RCHUNK],
                            start=(kd == 0), stop=(kd == 1),
                        )
            pace_fillers()
            # ramp the pipeline down toward the end of the last chunk so
            # the tail drain after the final ST group is short
            depth = 6 if not (r == 3 and g >= ncols - 4 * G) else 3
            while len(pending) >= depth:
                drain_one()
            pdt = F8 if fp8 else BF16
            msk = m8_sb if fp8 else m16_sb
            pt = pt_pool.tile([128, G * RCHUNK], pdt, tag="pt8" if fp8 else "pt16")
            # fp8 chunks store P' = exp(s/16 - 2): score outliers reach
            # ~16*6 (heavy |q||k| tails), and exp would overflow fp8e4's
            # 240 max -> inf -> NaN after the 0-mask. The constant bias
            # cancels exactly in the row-sum normalization.
            bias = nbias[:] if fp8 else 0.0
            if u0:
                st_v = st[:].rearrange("p (c w) -> p c w", w=RCHUNK)[:, :, u0:]
                pt_v = pt[:].rearrange("p (c w) -> p c w", w=RCHUNK)[:, :, u0:]
                nc.scalar.activation(
                    pt_v, st_v, mybir.ActivationFunctionType.Exp,
                    scale=0.0625, bias=bias,
                )
            else:
                nc.scalar.activation(
                    pt[:], st[:], mybir.ActivationFunctionType.Exp,
                    scale=0.0625, bias=bias,
                )
            if gb >= 0:
                # diagonal band: only the 128-row window starting at u0 of
                # each tile mixes causal/non-causal entries (rows below are
                # never read thanks to the u0 skip, rows above are fully
                # causal), so the 0/1 mask covers just that window.
                pt_w = pt[:].rearrange(
                    "p (c w) -> p c w", w=RCHUNK)[:, :, u0:u0 + 128]
                mk_w = msk[:, gb * 128:(gb + G) * 128].rearrange(
                    "p (c w) -> p c w", w=128)
                _mask_mul(nc, r, gb, pt_w, mk_w)
            if dbg is not None and r == 1 and g in (0, 16, 24):
                key = {0: "ptA_sb", 16: "ptB_sb", 24: "ptC_sb"}[g]
                tl = const.tile([128, 1024], F8, tag=key, name=key)
                dbg[key] = tl
                nc.gpsimd.tensor_copy(tl[:], pt[:])
            is_last = (g + G >= ncols)
            pending.append((
                lambda pt=pt, g=g, u0=u0, f=emit_pv: f(pt, g, u0),
                epilogue if is_last else None,
            ))
    while pending:
        drain_one()
    if dbg is not None:
        nc.sync.dma_start(dbg["ptA"][:, :], dbg["ptA_sb"][:])
        nc.sync.dma_start(dbg["ptB"][:, :], dbg["ptB_sb"][:])
        nc.sync.dma_start(dbg["ptC"][:, :], dbg["ptC_sb"][:])
        nc.sync.dma_start(dbg["pv0"][:, :], dbg["pv0_sb"][:])
        nc.sync.dma_start(dbg["pv1"][:, :], dbg["pv1_sb"][:])
        nc.sync.dma_start(dbg["k8o"][:, :], k8_sb[:])
        nc.sync.dma_start(dbg["q8o"][:, :], q8_sb[:])
        nc.sync.dma_start(dbg["vt8o"][:, :], vt8_sb[:])
        nc.sync.dma_start(dbg["m8o"][:, :], m8_sb[:])


def _mask_mul(nc, r, gb, pt_v, mk_v):
    # chunks 1-2 run while DVE is saturated with K/V copies -> gpsimd,
    # except the last band groups (gb >= 8) whose PVs drain into the next
    # chunk: the slow gpsimd there would stall the in-order PE at the
    # boundary. chunk 0 (bf16, 2x mode) and chunk 3 (DVE idle) -> DVE.
    if r in (1, 2, 3) and gb < 8:
        nc.gpsimd.tensor_mul(pt_v, pt_v, mk_v)
    else:
        nc.vector.tensor_mul(pt_v, pt_v, mk_v)


def _host_inputs(x, Wq, Wk, Wv):
    xT = np.ascontiguousarray(x.T)                       # [256, 8192] f32
    x8 = np.ascontiguousarray(
        xT.reshape(2, 128, S).transpose(1, 0, 2).reshape(128, 2 * S)
    ).astype(NPF8)
    xT16 = np.ascontiguousarray(xT[:, :BCOLS]).astype(NPBF16).reshape(2, 128, BCOLS)
    wqb = np.ascontiguousarray(Wq.T).astype(NPBF16).reshape(2, 128, DIM)
    wkb = np.ascontiguousarray(Wk.T).astype(NPBF16).reshape(2, 128, DIM)
    wvb = np.ascontiguousarray(Wv.T).astype(NPBF16).reshape(2, 128, DE)
    wk8 = np.ascontiguousarray(
        Wk.T.reshape(2, 128, DIM).transpose(1, 0, 2).reshape(128, 2 * DIM)
    ).astype(NPF8)
    wv8 = np.ascontiguousarray(
        Wv.T.reshape(2, 128, DE).transpose(1, 0, 2).reshape(128, 2 * DE)
    ).astype(NPF8)
    k_idx = np.arange(128)[:, None, None]
    jp = np.arange(BAND)[None, :, None]
    u = 128 * (jp >= 8) + np.arange(128)[None, None, :]
    in_maps = []
    for c in range(NCORES):
        xq = np.ascontiguousarray(x[c::NCORES].T).astype(NPBF16).reshape(2, 128, RPC)
        m = (128 * jp + k_idx <= 8 * u + c)
        m = np.ascontiguousarray(m.reshape(128, BAND * 128))
        in_maps.append({
            "x8": x8, "xT": xT16, "xqT": xq, "wqT": wqb, "wkT": wkb,
            "wvT": wvb, "wk8": wk8, "wv8": wv8,
            "m16": m.astype(NPBF16), "m8": m.astype(NPF8),
        })
    return in_maps


def kernel(x, Wq, Wk, Wv, _trace=False, _trace_kwargs=None):
    if "nc" not in _cached:
        _cached["nc"] = _build_nc()
    nc = _cached["nc"]
    in_maps = _host_inputs(
        np.asarray(x, np.float32), np.asarray(Wq, np.float32),
        np.asarray(Wk, np.float32), np.asarray(Wv, np.float32),
    )
    kw = dict(_trace_kwargs or {})
    res = run_bass_kernel_spmd(
        nc, in_maps, core_ids=list(range(NCORES)), trace=_trace, **kw
    )
    out = np.empty((S, DE), np.float32)
    for c in range(NCORES):
        out[c::NCORES] = res.results[c]["out"]
    _cached["last_results"] = res
    return out


# revision 54
# speedup vs baseline: 1.0806x; 1.0262x over previous
"""Causal attention head (S=8192, De=dim=256) on 8 trn2 NeuronCores.

Math (reference):
    Q = Wq @ x.T; K = Wk @ x.T; V = Wv @ x.T
    S = (Q.T @ K) / sqrt(256); causal mask (upper tri -> -inf)
    out = softmax(S, axis=1) @ V.T          # [8192, 256]

Sharding: core c owns rows c::8 (stride-8 interleave) -> every core's
row block has a near-identical causal prefix profile, so the SPMD kernel
is identical across cores; all per-core variation is input data.

Per-core kernel, fp8e4 DoubleRow edition. The PE's fp8 DoubleRow mode
contracts 2x128 partitions per pass at 0.5 cycles/row (4x bf16 for a
256-deep contraction), so the large matmuls run in fp8; the few-key
early causal rows (chunk 0 = local rows 0..255) keep an exact bf16
score+PV path since score noise doesn't average out there.
  - K: col tiles 0..15 bf16-generated (kept as k16 for chunk-0 ST,
    gpsimd-quantized into k8), tiles 16..63 fp8 DoubleRow into k8.
  - V: tiles 0..15 bf16 into vt16 (chunk-0 PV) + gpsimd-converted into
    vt8; tiles 16..63 fp8 DoubleRow straight into vt8.
  - Q bf16 -> q8 (all rows) and q16 (chunk-0 rows).
  - scores: S.T tile = K_j.T @ Q_r as ONE DoubleRow inst per 128-col
    tile for chunks 1-3, bf16 two-inst contraction for chunk 0.
  - exp on ACT: fp8 out with exponent bias -2 for chunks 1-3 (score
    outliers reach ~95, exp(s/16) would overflow fp8e4's 240 max; the
    bias cancels in the row-sum normalization), bf16 out for chunk 0.
  - causal band masks: 0/1 multiply over just the 128-row mixed window
    per tile; gpsimd for early band groups, DVE (2x bf16) otherwise.
  - PV: chunk 0 in bf16 (exact V), chunks 1-3 DoubleRow over col-tile
    pairs, moving dim split (0:128 | 128:257) to stay under the 512
    moving-row ISA limit; ones column yields row sums for free.
  - K/V generation is emitted as "filler" groups interleaved into the
    attention chunks (just-in-time via ensure_*), so their PSUM->SBUF
    copies overlap the exp-bound phase; the softmax pipeline is 6 deep
    and carried across chunk boundaries.
No softmax max-subtraction needed in fp32: |scores/16| <= ~6.
"""

import sys

sys.path.insert(0, "/opt/trn_rl_repo")

from contextlib import ExitStack

import ml_dtypes
import numpy as np

import concourse.bass as bass
import concourse.mybir as mybir
import concourse.tile as tile
from concourse import bacc
from concourse.bass_utils import run_bass_kernel_spmd

BF16 = mybir.dt.bfloat16
F8 = mybir.dt.float8e4
F32 = mybir.dt.float32
NPBF16 = ml_dtypes.bfloat16
NPF8 = ml_dtypes.float8_e4m3
DR = mybir.MatmulPerfMode.DoubleRow

S, DIM, DE = 8192, 256, 256
NCORES = 8
RPC = S // NCORES          # 1024 rows per core
RCHUNK = 256               # rows per S.T matmul (moving free dim)
NRC = RPC // RCHUNK        # 4 row chunks per core
CT = 128                   # col tile (PE partition)
NCT = S // CT              # 64 col tiles total
BAND = 2048 // CT          # 16 col tiles per causal band of a row chunk
VW = DE + 1                # V.T chunk width incl. ones column
BCOLS = 2048               # cols covered by the bf16 K/V path (tiles < 16)

_cached = {}


def _build_nc(repeat=0, debug=False):
    nc = bacc.Bacc("TRN2", target_bir_lowering=False, debug=False,
                   num_devices=NCORES)
    x8d = nc.dram_tensor("x8", [128, 2 * S], F8, kind="ExternalInput")
    xTd = nc.dram_tensor("xT", [2, 128, BCOLS], BF16, kind="ExternalInput")
    xqT = nc.dram_tensor("xqT", [2, 128, RPC], BF16, kind="ExternalInput")
    wqT = nc.dram_tensor("wqT", [2, 128, DIM], BF16, kind="ExternalInput")
    wkT = nc.dram_tensor("wkT", [2, 128, DIM], BF16, kind="ExternalInput")
    wvT = nc.dram_tensor("wvT", [2, 128, DE], BF16, kind="ExternalInput")
    wk8d = nc.dram_tensor("wk8", [128, 2 * DIM], F8, kind="ExternalInput")
    wv8d = nc.dram_tensor("wv8", [128, 2 * DE], F8, kind="ExternalInput")
    m16d = nc.dram_tensor("m16", [128, BAND * 128], BF16, kind="ExternalInput")
    m8d = nc.dram_tensor("m8", [128, BAND * 128], F8, kind="ExternalInput")
    outd = nc.dram_tensor("out", [RPC, DE], F32, kind="ExternalOutput")
    dbg = None
    if debug:
        dbg = {
            "k8o": nc.dram_tensor("k8o", [128, 2 * S], F8, kind="ExternalOutput"),
            "q8o": nc.dram_tensor("q8o", [128, 2 * RPC], F8, kind="ExternalOutput"),
            "vt8o": nc.dram_tensor("vt8o", [128, NCT * VW], F8, kind="ExternalOutput"),
            "m8o": nc.dram_tensor("m8o", [128, BAND * 128], F8, kind="ExternalOutput"),
            "ptA": nc.dram_tensor("ptA", [128, 1024], F8, kind="ExternalOutput"),
            "ptB": nc.dram_tensor("ptB", [128, 1024], F8, kind="ExternalOutput"),
            "ptC": nc.dram_tensor("ptC", [128, 1024], F8, kind="ExternalOutput"),
            "pv0": nc.dram_tensor("pv0", [128, VW], F32, kind="ExternalOutput"),
            "pv1": nc.dram_tensor("pv1", [128, VW], F32, kind="ExternalOutput"),
        }

    with tile.TileContext(nc) as tc, ExitStack() as ctx:
        const = ctx.enter_context(tc.tile_pool(name="const", bufs=1))
        ps_st = ctx.enter_context(tc.tile_pool(name="ps_st", bufs=3, space="PSUM"))
        ps_pv = ctx.enter_context(tc.tile_pool(name="ps_pv", bufs=1, space="PSUM"))
        pt_pool = ctx.enter_context(tc.tile_pool(name="pt", bufs=9))
        ep_pool = ctx.enter_context(tc.tile_pool(name="ep", bufs=4))

        def body(_iv=None):
            _emit(nc, tc, const, ps_st, ps_pv, pt_pool, ep_pool,
                  x8d, xTd, xqT, wqT, wkT, wvT, wk8d, wv8d, m16d, m8d, outd,
                  dbg)

        if repeat:
            with tc.For_i(0, repeat, 1) as _iv:
                body(_iv)
        else:
            body()

    nc.compile()
    return nc


def _emit(nc, tc, const, ps_st, ps_pv, pt_pool, ep_pool,
          x8d, xTd, xqT, wqT, wkT, wvT, wk8d, wv8d, m16d, m8d, outd,
          dbg=None):
    # ---- constants / staged inputs in SBUF ----
    x8_sb = const.tile([128, 2 * S], F8, tag="x8")
    xt_sb = [const.tile([128, BCOLS], BF16, tag=f"xt{i}", name=f"xt{i}") for i in range(2)]
    xq_sb = [const.tile([128, RPC], BF16, tag=f"xq{i}", name=f"xq{i}") for i in range(2)]
    wq_sb = const.tile([128, 2 * DIM], BF16, tag="wq")
    wk_sb = const.tile([128, 2 * DIM], BF16, tag="wk")
    wv_sb = const.tile([128, 2 * DE], BF16, tag="wv")
    wk8_sb = const.tile([128, 2 * DIM], F8, tag="wk8")
    wv8_sb = const.tile([128, 2 * DE], F8, tag="wv8")
    m16_sb = const.tile([128, BAND * 128], BF16, tag="m16")
    m8_sb = const.tile([128, BAND * 128], F8, tag="m8")
    k8_sb = const.tile([128, 2 * S], F8, tag="k8")
    q8_sb = const.tile([128, 2 * RPC], F8, tag="q8")
    k16_sb = [const.tile([128, BCOLS], BF16, tag=f"k16_{i}", name=f"k16_{i}")
              for i in range(2)]
    q16_sb = [const.tile([128, RCHUNK], BF16, tag=f"q16_{i}", name=f"q16_{i}")
              for i in range(2)]
    vt8_sb = const.tile([128, NCT * VW], F8, tag="vt8")
    vt16_sb = const.tile([128, BAND * VW], BF16, tag="vt16")

    x8_3 = x8_sb[:].rearrange("p (i c) -> p i c", i=2)     # [128,2,S]
    wk8_3 = wk8_sb[:].rearrange("p (i d) -> p i d", i=2)   # [128,2,256]
    wv8_3 = wv8_sb[:].rearrange("p (i e) -> p i e", i=2)   # [128,2,256]
    k8_3 = k8_sb[:].rearrange("p (i c) -> p i c", i=2)     # [128,2,S]
    q8_3 = q8_sb[:].rearrange("p (i r) -> p i r", i=2)     # [128,2,RPC]

    # input staging: weights ride the otherwise-idle DVE queue, x.T (bf16)
    # in fine chunks on ACT (K-bf16 consumes it first), x8 immediately on
    # SP in consumption order, xq+masks via gpsimd SWDGE.
    # xq + wq lead the SP queue: Q-gen is the serial prefix of the whole
    # kernel, and the SWDGE path starts ~2us slower than HWDGE
    for i in range(2):
        nc.sync.dma_start(xq_sb[i][:], xqT[i, :, :])
        nc.sync.dma_start(wq_sb[:, i * DIM:(i + 1) * DIM], wqT[i, :, :])
    for i in range(2):
        nc.gpsimd.dma_start(wk_sb[:, i * DIM:(i + 1) * DIM], wkT[i, :, :])
        nc.gpsimd.dma_start(wv_sb[:, i * DE:(i + 1) * DE], wvT[i, :, :])
    nc.gpsimd.dma_start(wv8_sb[:], wv8d[:, :])
    nc.gpsimd.dma_start(wk8_sb[:], wk8d[:, :])
    nc.gpsimd.dma_start(m8_sb[:], m8d[:, :])
    nc.gpsimd.dma_start(m16_sb[:], m16d[:, :])
    for o in range(0, BCOLS, 1024):
        for i in range(2):
            nc.scalar.dma_start(xt_sb[i][:, o:o + 1024], xTd[i, :, o:o + 1024])
    XCH = 2048
    for o in range(0, S, XCH):
        for i in range(2):
            nc.sync.dma_start(x8_sb[:, i * S + o: i * S + o + XCH],
                              x8d[:, i * S + o: i * S + o + XCH])
    # per-partition bias vector for the fp8 exp (see below)
    nbias = const.tile([128, 1], F32, tag="nbias")
    nc.vector.memset(nbias[:], -2.0)
    # dummy activation right away: pulls the 1.3us Exp table load into the
    # initial DMA-wait window instead of the first real exp
    warm = const.tile([128, 1], F32, tag="warm")
    nc.scalar.activation(warm[:], nbias[:],
                         mybir.ActivationFunctionType.Exp)
    # ones columns for V.T: only col 256 of each VW-chunk needs the 1.0
    # (on DVE: strided single-element writes, trivially cheap)
    nc.vector.memset(
        vt8_sb[:].rearrange("p (c w) -> p c w", w=VW)[:, :, DE:DE + 1], 1.0)
    nc.vector.memset(
        vt16_sb[:].rearrange("p (c w) -> p c w", w=VW)[:, :, DE:DE + 1], 1.0)

    # round-robin of PSUM->SBUF copy engines; ACT also runs all exps so it
    # gets a lighter share.
    cp_state = [0]

    def cp(dst, src):
        cp_state[0] += 1
        # the first ~12 copies happen before the exp stream ramps up, so
        # ACT can share them evenly; after that ACT is exp-bound and only
        # takes every 8th
        if cp_state[0] <= 18 and cp_state[0] % 2 == 0:
            nc.scalar.copy(dst, src)
        else:
            nc.vector.tensor_copy(dst, src)

    # ---- Q = Wq @ xq.T -> quantized q8 ----
    for d in range(2):
        ps = ps_st.tile([128, 1024], F32, tag="st", name="psq")
        for n in range(0, RPC, 512):
            for kd in range(2):
                nc.tensor.matmul(
                    ps[:, n:n + 512],
                    wq_sb[:, kd * DIM + d * 128: kd * DIM + d * 128 + 128],
                    xq_sb[kd][:, n:n + 512],
                    start=(kd == 0), stop=(kd == 1),
                )
        cp(q8_sb[:, d * RPC:(d + 1) * RPC], ps[:])
        cp(q16_sb[d][:], ps[:, 0:RCHUNK])

    # ---- filler closures, interleaved into the attention chunks so their
    # PSUM->SBUF copies (DVE/ACT) overlap the exp-bound phase: bf16 V.T
    # tiles 0..15 into vt16 (chunk-0 PV), fp8 DoubleRow vt8 (all 64
    # tiles), and fp8 K tiles 16..63.
    def _k16_group(d, p):
        # K tiles 0..15 in bf16 (x.T cols < 2048), kept in bf16 for
        # chunk-0's exact ST and gpsimd-quantized to k8 for the fp8 chunks
        def go():
            ps = ps_st.tile([128, 1024], F32, tag="st", name="psk")
            for nn in range(0, 1024, 512):
                for kd in range(2):
                    nc.tensor.matmul(
                        ps[:, nn:nn + 512],
                        wk_sb[:, kd * DIM + d * 128: kd * DIM + d * 128 + 128],
                        xt_sb[kd][:, p + nn:p + nn + 512],
                        start=(kd == 0), stop=(kd == 1),
                    )
            cp(k16_sb[d][:, p:p + 1024], ps[:])
            nc.gpsimd.tensor_copy(k8_sb[:, d * S + p:d * S + p + 1024],
                                  k16_sb[d][:, p:p + 1024])
        return go

    def _v16_group(g4):
        def go():
            ps = ps_st.tile([128, 1024], F32, tag="st", name="psv")
            for jj in range(4):
                j = g4 + jj
                for kd in range(2):
                    nc.tensor.matmul(
                        ps[:, jj * DE:(jj + 1) * DE],
                        xt_sb[kd][:, j * CT:(j + 1) * CT],
                        wv_sb[:, kd * DE:(kd + 1) * DE],
                        start=(kd == 0), stop=(kd == 1),
                    )
            vt_view = vt16_sb[:, g4 * VW:(g4 + 4) * VW].rearrange(
                "p (c w) -> p c w", w=VW)[:, :, 0:DE]
            cp(vt_view, ps[:].rearrange("p (c w) -> p c w", w=DE))
            nc.gpsimd.tensor_copy(vt8_sb[:, g4 * VW:(g4 + 4) * VW],
                                  vt16_sb[:, g4 * VW:(g4 + 4) * VW])
        return go

    def _v8_group(g4):
        def go():
            ps = ps_st.tile([128, 1024], F32, tag="st", name="psv8")
            for jj in range(4):
                j = g4 + jj
                nc.tensor.matmul(
                    ps[:, jj * DE:(jj + 1) * DE],
                    x8_3[:, :, j * CT:(j + 1) * CT],
                    wv8_3,
                    start=True, stop=True, perf_mode=DR,
                )
            vt_view = vt8_sb[:, g4 * VW:(g4 + 4) * VW].rearrange(
                "p (c w) -> p c w", w=VW)[:, :, 0:DE]
            cp(vt_view, ps[:].rearrange("p (c w) -> p c w", w=DE))
        return go

    def _k8_group(d, c0):
        def go():
            ps = ps_st.tile([128, 1024], F32, tag="st", name="psk8")
            for nn in range(0, 1024, 256):
                nc.tensor.matmul(
                    ps[:, nn:nn + 256],
                    wk8_3[:, :, d * 128:(d + 1) * 128],
                    x8_3[:, :, c0 + nn:c0 + nn + 256],
                    start=True, stop=True, perf_mode=DR,
                )
            cp(k8_sb[:, d * S + c0:d * S + c0 + 1024], ps[:])
        return go

    k16_fill = [(p, _k16_group(d, p))
                for p in range(0, BCOLS, 1024) for d in range(2)]
    v16_fill = [(g4, _v16_group(g4)) for g4 in range(0, BAND, 4)]
    v8_fill = [(g4, _v8_group(g4)) for g4 in range(BAND, NCT, 4)]
    k8_fill = [(c0, _k8_group(d, c0))
               for c0 in range(BCOLS, S, 1024) for d in range(2)]

    def ensure_k16(upto_col):
        while k16_fill and k16_fill[0][0] < upto_col:
            k16_fill.pop(0)[1]()

    def ensure_v16(upto_tile):
        while v16_fill and v16_fill[0][0] < upto_tile:
            v16_fill.pop(0)[1]()

    def ensure_v8(upto_tile):
        while v8_fill and v8_fill[0][0] < upto_tile:
            v8_fill.pop(0)[1]()

    def ensure_k8(upto_col):
        while k8_fill and k8_fill[0][0] < upto_col:
            k8_fill.pop(0)[1]()

    # proportional pacing: spread the filler groups over the 40 ST groups
    # so their PSUM->SBUF copies never pile up on DVE; vt16 first (chunk-0
    # PV drains earliest), then vt8 slightly ahead of k8 (PV trails ST)
    n_fill = (len(k16_fill) + len(v16_fill) + len(v8_fill)
              + len(k8_fill))
    pace = [0]

    def pace_fillers():
        pace[0] += 1
        target = n_fill * pace[0] // 40
        while (len(k16_fill) + len(v16_fill) + len(v8_fill)
               + len(k8_fill)) > n_fill - target:
            if k16_fill:
                k16_fill.pop(0)[1]()
            elif v16_fill:
                v16_fill.pop(0)[1]()
            elif v8_fill and (len(v8_fill) * 2 >= len(k8_fill) or not k8_fill):
                v8_fill.pop(0)[1]()
            elif k8_fill:
                k8_fill.pop(0)[1]()
            else:
                break

    # ---- attention: per row chunk, stream causal col tiles ----
    # col tiles in groups of G: one 2-bank PSUM tile holds G S.T tiles side
    # by side -> a single ACT exp (and a single band mask multiply) covers
    # the whole group, amortizing ACT overhead.
    #
    # software pipeline, depth 4, carried ACROSS chunk boundaries: PV for
    # group g is emitted after the ST matmuls of group g+4, and a chunk's
    # last PVs (plus its epilogue) drain while the next chunk's ST/exp
    # stream is already running.
    G = 4
    from collections import deque
    pending = deque()  # (emit_pv_fn, tail_fn_or_None)

    def drain_one():
        fn, tail = pending.popleft()
        fn()
        if tail is not None:
            tail()

    for r in range(NRC):
        fp8 = r > 0
        ncols = BAND * (r + 1)
        # h=0 (rows 0..127) accumulation ends at col tile 16r+7 (later
        # tiles are fully masked there); h=1 runs to the last tile.
        last_j = {0: min(BAND * r + 7, ncols - 1), 1: ncols - 1}
        pvbox = []

        def get_pv(pvbox=pvbox):
            # lazy: allocated at the first PV drain, which happens after
            # the previous chunk's epilogue has been emitted (bufs=1 ring)
            if not pvbox:
                pvbox.append([
                    ps_pv.tile([128, VW], F32, tag=f"pv{h}", name=f"pv{h}")
                    for h in range(2)
                ])
            return pvbox[0]

        if fp8:
            # DoubleRow PV over col-tile pairs; moving dim split at 128.
            def emit_pv(pt, g, u0, get_pv=get_pv, last_j=last_j):
                ensure_v16(min(g + G, BAND))
                ensure_v8(g + 3 * G)
                pv = get_pv()
                pt3 = pt[:].rearrange("p (c u) -> p c u", u=RCHUNK)
                for t in range(0, G, 2):
                    j = g + t
                    lhs = pt3[:, t:t + 2]  # [128, 2, 256] both tiles
                    rhs = vt8_sb[:, j * VW:(j + 2) * VW].rearrange(
                        "p (c w) -> p c w", w=VW)
                    for h in ((1,) if u0 else (0, 1)):
                        # PSUM start=1 marks the whole 2KB zero region
                        # pending-zero; each write to a pending byte zeroes
                        # then writes. So ONLY the very first inst starts:
                        # the w1 split's first write rides the same mark. A
                        # second start would re-mark w0's bytes and wipe its
                        # pair-0 contribution on the next accumulation.
                        first = (j == 0)
                        last = (j + 1 == last_j[h])
                        nc.tensor.matmul(
                            pv[h][:, 0:128],
                            lhs[:, :, h * 128:h * 128 + 128],
                            rhs[:, :, 0:128],
                            start=first, stop=False, perf_mode=DR,
                            skip_group_check=True,
                        )
                        nc.tensor.matmul(
                            pv[h][:, 128:VW],
                            lhs[:, :, h * 128:h * 128 + 128],
                            rhs[:, :, 128:VW],
                            start=False, stop=last, perf_mode=DR,
                            skip_group_check=True,
                        )
        else:
            def emit_pv(pt, g, u0, get_pv=get_pv, last_j=last_j):
                ensure_v16(min(g + 3 * G, BAND))
                pv = get_pv()
                for t in range(G):
                    j = g + t
                    for h in ((1,) if u0 else (0, 1)):
                        nc.tensor.matmul(
                            pv[h][:],
                            pt[:, t * RCHUNK + h * 128: t * RCHUNK + h * 128 + 128],
                            vt16_sb[:, j * VW:(j + 1) * VW],
                            start=(j == 0), stop=(j == last_j[h]),
                        )

        def epilogue(get_pv=get_pv, r=r):
            pv = get_pv()
            if dbg is not None and r == 1:
                for h in range(2):
                    tl = const.tile([128, VW], F32, tag=f"pv{h}_sb",
                                    name=f"pv{h}_sb")
                    dbg[f"pv{h}_sb"] = tl
                    nc.scalar.copy(tl[:], pv[h][:])
            for h in range(2):
                linv = ep_pool.tile([128, 1], F32, tag="linv")
                nc.vector.reciprocal(linv[:], pv[h][:, DE:DE + 1])
                osb = ep_pool.tile([128, DE], F32, tag="osb")
                nc.vector.tensor_scalar_mul(osb[:], pv[h][:, 0:DE], linv[:])
                rows = r * RCHUNK + h * 128
                nc.sync.dma_start(outd[rows:rows + 128, :], osb[:])

        for g in range(0, ncols, G):
            gb = g - BAND * r
            if r == 0:
                ensure_k16(CT * (g + G))
            else:
                # prefetch margin: the filler's PSUM->SBUF copy takes ~1.2us,
                # so pull k8 coverage ~2 groups ahead of the ST that reads it
                ensure_k8(CT * (g + 3 * G))
            # when every col tile in the group has j' >= 8, rows 0..127
            # of the chunk are entirely non-causal: compute only the
            # high 128 rows (u0=128) and skip the h=0 PV matmuls.
            u0 = 128 if gb >= 8 else 0
            st = ps_st.tile([128, G * RCHUNK], F32, tag="st")
            for t in range(G):
                j = g + t
                if fp8:
                    nc.tensor.matmul(
                        st[:, t * RCHUNK + u0:(t + 1) * RCHUNK],
                        k8_3[:, :, j * CT:(j + 1) * CT],
                        q8_3[:, :, r * RCHUNK + u0:(r + 1) * RCHUNK],
                        start=True, stop=True, perf_mode=DR,
                    )
                else:
                    # chunk 0 holds the few-key causal rows, most sensitive
                    # to score noise: exact bf16 K/Q there
                    for kd in range(2):
                        nc.tensor.matmul(
                            st[:, t * RCHUNK + u0:(t + 1) * RCHUNK],
                            k16_sb[kd][:, j * CT:(j + 1) * CT],
                            q16_sb[kd][:, u0:RCHUNK],
                            start=(kd == 0), stop=(kd == 1),
                        )
            pace_fillers()
            # ramp the pipeline down toward the end of the last chunk so
            # the tail drain after the final ST group is short
            depth = 6 if not (r == 3 and g >= ncols - 4 * G) else 3
            while len(pending) >= depth:
                drain_one()
            pdt = F8 if fp8 else BF16
            msk = m8_sb if fp8 else m16_sb
            pt = pt_pool.tile([128, G * RCHUNK], pdt, tag="pt8" if fp8 else "pt16")
            # fp8 chunks store P' = exp(s/16 - 2): score outliers reach
            # ~16*6 (heavy |q||k| tails), and exp would overflow fp8e4's
            # 240 max -> inf -> NaN after the 0-mask. The constant bias
            # cancels exactly in the row-sum normalization.
            bias = nbias[:] if fp8 else 0.0
            if u0:
                st_v = st[:].rearrange("p (c w) -> p c w", w=RCHUNK)[:, :, u0:]
                pt_v = pt[:].rearrange("p (c w) -> p c w", w=RCHUNK)[:, :, u0:]
                nc.scalar.activation(
                    pt_v, st_v, mybir.ActivationFunctionType.Exp,
                    scale=0.0625, bias=bias,
                )
            else:
                nc.scalar.activation(
                    pt[:], st[:], mybir.ActivationFunctionType.Exp,
                    scale=0.0625, bias=bias,
                )
            if gb >= 0:
                # diagonal band: only the 128-row window starting at u0 of
                # each tile mixes causal/non-causal entries (rows below are
                # never read thanks to the u0 skip, rows above are fully
                # causal), so the 0/1 mask covers just that window.
                pt_w = pt[:].rearrange(
                    "p (c w) -> p c w", w=RCHUNK)[:, :, u0:u0 + 128]
                mk_w = msk[:, gb * 128:(gb + G) * 128].rearrange(
                    "p (c w) -> p c w", w=128)
                _mask_mul(nc, r, gb, pt_w, mk_w)
            if dbg is not None and r == 1 and g in (0, 16, 24):
                key = {0: "ptA_sb", 16: "ptB_sb", 24: "ptC_sb"}[g]
                tl = const.tile([128, 1024], F8, tag=key, name=key)
                dbg[key] = tl
                nc.gpsimd.tensor_copy(tl[:], pt[:])
            is_last = (g + G >= ncols)
            pending.append((
                lambda pt=pt, g=g, u0=u0, f=emit_pv: f(pt, g, u0),
                epilogue if is_last else None,
            ))
    while pending:
        drain_one()
    if dbg is not None:
        nc.sync.dma_start(dbg["ptA"][:, :], dbg["ptA_sb"][:])
        nc.sync.dma_start(dbg["ptB"][:, :], dbg["ptB_sb"][:])
        nc.sync.dma_start(dbg["ptC"][:, :], dbg["ptC_sb"][:])
        nc.sync.dma_start(dbg["pv0"][:, :], dbg["pv0_sb"][:])
        nc.sync.dma_start(dbg["pv1"][:, :], dbg["pv1_sb"][:])
        nc.sync.dma_start(dbg["k8o"][:, :], k8_sb[:])
        nc.sync.dma_start(dbg["q8o"][:, :], q8_sb[:])
        nc.sync.dma_start(dbg["vt8o"][:, :], vt8_sb[:])
        nc.sync.dma_start(dbg["m8o"][:, :], m8_sb[:])


def _mask_mul(nc, r, gb, pt_v, mk_v):
    # chunks 1-2 run while DVE is saturated with K/V copies -> gpsimd,
    # except the last band groups (gb >= 8) whose PVs drain into the next
    # chunk: the slow gpsimd there would stall the in-order PE at the
    # boundary. chunk 0 (bf16, 2x mode) and chunk 3 (DVE idle) -> DVE.
    if r in (1, 2, 3) and gb < 8:
        nc.gpsimd.tensor_mul(pt_v, pt_v, mk_v)
    else:
        nc.vector.tensor_mul(pt_v, pt_v, mk_v)


def _host_inputs(x, Wq, Wk, Wv):
    xT = np.ascontiguousarray(x.T)                       # [256, 8192] f32
    x8 = np.ascontiguousarray(
        xT.reshape(2, 128, S).transpose(1, 0, 2).reshape(128, 2 * S)
    ).astype(NPF8)
    xT16 = np.ascontiguousarray(xT[:, :BCOLS]).astype(NPBF16).reshape(2, 128, BCOLS)
    wqb = np.ascontiguousarray(Wq.T).astype(NPBF16).reshape(2, 128, DIM)
    wkb = np.ascontiguousarray(Wk.T).astype(NPBF16).reshape(2, 128, DIM)
    wvb = np.ascontiguousarray(Wv.T).astype(NPBF16).reshape(2, 128, DE)
    wk8 = np.ascontiguousarray(
        Wk.T.reshape(2, 128, DIM).transpose(1, 0, 2).reshape(128, 2 * DIM)
    ).astype(NPF8)
    wv8 = np.ascontiguousarray(
        Wv.T.reshape(2, 128, DE).transpose(1, 0, 2).reshape(128, 2 * DE)
    ).astype(NPF8)
    k_idx = np.arange(128)[:, None, None]
    jp = np.arange(BAND)[None, :, None]
    u = 128 * (jp >= 8) + np.arange(128)[None, None, :]
    in_maps = []
    for c in range(NCORES):
        xq = np.ascontiguousarray(x[c::NCORES].T).astype(NPBF16).reshape(2, 128, RPC)
        m = (128 * jp + k_idx <= 8 * u + c)
        m = np.ascontiguousarray(m.reshape(128, BAND * 128))
        in_maps.append({
            "x8": x8, "xT": xT16, "xqT": xq, "wqT": wqb, "wkT": wkb,
            "wvT": wvb, "wk8": wk8, "wv8": wv8,
            "m16": m.astype(NPBF16), "m8": m.astype(NPF8),
        })
    return in_maps


def kernel(x, Wq, Wk, Wv, _trace=False, _trace_kwargs=None):
    if "nc" not in _cached:
        _cached["nc"] = _build_nc()
    nc = _cached["nc"]
    in_maps = _host_inputs(
        np.asarray(x, np.float32), np.asarray(Wq, np.float32),
        np.asarray(Wk, np.float32), np.asarray(Wv, np.float32),
    )
    kw = dict(_trace_kwargs or {})
    res = run_bass_kernel_spmd(
        nc, in_maps, core_ids=list(range(NCORES)), trace=_trace, **kw
    )
    out = np.empty((S, DE), np.float32)
    for c in range(NCORES):
        out[c::NCORES] = res.results[c]["out"]
    _cached["last_results"] = res
    return out


# revision 55
# speedup vs baseline: 1.1020x; 1.0198x over previous
"""Causal attention head (S=8192, De=dim=256) on 8 trn2 NeuronCores.

Math (reference):
    Q = Wq @ x.T; K = Wk @ x.T; V = Wv @ x.T
    S = (Q.T @ K) / sqrt(256); causal mask (upper tri -> -inf)
    out = softmax(S, axis=1) @ V.T          # [8192, 256]

Sharding: core c owns rows c::8 (stride-8 interleave) -> every core's
row block has a near-identical causal prefix profile, so the SPMD kernel
is identical across cores; all per-core variation is input data.

Per-core kernel, fp8e4 DoubleRow edition. The PE's fp8 DoubleRow mode
contracts 2x128 partitions per pass at 0.5 cycles/row (4x bf16 for a
256-deep contraction), so the large matmuls run in fp8; the few-key
early causal rows (chunk 0 = local rows 0..255) keep an exact bf16
score+PV path since score noise doesn't average out there.
  - K: col tiles 0..15 bf16-generated (kept as k16 for chunk-0 ST,
    gpsimd-quantized into k8), tiles 16..63 fp8 DoubleRow into k8.
  - V: tiles 0..15 bf16 into vt16 (chunk-0 PV) + gpsimd-converted into
    vt8; tiles 16..63 fp8 DoubleRow straight into vt8.
  - Q bf16 -> q8 (all rows) and q16 (chunk-0 rows).
  - scores: S.T tile = K_j.T @ Q_r as ONE DoubleRow inst per 128-col
    tile for chunks 1-3, bf16 two-inst contraction for chunk 0.
  - exp on ACT: fp8 out with exponent bias -2 for chunks 1-3 (score
    outliers reach ~95, exp(s/16) would overflow fp8e4's 240 max; the
    bias cancels in the row-sum normalization), bf16 out for chunk 0.
  - causal band masks: 0/1 multiply over just the 128-row mixed window
    per tile; gpsimd for early band groups, DVE (2x bf16) otherwise.
  - PV: chunk 0 in bf16 (exact V), chunks 1-3 DoubleRow over col-tile
    pairs, moving dim split (0:128 | 128:257) to stay under the 512
    moving-row ISA limit; ones column yields row sums for free.
  - K/V generation is emitted as "filler" groups interleaved into the
    attention chunks (just-in-time via ensure_*), so their PSUM->SBUF
    copies overlap the exp-bound phase; the softmax pipeline is 6 deep
    and carried across chunk boundaries.
No softmax max-subtraction needed in fp32: |scores/16| <= ~6.
"""

import sys

sys.path.insert(0, "/opt/trn_rl_repo")

from contextlib import ExitStack

import ml_dtypes
import numpy as np

import concourse.bass as bass
import concourse.mybir as mybir
import concourse.tile as tile
from concourse import bacc
from concourse.bass_utils import run_bass_kernel_spmd

BF16 = mybir.dt.bfloat16
F8 = mybir.dt.float8e4
F32 = mybir.dt.float32
NPBF16 = ml_dtypes.bfloat16
NPF8 = ml_dtypes.float8_e4m3
DR = mybir.MatmulPerfMode.DoubleRow

S, DIM, DE = 8192, 256, 256
NCORES = 8
RPC = S // NCORES          # 1024 rows per core
RCHUNK = 256               # rows per S.T matmul (moving free dim)
NRC = RPC // RCHUNK        # 4 row chunks per core
CT = 128                   # col tile (PE partition)
NCT = S // CT              # 64 col tiles total
BAND = 2048 // CT          # 16 col tiles per causal band of a row chunk
VW = DE + 1                # V.T chunk width incl. ones column
BCOLS = 2048               # cols covered by the bf16 K/V path (tiles < 16)

_cached = {}


def _build_nc(repeat=0, debug=False):
    nc = bacc.Bacc("TRN2", target_bir_lowering=False, debug=False,
                   num_devices=NCORES)
    x8d = nc.dram_tensor("x8", [128, 2 * S], F8, kind="ExternalInput")
    xTd = nc.dram_tensor("xT", [2, 128, BCOLS], BF16, kind="ExternalInput")
    xqT = nc.dram_tensor("xqT", [2, 128, RCHUNK], BF16, kind="ExternalInput")
    xq8d = nc.dram_tensor("xq8", [128, 2 * RPC], F8, kind="ExternalInput")
    wq8d = nc.dram_tensor("wq8", [128, 2 * DIM], F8, kind="ExternalInput")
    wqT = nc.dram_tensor("wqT", [2, 128, DIM], BF16, kind="ExternalInput")
    wkT = nc.dram_tensor("wkT", [2, 128, DIM], BF16, kind="ExternalInput")
    wvT = nc.dram_tensor("wvT", [2, 128, DE], BF16, kind="ExternalInput")
    wk8d = nc.dram_tensor("wk8", [128, 2 * DIM], F8, kind="ExternalInput")
    wv8d = nc.dram_tensor("wv8", [128, 2 * DE], F8, kind="ExternalInput")
    m16d = nc.dram_tensor("m16", [128, BAND * 128], BF16, kind="ExternalInput")
    m8d = nc.dram_tensor("m8", [128, BAND * 128], F8, kind="ExternalInput")
    outd = nc.dram_tensor("out", [RPC, DE], F32, kind="ExternalOutput")
    dbg = None
    if debug:
        dbg = {
            "k8o": nc.dram_tensor("k8o", [128, 2 * S], F8, kind="ExternalOutput"),
            "q8o": nc.dram_tensor("q8o", [128, 2 * RPC], F8, kind="ExternalOutput"),
            "vt8o": nc.dram_tensor("vt8o", [128, NCT * VW], F8, kind="ExternalOutput"),
            "m8o": nc.dram_tensor("m8o", [128, BAND * 128], F8, kind="ExternalOutput"),
            "ptA": nc.dram_tensor("ptA", [128, 1024], F8, kind="ExternalOutput"),
            "ptB": nc.dram_tensor("ptB", [128, 1024], F8, kind="ExternalOutput"),
            "ptC": nc.dram_tensor("ptC", [128, 1024], F8, kind="ExternalOutput"),
            "pv0": nc.dram_tensor("pv0", [128, VW], F32, kind="ExternalOutput"),
            "pv1": nc.dram_tensor("pv1", [128, VW], F32, kind="ExternalOutput"),
        }

    with tile.TileContext(nc) as tc, ExitStack() as ctx:
        const = ctx.enter_context(tc.tile_pool(name="const", bufs=1))
        ps_st = ctx.enter_context(tc.tile_pool(name="ps_st", bufs=3, space="PSUM"))
        ps_pv = ctx.enter_context(tc.tile_pool(name="ps_pv", bufs=1, space="PSUM"))
        pt_pool = ctx.enter_context(tc.tile_pool(name="pt", bufs=9))
        ep_pool = ctx.enter_context(tc.tile_pool(name="ep", bufs=4))

        def body(_iv=None):
            _emit(nc, tc, const, ps_st, ps_pv, pt_pool, ep_pool,
                  x8d, xTd, xqT, xq8d, wq8d, wqT, wkT, wvT, wk8d, wv8d,
                  m16d, m8d, outd, dbg)

        if repeat:
            with tc.For_i(0, repeat, 1) as _iv:
                body(_iv)
        else:
            body()

    nc.compile()
    return nc


def _emit(nc, tc, const, ps_st, ps_pv, pt_pool, ep_pool,
          x8d, xTd, xqT, xq8d, wq8d, wqT, wkT, wvT, wk8d, wv8d,
          m16d, m8d, outd, dbg=None):
    # ---- constants / staged inputs in SBUF ----
    x8_sb = const.tile([128, 2 * S], F8, tag="x8")
    xt_sb = [const.tile([128, BCOLS], BF16, tag=f"xt{i}", name=f"xt{i}") for i in range(2)]
    xq_sb = [const.tile([128, RCHUNK], BF16, tag=f"xq{i}", name=f"xq{i}") for i in range(2)]
    xq8_sb = const.tile([128, 2 * RPC], F8, tag="xq8")
    wq8_sb = const.tile([128, 2 * DIM], F8, tag="wq8")
    wq_sb = const.tile([128, 2 * DIM], BF16, tag="wq")
    wk_sb = const.tile([128, 2 * DIM], BF16, tag="wk")
    wv_sb = const.tile([128, 2 * DE], BF16, tag="wv")
    wk8_sb = const.tile([128, 2 * DIM], F8, tag="wk8")
    wv8_sb = const.tile([128, 2 * DE], F8, tag="wv8")
    m16_sb = const.tile([128, BAND * 128], BF16, tag="m16")
    m8_sb = const.tile([128, BAND * 128], F8, tag="m8")
    k8_sb = const.tile([128, 2 * S], F8, tag="k8")
    q8_sb = const.tile([128, 2 * RPC], F8, tag="q8")
    k16_sb = [const.tile([128, BCOLS], BF16, tag=f"k16_{i}", name=f"k16_{i}")
              for i in range(2)]
    q16_sb = [const.tile([128, RCHUNK], BF16, tag=f"q16_{i}", name=f"q16_{i}")
              for i in range(2)]
    vt8_sb = const.tile([128, NCT * VW], F8, tag="vt8")
    vt16_sb = const.tile([128, BAND * VW], BF16, tag="vt16")

    x8_3 = x8_sb[:].rearrange("p (i c) -> p i c", i=2)     # [128,2,S]
    wk8_3 = wk8_sb[:].rearrange("p (i d) -> p i d", i=2)   # [128,2,256]
    wv8_3 = wv8_sb[:].rearrange("p (i e) -> p i e", i=2)   # [128,2,256]
    k8_3 = k8_sb[:].rearrange("p (i c) -> p i c", i=2)     # [128,2,S]
    q8_3 = q8_sb[:].rearrange("p (i r) -> p i r", i=2)     # [128,2,RPC]
    xq8_3 = xq8_sb[:].rearrange("p (i r) -> p i r", i=2)
    wq8_3 = wq8_sb[:].rearrange("p (i d) -> p i d", i=2)

    # input staging: weights ride the otherwise-idle DVE queue, x.T (bf16)
    # in fine chunks on ACT (K-bf16 consumes it first), x8 immediately on
    # SP in consumption order, xq+masks via gpsimd SWDGE.
    # xq + wq lead the SP queue: Q-gen is the serial prefix of the whole
    # kernel, and the SWDGE path starts ~2us slower than HWDGE
    for i in range(2):
        nc.sync.dma_start(xq_sb[i][:], xqT[i, :, :])
        nc.sync.dma_start(wq_sb[:, i * DIM:(i + 1) * DIM], wqT[i, :, :])
    nc.sync.dma_start(xq8_sb[:], xq8d[:, :])
    nc.sync.dma_start(wq8_sb[:], wq8d[:, :])
    for i in range(2):
        nc.gpsimd.dma_start(wk_sb[:, i * DIM:(i + 1) * DIM], wkT[i, :, :])
        nc.gpsimd.dma_start(wv_sb[:, i * DE:(i + 1) * DE], wvT[i, :, :])
    nc.gpsimd.dma_start(wv8_sb[:], wv8d[:, :])
    nc.gpsimd.dma_start(wk8_sb[:], wk8d[:, :])
    nc.gpsimd.dma_start(m8_sb[:], m8d[:, :])
    nc.gpsimd.dma_start(m16_sb[:], m16d[:, :])
    for o in range(0, BCOLS, 1024):
        for i in range(2):
            nc.scalar.dma_start(xt_sb[i][:, o:o + 1024], xTd[i, :, o:o + 1024])
    XCH = 2048
    for o in range(0, S, XCH):
        for i in range(2):
            nc.sync.dma_start(x8_sb[:, i * S + o: i * S + o + XCH],
                              x8d[:, i * S + o: i * S + o + XCH])
    # per-partition bias vector for the fp8 exp (see below)
    nbias = const.tile([128, 1], F32, tag="nbias")
    nc.vector.memset(nbias[:], -2.0)
    # dummy activation right away: pulls the 1.3us Exp table load into the
    # initial DMA-wait window instead of the first real exp
    warm = const.tile([128, 1], F32, tag="warm")
    nc.scalar.activation(warm[:], nbias[:],
                         mybir.ActivationFunctionType.Exp)
    # ones columns for V.T: only col 256 of each VW-chunk needs the 1.0
    # (on DVE: strided single-element writes, trivially cheap)
    nc.vector.memset(
        vt8_sb[:].rearrange("p (c w) -> p c w", w=VW)[:, :, DE:DE + 1], 1.0)
    nc.vector.memset(
        vt16_sb[:].rearrange("p (c w) -> p c w", w=VW)[:, :, DE:DE + 1], 1.0)

    # round-robin of PSUM->SBUF copy engines; ACT also runs all exps so it
    # gets a lighter share.
    cp_state = [0]

    def cp(dst, src):
        cp_state[0] += 1
        # the first ~12 copies happen before the exp stream ramps up, so
        # ACT can share them evenly; after that ACT is exp-bound and only
        # takes every 8th
        if cp_state[0] <= 18 and cp_state[0] % 2 == 0:
            nc.scalar.copy(dst, src)
        else:
            nc.vector.tensor_copy(dst, src)

    # ---- Q, chunk-0 rows only, in bf16 (the serial prefix) ----
    for d in range(2):
        ps = ps_st.tile([128, 1024], F32, tag="st", name="psq")
        for kd in range(2):
            nc.tensor.matmul(
                ps[:, 0:RCHUNK],
                wq_sb[:, kd * DIM + d * 128: kd * DIM + d * 128 + 128],
                xq_sb[kd][:],
                start=(kd == 0), stop=(kd == 1),
            )
        cp(q16_sb[d][:], ps[:, 0:RCHUNK])

    def _q8_rest():
        # q8 rows 256..1023 via fp8 DoubleRow (chunks 1-3 average >=2048
        # keys, so the fp8 generation noise washes out; rows 0..255 of q8
        # are never read - chunk 0 uses q16)
        for d in range(2):
            ps = ps_st.tile([128, 1024], F32, tag="st", name="psq8")
            for n in range(RCHUNK, RPC, RCHUNK):
                nc.tensor.matmul(
                    ps[:, n:n + RCHUNK],
                    wq8_3[:, :, d * 128:(d + 1) * 128],
                    xq8_3[:, :, n:n + RCHUNK],
                    start=True, stop=True, perf_mode=DR,
                )
            cp(q8_sb[:, d * RPC + RCHUNK:(d + 1) * RPC], ps[:, RCHUNK:RPC])

    # ---- filler closures, interleaved into the attention chunks so their
    # PSUM->SBUF copies (DVE/ACT) overlap the exp-bound phase: bf16 V.T
    # tiles 0..15 into vt16 (chunk-0 PV), fp8 DoubleRow vt8 (all 64
    # tiles), and fp8 K tiles 16..63.
    def _k16_group(d, p):
        # K tiles 0..15 in bf16 (x.T cols < 2048), kept in bf16 for
        # chunk-0's exact ST and gpsimd-quantized to k8 for the fp8 chunks
        def go():
            ps = ps_st.tile([128, 1024], F32, tag="st", name="psk")
            for nn in range(0, 1024, 512):
                for kd in range(2):
                    nc.tensor.matmul(
                        ps[:, nn:nn + 512],
                        wk_sb[:, kd * DIM + d * 128: kd * DIM + d * 128 + 128],
                        xt_sb[kd][:, p + nn:p + nn + 512],
                        start=(kd == 0), stop=(kd == 1),
                    )
            cp(k16_sb[d][:, p:p + 1024], ps[:])
            nc.gpsimd.tensor_copy(k8_sb[:, d * S + p:d * S + p + 1024],
                                  k16_sb[d][:, p:p + 1024])
        return go

    def _v16_group(g4):
        def go():
            ps = ps_st.tile([128, 1024], F32, tag="st", name="psv")
            for jj in range(4):
                j = g4 + jj
                for kd in range(2):
                    nc.tensor.matmul(
                        ps[:, jj * DE:(jj + 1) * DE],
                        xt_sb[kd][:, j * CT:(j + 1) * CT],
                        wv_sb[:, kd * DE:(kd + 1) * DE],
                        start=(kd == 0), stop=(kd == 1),
                    )
            vt_view = vt16_sb[:, g4 * VW:(g4 + 4) * VW].rearrange(
                "p (c w) -> p c w", w=VW)[:, :, 0:DE]
            cp(vt_view, ps[:].rearrange("p (c w) -> p c w", w=DE))
            nc.gpsimd.tensor_copy(vt8_sb[:, g4 * VW:(g4 + 4) * VW],
                                  vt16_sb[:, g4 * VW:(g4 + 4) * VW])
        return go

    def _v8_group(g4):
        def go():
            ps = ps_st.tile([128, 1024], F32, tag="st", name="psv8")
            for jj in range(4):
                j = g4 + jj
                nc.tensor.matmul(
                    ps[:, jj * DE:(jj + 1) * DE],
                    x8_3[:, :, j * CT:(j + 1) * CT],
                    wv8_3,
                    start=True, stop=True, perf_mode=DR,
                )
            vt_view = vt8_sb[:, g4 * VW:(g4 + 4) * VW].rearrange(
                "p (c w) -> p c w", w=VW)[:, :, 0:DE]
            cp(vt_view, ps[:].rearrange("p (c w) -> p c w", w=DE))
        return go

    def _k8_group(d, c0):
        def go():
            ps = ps_st.tile([128, 1024], F32, tag="st", name="psk8")
            for nn in range(0, 1024, 256):
                nc.tensor.matmul(
                    ps[:, nn:nn + 256],
                    wk8_3[:, :, d * 128:(d + 1) * 128],
                    x8_3[:, :, c0 + nn:c0 + nn + 256],
                    start=True, stop=True, perf_mode=DR,
                )
            cp(k8_sb[:, d * S + c0:d * S + c0 + 1024], ps[:])
        return go

    k16_fill = [(p, _k16_group(d, p))
                for p in range(0, BCOLS, 1024) for d in range(2)]
    q8_fill = [_q8_rest]
    v16_fill = [(g4, _v16_group(g4)) for g4 in range(0, BAND, 4)]
    v8_fill = [(g4, _v8_group(g4)) for g4 in range(BAND, NCT, 4)]
    k8_fill = [(c0, _k8_group(d, c0))
               for c0 in range(BCOLS, S, 1024) for d in range(2)]

    def ensure_k16(upto_col):
        while k16_fill and k16_fill[0][0] < upto_col:
            k16_fill.pop(0)[1]()

    def ensure_v16(upto_tile):
        while v16_fill and v16_fill[0][0] < upto_tile:
            v16_fill.pop(0)[1]()

    def ensure_v8(upto_tile):
        while v8_fill and v8_fill[0][0] < upto_tile:
            v8_fill.pop(0)[1]()

    def ensure_k8(upto_col):
        while k8_fill and k8_fill[0][0] < upto_col:
            k8_fill.pop(0)[1]()

    # proportional pacing: spread the filler groups over the 40 ST groups
    # so their PSUM->SBUF copies never pile up on DVE; vt16 first (chunk-0
    # PV drains earliest), then vt8 slightly ahead of k8 (PV trails ST)
    n_fill = (len(k16_fill) + len(v16_fill) + len(v8_fill)
              + len(k8_fill) + len(q8_fill))
    pace = [0]

    def pace_fillers():
        pace[0] += 1
        target = n_fill * pace[0] // 40
        while (len(k16_fill) + len(v16_fill) + len(v8_fill)
               + len(k8_fill) + len(q8_fill)) > n_fill - target:
            if q8_fill:
                q8_fill.pop(0)()
            elif k16_fill:
                k16_fill.pop(0)[1]()
            elif v16_fill:
                v16_fill.pop(0)[1]()
            elif v8_fill and (len(v8_fill) * 2 >= len(k8_fill) or not k8_fill):
                v8_fill.pop(0)[1]()
            elif k8_fill:
                k8_fill.pop(0)[1]()
            else:
                break

    # ---- attention: per row chunk, stream causal col tiles ----
    # col tiles in groups of G: one 2-bank PSUM tile holds G S.T tiles side
    # by side -> a single ACT exp (and a single band mask multiply) covers
    # the whole group, amortizing ACT overhead.
    #
    # software pipeline, depth 4, carried ACROSS chunk boundaries: PV for
    # group g is emitted after the ST matmuls of group g+4, and a chunk's
    # last PVs (plus its epilogue) drain while the next chunk's ST/exp
    # stream is already running.
    G = 4
    from collections import deque
    pending = deque()  # (emit_pv_fn, tail_fn_or_None)

    def drain_one():
        fn, tail = pending.popleft()
        fn()
        if tail is not None:
            tail()

    for r in range(NRC):
        fp8 = r > 0
        ncols = BAND * (r + 1)
        # h=0 (rows 0..127) accumulation ends at col tile 16r+7 (later
        # tiles are fully masked there); h=1 runs to the last tile.
        last_j = {0: min(BAND * r + 7, ncols - 1), 1: ncols - 1}
        pvbox = []

        def get_pv(pvbox=pvbox):
            # lazy: allocated at the first PV drain, which happens after
            # the previous chunk's epilogue has been emitted (bufs=1 ring)
            if not pvbox:
                pvbox.append([
                    ps_pv.tile([128, VW], F32, tag=f"pv{h}", name=f"pv{h}")
                    for h in range(2)
                ])
            return pvbox[0]

        if fp8:
            # DoubleRow PV over col-tile pairs; moving dim split at 128.
            def emit_pv(pt, g, u0, get_pv=get_pv, last_j=last_j):
                ensure_v16(min(g + G, BAND))
                ensure_v8(g + 3 * G)
                pv = get_pv()
                pt3 = pt[:].rearrange("p (c u) -> p c u", u=RCHUNK)
                for t in range(0, G, 2):
                    j = g + t
                    lhs = pt3[:, t:t + 2]  # [128, 2, 256] both tiles
                    rhs = vt8_sb[:, j * VW:(j + 2) * VW].rearrange(
                        "p (c w) -> p c w", w=VW)
                    for h in ((1,) if u0 else (0, 1)):
                        # PSUM start=1 marks the whole 2KB zero region
                        # pending-zero; each write to a pending byte zeroes
                        # then writes. So ONLY the very first inst starts:
                        # the w1 split's first write rides the same mark. A
                        # second start would re-mark w0's bytes and wipe its
                        # pair-0 contribution on the next accumulation.
                        first = (j == 0)
                        last = (j + 1 == last_j[h])
                        nc.tensor.matmul(
                            pv[h][:, 0:128],
                            lhs[:, :, h * 128:h * 128 + 128],
                            rhs[:, :, 0:128],
                            start=first, stop=False, perf_mode=DR,
                            skip_group_check=True,
                        )
                        nc.tensor.matmul(
                            pv[h][:, 128:VW],
                            lhs[:, :, h * 128:h * 128 + 128],
                            rhs[:, :, 128:VW],
                            start=False, stop=last, perf_mode=DR,
                            skip_group_check=True,
                        )
        else:
            def emit_pv(pt, g, u0, get_pv=get_pv, last_j=last_j):
                ensure_v16(min(g + 3 * G, BAND))
                pv = get_pv()
                for t in range(G):
                    j = g + t
                    for h in ((1,) if u0 else (0, 1)):
                        nc.tensor.matmul(
                            pv[h][:],
                            pt[:, t * RCHUNK + h * 128: t * RCHUNK + h * 128 + 128],
                            vt16_sb[:, j * VW:(j + 1) * VW],
                            start=(j == 0), stop=(j == last_j[h]),
                        )

        def epilogue(get_pv=get_pv, r=r):
            pv = get_pv()
            if dbg is not None and r == 1:
                for h in range(2):
                    tl = const.tile([128, VW], F32, tag=f"pv{h}_sb",
                                    name=f"pv{h}_sb")
                    dbg[f"pv{h}_sb"] = tl
                    nc.scalar.copy(tl[:], pv[h][:])
            for h in range(2):
                linv = ep_pool.tile([128, 1], F32, tag="linv")
                nc.vector.reciprocal(linv[:], pv[h][:, DE:DE + 1])
                osb = ep_pool.tile([128, DE], F32, tag="osb")
                nc.vector.tensor_scalar_mul(osb[:], pv[h][:, 0:DE], linv[:])
                rows = r * RCHUNK + h * 128
                nc.sync.dma_start(outd[rows:rows + 128, :], osb[:])

        for g in range(0, ncols, G):
            gb = g - BAND * r
            if r == 0:
                ensure_k16(CT * (g + G))
            else:
                while q8_fill:  # chunks 1-3 read q8
                    q8_fill.pop(0)()
                # prefetch margin: the filler's PSUM->SBUF copy takes ~1.2us,
                # so pull k8 coverage ~2 groups ahead of the ST that reads it
                ensure_k8(CT * (g + 3 * G))
            # when every col tile in the group has j' >= 8, rows 0..127
            # of the chunk are entirely non-causal: compute only the
            # high 128 rows (u0=128) and skip the h=0 PV matmuls.
            u0 = 128 if gb >= 8 else 0
            st = ps_st.tile([128, G * RCHUNK], F32, tag="st")
            for t in range(G):
                j = g + t
                if fp8:
                    nc.tensor.matmul(
                        st[:, t * RCHUNK + u0:(t + 1) * RCHUNK],
                        k8_3[:, :, j * CT:(j + 1) * CT],
                        q8_3[:, :, r * RCHUNK + u0:(r + 1) * RCHUNK],
                        start=True, stop=True, perf_mode=DR,
                    )
                else:
                    # chunk 0 holds the few-key causal rows, most sensitive
                    # to score noise: exact bf16 K/Q there
                    for kd in range(2):
                        nc.tensor.matmul(
                            st[:, t * RCHUNK + u0:(t + 1) * RCHUNK],
                            k16_sb[kd][:, j * CT:(j + 1) * CT],
                            q16_sb[kd][:, u0:RCHUNK],
                            start=(kd == 0), stop=(kd == 1),
                        )
            pace_fillers()
            # ramp the pipeline down toward the end of the last chunk so
            # the tail drain after the final ST group is short
            depth = 6 if not (r == 3 and g >= ncols - 4 * G) else 3
            while len(pending) >= depth:
                drain_one()
            pdt = F8 if fp8 else BF16
            msk = m8_sb if fp8 else m16_sb
            pt = pt_pool.tile([128, G * RCHUNK], pdt, tag="pt8" if fp8 else "pt16")
            # fp8 chunks store P' = exp(s/16 - 2): score outliers reach
            # ~16*6 (heavy |q||k| tails), and exp would overflow fp8e4's
            # 240 max -> inf -> NaN after the 0-mask. The constant bias
            # cancels exactly in the row-sum normalization.
            bias = nbias[:] if fp8 else 0.0
            if u0:
                st_v = st[:].rearrange("p (c w) -> p c w", w=RCHUNK)[:, :, u0:]
                pt_v = pt[:].rearrange("p (c w) -> p c w", w=RCHUNK)[:, :, u0:]
                nc.scalar.activation(
                    pt_v, st_v, mybir.ActivationFunctionType.Exp,
                    scale=0.0625, bias=bias,
                )
            else:
                nc.scalar.activation(
                    pt[:], st[:], mybir.ActivationFunctionType.Exp,
                    scale=0.0625, bias=bias,
                )
            if gb >= 0:
                # diagonal band: only the 128-row window starting at u0 of
                # each tile mixes causal/non-causal entries (rows below are
                # never read thanks to the u0 skip, rows above are fully
                # causal), so the 0/1 mask covers just that window.
                pt_w = pt[:].rearrange(
                    "p (c w) -> p c w", w=RCHUNK)[:, :, u0:u0 + 128]
                mk_w = msk[:, gb * 128:(gb + G) * 128].rearrange(
                    "p (c w) -> p c w", w=128)
                _mask_mul(nc, r, gb, pt_w, mk_w)
            if dbg is not None and r == 1 and g in (0, 16, 24):
                key = {0: "ptA_sb", 16: "ptB_sb", 24: "ptC_sb"}[g]
                tl = const.tile([128, 1024], F8, tag=key, name=key)
                dbg[key] = tl
                nc.gpsimd.tensor_copy(tl[:], pt[:])
            is_last = (g + G >= ncols)
            pending.append((
                lambda pt=pt, g=g, u0=u0, f=emit_pv: f(pt, g, u0),
                epilogue if is_last else None,
            ))
    while pending:
        drain_one()
    if dbg is not None:
        nc.sync.dma_start(dbg["ptA"][:, :], dbg["ptA_sb"][:])
        nc.sync.dma_start(dbg["ptB"][:, :], dbg["ptB_sb"][:])
        nc.sync.dma_start(dbg["ptC"][:, :], dbg["ptC_sb"][:])
        nc.sync.dma_start(dbg["pv0"][:, :], dbg["pv0_sb"][:])
        nc.sync.dma_start(dbg["pv1"][:, :], dbg["pv1_sb"][:])
        nc.sync.dma_start(dbg["k8o"][:, :], k8_sb[:])
        nc.sync.dma_start(dbg["q8o"][:, :], q8_sb[:])
        nc.sync.dma_start(dbg["vt8o"][:, :], vt8_sb[:])
        nc.sync.dma_start(dbg["m8o"][:, :], m8_sb[:])


def _mask_mul(nc, r, gb, pt_v, mk_v):
    # chunks 1-2 run while DVE is saturated with K/V copies -> gpsimd,
    # except the last band groups (gb >= 8) whose PVs drain into the next
    # chunk: the slow gpsimd there would stall the in-order PE at the
    # boundary. chunk 0 (bf16, 2x mode) and chunk 3 (DVE idle) -> DVE.
    if r in (1, 2, 3) and gb < 8:
        nc.gpsimd.tensor_mul(pt_v, pt_v, mk_v)
    else:
        nc.vector.tensor_mul(pt_v, pt_v, mk_v)


def _host_inputs(x, Wq, Wk, Wv):
    xT = np.ascontiguousarray(x.T)                       # [256, 8192] f32
    x8 = np.ascontiguousarray(
        xT.reshape(2, 128, S).transpose(1, 0, 2).reshape(128, 2 * S)
    ).astype(NPF8)
    xT16 = np.ascontiguousarray(xT[:, :BCOLS]).astype(NPBF16).reshape(2, 128, BCOLS)
    wqb = np.ascontiguousarray(Wq.T).astype(NPBF16).reshape(2, 128, DIM)
    wkb = np.ascontiguousarray(Wk.T).astype(NPBF16).reshape(2, 128, DIM)
    wvb = np.ascontiguousarray(Wv.T).astype(NPBF16).reshape(2, 128, DE)
    wk8 = np.ascontiguousarray(
        Wk.T.reshape(2, 128, DIM).transpose(1, 0, 2).reshape(128, 2 * DIM)
    ).astype(NPF8)
    wv8 = np.ascontiguousarray(
        Wv.T.reshape(2, 128, DE).transpose(1, 0, 2).reshape(128, 2 * DE)
    ).astype(NPF8)
    k_idx = np.arange(128)[:, None, None]
    jp = np.arange(BAND)[None, :, None]
    u = 128 * (jp >= 8) + np.arange(128)[None, None, :]
    in_maps = []
    wq8 = np.ascontiguousarray(
        Wq.T.reshape(2, 128, DIM).transpose(1, 0, 2).reshape(128, 2 * DIM)
    ).astype(NPF8)
    for c in range(NCORES):
        xqf = np.ascontiguousarray(x[c::NCORES].T)
        xq = np.ascontiguousarray(
            xqf[:, :RCHUNK]).astype(NPBF16).reshape(2, 128, RCHUNK)
        xq8 = np.ascontiguousarray(
            xqf.reshape(2, 128, RPC).transpose(1, 0, 2).reshape(128, 2 * RPC)
        ).astype(NPF8)
        m = (128 * jp + k_idx <= 8 * u + c)
        m = np.ascontiguousarray(m.reshape(128, BAND * 128))
        in_maps.append({
            "x8": x8, "xT": xT16, "xqT": xq, "xq8": xq8, "wq8": wq8,
            "wqT": wqb, "wkT": wkb,
            "wvT": wvb, "wk8": wk8, "wv8": wv8,
            "m16": m.astype(NPBF16), "m8": m.astype(NPF8),
        })
    return in_maps


def kernel(x, Wq, Wk, Wv, _trace=False, _trace_kwargs=None):
    if "nc" not in _cached:
        _cached["nc"] = _build_nc()
    nc = _cached["nc"]
    in_maps = _host_inputs(
        np.asarray(x, np.float32), np.asarray(Wq, np.float32),
        np.asarray(Wk, np.float32), np.asarray(Wv, np.float32),
    )
    kw = dict(_trace_kwargs or {})
    res = run_bass_kernel_spmd(
        nc, in_maps, core_ids=list(range(NCORES)), trace=_trace, **kw
    )
    out = np.empty((S, DE), np.float32)
    for c in range(NCORES):
        out[c::NCORES] = res.results[c]["out"]
    _cached["last_results"] = res
    return out


# revision 69
# speedup vs baseline: 1.1036x; 1.0014x over previous
"""Causal attention head (S=8192, De=dim=256) on 8 trn2 NeuronCores.

Math (reference):
    Q = Wq @ x.T; K = Wk @ x.T; V = Wv @ x.T
    S = (Q.T @ K) / sqrt(256); causal mask (upper tri -> -inf)
    out = softmax(S, axis=1) @ V.T          # [8192, 256]

Sharding: core c owns rows c::8 (stride-8 interleave) -> every core's
row block has a near-identical causal prefix profile, so the SPMD kernel
is identical across cores; all per-core variation is input data.

Per-core kernel, fp8e4 DoubleRow edition. The PE's fp8 DoubleRow mode
contracts 2x128 partitions per pass at 0.5 cycles/row (4x bf16 for a
256-deep contraction), so the large matmuls run in fp8; the few-key
early causal rows (chunk 0 = local rows 0..255) keep an exact bf16
score+PV path since score noise doesn't average out there.
  - K: col tiles 0..15 bf16-generated (kept as k16 for chunk-0 ST,
    gpsimd-quantized into k8), tiles 16..63 fp8 DoubleRow into k8.
  - V: tiles 0..15 bf16 into vt16 (chunk-0 PV) + gpsimd-converted into
    vt8; tiles 16..63 fp8 DoubleRow straight into vt8.
  - Q bf16 -> q8 (all rows) and q16 (chunk-0 rows).
  - scores: S.T tile = K_j.T @ Q_r as ONE DoubleRow inst per 128-col
    tile for chunks 1-3, bf16 two-inst contraction for chunk 0.
  - exp on ACT: fp8 out with exponent bias -2 for chunks 1-3 (score
    outliers reach ~95, exp(s/16) would overflow fp8e4's 240 max; the
    bias cancels in the row-sum normalization), bf16 out for chunk 0.
  - causal band masks: 0/1 multiply over just the 128-row mixed window
    per tile; gpsimd for early band groups, DVE (2x bf16) otherwise.
  - PV: chunk 0 in bf16 (exact V), chunks 1-3 DoubleRow over col-tile
    pairs, moving dim split (0:128 | 128:257) to stay under the 512
    moving-row ISA limit; ones column yields row sums for free.
  - K/V generation is emitted as "filler" groups interleaved into the
    attention chunks (just-in-time via ensure_*), so their PSUM->SBUF
    copies overlap the exp-bound phase; the softmax pipeline is 6 deep
    and carried across chunk boundaries.
No softmax max-subtraction needed in fp32: |scores/16| <= ~6.
"""

import sys

sys.path.insert(0, "/opt/trn_rl_repo")

from contextlib import ExitStack

import ml_dtypes
import numpy as np

import concourse.bass as bass
import concourse.mybir as mybir
import concourse.tile as tile
from concourse import bacc
from concourse.bass_utils import run_bass_kernel_spmd

BF16 = mybir.dt.bfloat16
F8 = mybir.dt.float8e4
F32 = mybir.dt.float32
NPBF16 = ml_dtypes.bfloat16
NPF8 = ml_dtypes.float8_e4m3
DR = mybir.MatmulPerfMode.DoubleRow

S, DIM, DE = 8192, 256, 256
NCORES = 8
RPC = S // NCORES          # 1024 rows per core
RCHUNK = 256               # rows per S.T matmul (moving free dim)
NRC = RPC // RCHUNK        # 4 row chunks per core
CT = 128                   # col tile (PE partition)
NCT = S // CT              # 64 col tiles total
BAND = 2048 // CT          # 16 col tiles per causal band of a row chunk
VW = DE + 1                # V.T chunk width incl. ones column
BCOLS = 2048               # cols covered by the bf16 K/V path (tiles < 16)

_cached = {}


def _build_nc(repeat=0, debug=False):
    nc = bacc.Bacc("TRN2", target_bir_lowering=False, debug=False,
                   num_devices=NCORES)
    x8d = nc.dram_tensor("x8", [128, 2 * S], F8, kind="ExternalInput")
    xTd = nc.dram_tensor("xT", [2, 128, BCOLS], BF16, kind="ExternalInput")
    xqT = nc.dram_tensor("xqT", [2, 128, RCHUNK], BF16, kind="ExternalInput")
    xq8d = nc.dram_tensor("xq8", [128, 2 * RPC], F8, kind="ExternalInput")
    wq8d = nc.dram_tensor("wq8", [128, 2 * DIM], F8, kind="ExternalInput")
    wqT = nc.dram_tensor("wqT", [2, 128, DIM], BF16, kind="ExternalInput")
    wkT = nc.dram_tensor("wkT", [2, 128, DIM], BF16, kind="ExternalInput")
    wvT = nc.dram_tensor("wvT", [2, 128, DE], BF16, kind="ExternalInput")
    wk8d = nc.dram_tensor("wk8", [128, 2 * DIM], F8, kind="ExternalInput")
    wv8d = nc.dram_tensor("wv8", [128, 2 * DE], F8, kind="ExternalInput")
    m16d = nc.dram_tensor("m16", [128, BAND * 128], BF16, kind="ExternalInput")
    m8d = nc.dram_tensor("m8", [128, BAND * 128], F8, kind="ExternalInput")
    outd = nc.dram_tensor("out", [RPC, DE], F32, kind="ExternalOutput")
    dbg = None
    if debug:
        dbg = {
            "k8o": nc.dram_tensor("k8o", [128, 2 * S], F8, kind="ExternalOutput"),
            "q8o": nc.dram_tensor("q8o", [128, 2 * RPC], F8, kind="ExternalOutput"),
            "vt8o": nc.dram_tensor("vt8o", [128, NCT * VW], F8, kind="ExternalOutput"),
            "m8o": nc.dram_tensor("m8o", [128, BAND * 128], F8, kind="ExternalOutput"),
            "ptA": nc.dram_tensor("ptA", [128, 1024], F8, kind="ExternalOutput"),
            "ptB": nc.dram_tensor("ptB", [128, 1024], F8, kind="ExternalOutput"),
            "ptC": nc.dram_tensor("ptC", [128, 1024], F8, kind="ExternalOutput"),
            "pv0": nc.dram_tensor("pv0", [128, VW], F32, kind="ExternalOutput"),
            "pv1": nc.dram_tensor("pv1", [128, VW], F32, kind="ExternalOutput"),
        }

    with tile.TileContext(nc) as tc, ExitStack() as ctx:
        const = ctx.enter_context(tc.tile_pool(name="const", bufs=1))
        ps_st = ctx.enter_context(tc.tile_pool(name="ps_st", bufs=3, space="PSUM"))
        ps_pv = ctx.enter_context(tc.tile_pool(name="ps_pv", bufs=1, space="PSUM"))
        pt_pool = ctx.enter_context(tc.tile_pool(name="pt", bufs=9))
        ep_pool = ctx.enter_context(tc.tile_pool(name="ep", bufs=4))

        def body(_iv=None):
            _emit(nc, tc, const, ps_st, ps_pv, pt_pool, ep_pool,
                  x8d, xTd, xqT, xq8d, wq8d, wqT, wkT, wvT, wk8d, wv8d,
                  m16d, m8d, outd, dbg)

        if repeat:
            with tc.For_i(0, repeat, 1) as _iv:
                body(_iv)
        else:
            body()

    nc.compile()
    return nc


def _emit(nc, tc, const, ps_st, ps_pv, pt_pool, ep_pool,
          x8d, xTd, xqT, xq8d, wq8d, wqT, wkT, wvT, wk8d, wv8d,
          m16d, m8d, outd, dbg=None):
    # ---- constants / staged inputs in SBUF ----
    x8_sb = const.tile([128, 2 * S], F8, tag="x8")
    xt_sb = [const.tile([128, BCOLS], BF16, tag=f"xt{i}", name=f"xt{i}") for i in range(2)]
    xq_sb = [const.tile([128, RCHUNK], BF16, tag=f"xq{i}", name=f"xq{i}") for i in range(2)]
    xq8_sb = const.tile([128, 2 * RPC], F8, tag="xq8")
    wq8_sb = const.tile([128, 2 * DIM], F8, tag="wq8")
    wq_sb = const.tile([128, 2 * DIM], BF16, tag="wq")
    wk_sb = const.tile([128, 2 * DIM], BF16, tag="wk")
    wv_sb = const.tile([128, 2 * DE], BF16, tag="wv")
    wk8_sb = const.tile([128, 2 * DIM], F8, tag="wk8")
    wv8_sb = const.tile([128, 2 * DE], F8, tag="wv8")
    m16_sb = const.tile([128, BAND * 128], BF16, tag="m16")
    m8_sb = const.tile([128, BAND * 128], F8, tag="m8")
    k8_sb = const.tile([128, 2 * S], F8, tag="k8")
    q8_sb = const.tile([128, 2 * RPC], F8, tag="q8")
    k16_sb = [const.tile([128, BCOLS], BF16, tag=f"k16_{i}", name=f"k16_{i}")
              for i in range(2)]
    q16_sb = [const.tile([128, RCHUNK], BF16, tag=f"q16_{i}", name=f"q16_{i}")
              for i in range(2)]
    vt8_sb = const.tile([128, NCT * VW], F8, tag="vt8")
    vt16_sb = const.tile([128, BAND * VW], BF16, tag="vt16")

    x8_3 = x8_sb[:].rearrange("p (i c) -> p i c", i=2)     # [128,2,S]
    wk8_3 = wk8_sb[:].rearrange("p (i d) -> p i d", i=2)   # [128,2,256]
    wv8_3 = wv8_sb[:].rearrange("p (i e) -> p i e", i=2)   # [128,2,256]
    k8_3 = k8_sb[:].rearrange("p (i c) -> p i c", i=2)     # [128,2,S]
    q8_3 = q8_sb[:].rearrange("p (i r) -> p i r", i=2)     # [128,2,RPC]
    xq8_3 = xq8_sb[:].rearrange("p (i r) -> p i r", i=2)
    wq8_3 = wq8_sb[:].rearrange("p (i d) -> p i d", i=2)

    # input staging: weights ride the otherwise-idle DVE queue, x.T (bf16)
    # in fine chunks on ACT (K-bf16 consumes it first), x8 immediately on
    # SP in consumption order, xq+masks via gpsimd SWDGE.
    # xq + wq lead the SP queue: Q-gen is the serial prefix of the whole
    # kernel, and the SWDGE path starts ~2us slower than HWDGE
    for i in range(2):
        nc.sync.dma_start(xq_sb[i][:], xqT[i, :, :])
        nc.sync.dma_start(wq_sb[:, i * DIM:(i + 1) * DIM], wqT[i, :, :])
    nc.sync.dma_start(xq8_sb[:], xq8d[:, :])
    nc.sync.dma_start(wq8_sb[:], wq8d[:, :])
    for i in range(2):
        nc.gpsimd.dma_start(wk_sb[:, i * DIM:(i + 1) * DIM], wkT[i, :, :])
        nc.gpsimd.dma_start(wv_sb[:, i * DE:(i + 1) * DE], wvT[i, :, :])
    nc.gpsimd.dma_start(wv8_sb[:], wv8d[:, :])
    nc.gpsimd.dma_start(wk8_sb[:], wk8d[:, :])
    nc.gpsimd.dma_start(m8_sb[:], m8d[:, :])
    nc.gpsimd.dma_start(m16_sb[:], m16d[:, :])
    # first xt half on ACT (fast start for k16 p=0); second half on SP
    # so ACT's sequencer is free when the exp stream begins
    for i in range(2):
        nc.scalar.dma_start(xt_sb[i][:, 0:1024], xTd[i, :, 0:1024])
    for i in range(2):
        nc.sync.dma_start(xt_sb[i][:, 1024:BCOLS], xTd[i, :, 1024:BCOLS])
    XCH = 2048
    for o in range(0, S, XCH):
        for i in range(2):
            nc.sync.dma_start(x8_sb[:, i * S + o: i * S + o + XCH],
                              x8d[:, i * S + o: i * S + o + XCH])
    # per-partition bias vector for the fp8 exp (see below)
    nbias = const.tile([128, 1], F32, tag="nbias")
    nc.vector.memset(nbias[:], -2.0)
    # dummy activation right away: pulls the 1.3us Exp table load into the
    # initial DMA-wait window instead of the first real exp
    warm = const.tile([128, 1], F32, tag="warm")
    nc.scalar.activation(warm[:], nbias[:],
                         mybir.ActivationFunctionType.Exp)
    # ones columns for V.T: only col 256 of each VW-chunk needs the 1.0
    # (on DVE: strided single-element writes, trivially cheap)
    nc.vector.memset(
        vt8_sb[:].rearrange("p (c w) -> p c w", w=VW)[:, :, DE:DE + 1], 1.0)
    nc.vector.memset(
        vt16_sb[:].rearrange("p (c w) -> p c w", w=VW)[:, :, DE:DE + 1], 1.0)

    # round-robin of PSUM->SBUF copy engines; ACT also runs all exps so it
    # gets a lighter share.
    cp_state = [0]

    def cp(dst, src):
        cp_state[0] += 1
        # the first ~12 copies happen before the exp stream ramps up, so
        # ACT can share them evenly; after that ACT is exp-bound and only
        # takes every 8th
        if cp_state[0] <= 18 and cp_state[0] % 2 == 0:
            nc.scalar.copy(dst, src)
        else:
            nc.vector.tensor_copy(dst, src)

    # ---- Q, chunk-0 rows only, in bf16 (the serial prefix) ----
    for d in range(2):
        ps = ps_st.tile([128, 1024], F32, tag="st", name="psq")
        for kd in range(2):
            nc.tensor.matmul(
                ps[:, 0:RCHUNK],
                wq_sb[:, kd * DIM + d * 128: kd * DIM + d * 128 + 128],
                xq_sb[kd][:],
                start=(kd == 0), stop=(kd == 1),
            )
        cp(q16_sb[d][:], ps[:, 0:RCHUNK])

    def _q8_rest():
        # q8 rows 256..1023 via fp8 DoubleRow (chunks 1-3 average >=2048
        # keys, so the fp8 generation noise washes out; rows 0..255 of q8
        # are never read - chunk 0 uses q16)
        for d in range(2):
            ps = ps_st.tile([128, 1024], F32, tag="st", name="psq8")
            for n in range(RCHUNK, RPC, RCHUNK):
                nc.tensor.matmul(
                    ps[:, n:n + RCHUNK],
                    wq8_3[:, :, d * 128:(d + 1) * 128],
                    xq8_3[:, :, n:n + RCHUNK],
                    start=True, stop=True, perf_mode=DR,
                )
            cp(q8_sb[:, d * RPC + RCHUNK:(d + 1) * RPC], ps[:, RCHUNK:RPC])

    # ---- filler closures, interleaved into the attention chunks so their
    # PSUM->SBUF copies (DVE/ACT) overlap the exp-bound phase: bf16 V.T
    # tiles 0..15 into vt16 (chunk-0 PV), fp8 DoubleRow vt8 (all 64
    # tiles), and fp8 K tiles 16..63.
    def _k16_group(d, p):
        # K tiles 0..15 in bf16 (x.T cols < 2048), kept in bf16 for
        # chunk-0's exact ST and gpsimd-quantized to k8 for the fp8 chunks
        def go():
            ps = ps_st.tile([128, 1024], F32, tag="st", name="psk")
            for nn in range(0, 1024, 512):
                for kd in range(2):
                    nc.tensor.matmul(
                        ps[:, nn:nn + 512],
                        wk_sb[:, kd * DIM + d * 128: kd * DIM + d * 128 + 128],
                        xt_sb[kd][:, p + nn:p + nn + 512],
                        start=(kd == 0), stop=(kd == 1),
                    )
            cp(k16_sb[d][:, p:p + 1024], ps[:])
            nc.gpsimd.tensor_copy(k8_sb[:, d * S + p:d * S + p + 1024],
                                  k16_sb[d][:, p:p + 1024])
        return go

    def _v16_group(g4):
        def go():
            ps = ps_st.tile([128, 1024], F32, tag="st", name="psv")
            for jj in range(4):
                j = g4 + jj
                for kd in range(2):
                    nc.tensor.matmul(
                        ps[:, jj * DE:(jj + 1) * DE],
                        xt_sb[kd][:, j * CT:(j + 1) * CT],
                        wv_sb[:, kd * DE:(kd + 1) * DE],
                        start=(kd == 0), stop=(kd == 1),
                    )
            vt_view = vt16_sb[:, g4 * VW:(g4 + 4) * VW].rearrange(
                "p (c w) -> p c w", w=VW)[:, :, 0:DE]
            cp(vt_view, ps[:].rearrange("p (c w) -> p c w", w=DE))
            nc.gpsimd.tensor_copy(vt8_sb[:, g4 * VW:(g4 + 4) * VW],
                                  vt16_sb[:, g4 * VW:(g4 + 4) * VW])
        return go

    def _v8_group(g4):
        def go():
            ps = ps_st.tile([128, 1024], F32, tag="st", name="psv8")
            for jj in range(4):
                j = g4 + jj
                nc.tensor.matmul(
                    ps[:, jj * DE:(jj + 1) * DE],
                    x8_3[:, :, j * CT:(j + 1) * CT],
                    wv8_3,
                    start=True, stop=True, perf_mode=DR,
                )
            vt_view = vt8_sb[:, g4 * VW:(g4 + 4) * VW].rearrange(
                "p (c w) -> p c w", w=VW)[:, :, 0:DE]
            cp(vt_view, ps[:].rearrange("p (c w) -> p c w", w=DE))
        return go

    def _k8_group(d, c0):
        def go():
            ps = ps_st.tile([128, 1024], F32, tag="st", name="psk8")
            for nn in range(0, 1024, 256):
                nc.tensor.matmul(
                    ps[:, nn:nn + 256],
                    wk8_3[:, :, d * 128:(d + 1) * 128],
                    x8_3[:, :, c0 + nn:c0 + nn + 256],
                    start=True, stop=True, perf_mode=DR,
                )
            cp(k8_sb[:, d * S + c0:d * S + c0 + 1024], ps[:])
        return go

    k16_fill = [(p, _k16_group(d, p))
                for p in range(0, BCOLS, 1024) for d in range(2)]
    q8_fill = [_q8_rest]
    v16_fill = [(g4, _v16_group(g4)) for g4 in range(0, BAND, 4)]
    v8_fill = [(g4, _v8_group(g4)) for g4 in range(BAND, NCT, 4)]
    k8_fill = [(c0, _k8_group(d, c0))
               for c0 in range(BCOLS, S, 1024) for d in range(2)]

    def ensure_k16(upto_col):
        while k16_fill and k16_fill[0][0] < upto_col:
            k16_fill.pop(0)[1]()

    def ensure_v16(upto_tile):
        while v16_fill and v16_fill[0][0] < upto_tile:
            v16_fill.pop(0)[1]()

    def ensure_v8(upto_tile):
        while v8_fill and v8_fill[0][0] < upto_tile:
            v8_fill.pop(0)[1]()

    def ensure_k8(upto_col):
        while k8_fill and k8_fill[0][0] < upto_col:
            k8_fill.pop(0)[1]()

    # proportional pacing: spread the filler groups over the 40 ST groups
    # so their PSUM->SBUF copies never pile up on DVE; vt16 first (chunk-0
    # PV drains earliest), then vt8 slightly ahead of k8 (PV trails ST)
    n_fill = (len(k16_fill) + len(v16_fill) + len(v8_fill)
              + len(k8_fill) + len(q8_fill))
    pace = [0]

    def pace_fillers():
        pace[0] += 1
        target = n_fill * pace[0] // 40
        while (len(k16_fill) + len(v16_fill) + len(v8_fill)
               + len(k8_fill) + len(q8_fill)) > n_fill - target:
            if q8_fill:
                q8_fill.pop(0)()
            elif k16_fill:
                k16_fill.pop(0)[1]()
            elif v16_fill:
                v16_fill.pop(0)[1]()
            elif v8_fill and (len(v8_fill) * 2 >= len(k8_fill) or not k8_fill):
                v8_fill.pop(0)[1]()
            elif k8_fill:
                k8_fill.pop(0)[1]()
            else:
                break

    # ---- attention: per row chunk, stream causal col tiles ----
    # col tiles in groups of G: one 2-bank PSUM tile holds G S.T tiles side
    # by side -> a single ACT exp (and a single band mask multiply) covers
    # the whole group, amortizing ACT overhead.
    #
    # software pipeline, depth 4, carried ACROSS chunk boundaries: PV for
    # group g is emitted after the ST matmuls of group g+4, and a chunk's
    # last PVs (plus its epilogue) drain while the next chunk's ST/exp
    # stream is already running.
    G = 4
    from collections import deque
    pending = deque()  # (emit_pv_fn, tail_fn_or_None)

    def drain_one():
        fn, tail = pending.popleft()
        fn()
        if tail is not None:
            tail()

    for r in range(NRC):
        fp8 = r > 0
        ncols = BAND * (r + 1)
        # h=0 (rows 0..127) accumulation ends at col tile 16r+7 (later
        # tiles are fully masked there); h=1 runs to the last tile.
        last_j = {0: min(BAND * r + 7, ncols - 1), 1: ncols - 1}
        pvbox = []

        def get_pv(pvbox=pvbox):
            # lazy: allocated at the first PV drain, which happens after
            # the previous chunk's epilogue has been emitted (bufs=1 ring)
            if not pvbox:
                pvbox.append([
                    ps_pv.tile([128, VW], F32, tag=f"pv{h}", name=f"pv{h}")
                    for h in range(2)
                ])
            return pvbox[0]

        if fp8:
            # DoubleRow PV over col-tile pairs; moving dim split at 128.
            def emit_pv(pt, g, u0, get_pv=get_pv, last_j=last_j):
                ensure_v16(min(g + G, BAND))
                ensure_v8(g + 3 * G)
                pv = get_pv()
                pt3 = pt[:].rearrange("p (c u) -> p c u", u=RCHUNK)
                for t in range(0, G, 2):
                    j = g + t
                    lhs = pt3[:, t:t + 2]  # [128, 2, 256] both tiles
                    rhs = vt8_sb[:, j * VW:(j + 2) * VW].rearrange(
                        "p (c w) -> p c w", w=VW)
                    for h in ((1,) if u0 else (0, 1)):
                        # PSUM start=1 marks the whole 2KB zero region
                        # pending-zero; each write to a pending byte zeroes
                        # then writes. So ONLY the very first inst starts:
                        # the w1 split's first write rides the same mark. A
                        # second start would re-mark w0's bytes and wipe its
                        # pair-0 contribution on the next accumulation.
                        first = (j == 0)
                        last = (j + 1 == last_j[h])
                        nc.tensor.matmul(
                            pv[h][:, 0:128],
                            lhs[:, :, h * 128:h * 128 + 128],
                            rhs[:, :, 0:128],
                            start=first, stop=False, perf_mode=DR,
                            skip_group_check=True,
                        )
                        nc.tensor.matmul(
                            pv[h][:, 128:VW],
                            lhs[:, :, h * 128:h * 128 + 128],
                            rhs[:, :, 128:VW],
                            start=False, stop=last, perf_mode=DR,
                            skip_group_check=True,
                        )
        else:
            def emit_pv(pt, g, u0, get_pv=get_pv, last_j=last_j):
                ensure_v16(min(g + 3 * G, BAND))
                pv = get_pv()
                for t in range(G):
                    j = g + t
                    for h in ((1,) if u0 else (0, 1)):
                        nc.tensor.matmul(
                            pv[h][:],
                            pt[:, t * RCHUNK + h * 128: t * RCHUNK + h * 128 + 128],
                            vt16_sb[:, j * VW:(j + 1) * VW],
                            start=(j == 0), stop=(j == last_j[h]),
                        )

        def epilogue(get_pv=get_pv, r=r):
            pv = get_pv()
            if dbg is not None and r == 1:
                for h in range(2):
                    tl = const.tile([128, VW], F32, tag=f"pv{h}_sb",
                                    name=f"pv{h}_sb")
                    dbg[f"pv{h}_sb"] = tl
                    nc.scalar.copy(tl[:], pv[h][:])
            for h in range(2):
                linv = ep_pool.tile([128, 1], F32, tag="linv")
                nc.vector.reciprocal(linv[:], pv[h][:, DE:DE + 1])
                osb = ep_pool.tile([128, DE], F32, tag="osb")
                nc.vector.tensor_scalar_mul(osb[:], pv[h][:, 0:DE], linv[:])
                rows = r * RCHUNK + h * 128
                nc.sync.dma_start(outd[rows:rows + 128, :], osb[:])

        for g in range(0, ncols, G):
            gb = g - BAND * r
            if r == 0:
                ensure_k16(CT * (g + G))
            else:
                while q8_fill:  # chunks 1-3 read q8
                    q8_fill.pop(0)()
                # prefetch margin: the filler's PSUM->SBUF copy takes ~1.2us,
                # so pull k8 coverage ~2 groups ahead of the ST that reads it
                ensure_k8(CT * (g + 3 * G))
            # when every col tile in the group has j' >= 8, rows 0..127
            # of the chunk are entirely non-causal: compute only the
            # high 128 rows (u0=128) and skip the h=0 PV matmuls.
            u0 = 128 if gb >= 8 else 0
            st = ps_st.tile([128, G * RCHUNK], F32, tag="st")
            for t in range(G):
                j = g + t
                if fp8:
                    nc.tensor.matmul(
                        st[:, t * RCHUNK + u0:(t + 1) * RCHUNK],
                        k8_3[:, :, j * CT:(j + 1) * CT],
                        q8_3[:, :, r * RCHUNK + u0:(r + 1) * RCHUNK],
                        start=True, stop=True, perf_mode=DR,
                    )
                else:
                    # chunk 0 holds the few-key causal rows, most sensitive
                    # to score noise: exact bf16 K/Q there
                    for kd in range(2):
                        nc.tensor.matmul(
                            st[:, t * RCHUNK + u0:(t + 1) * RCHUNK],
                            k16_sb[kd][:, j * CT:(j + 1) * CT],
                            q16_sb[kd][:, u0:RCHUNK],
                            start=(kd == 0), stop=(kd == 1),
                        )
            pace_fillers()
            # ramp the pipeline down toward the end of the last chunk so
            # the tail drain after the final ST group is short
            depth = 6 if not (r == 3 and g >= ncols - 4 * G) else 3
            while len(pending) >= depth:
                drain_one()
            pdt = F8 if fp8 else BF16
            msk = m8_sb if fp8 else m16_sb
            pt = pt_pool.tile([128, G * RCHUNK], pdt, tag="pt8" if fp8 else "pt16")
            # fp8 chunks store P' = exp(s/16 - 2): score outliers reach
            # ~16*6 (heavy |q||k| tails), and exp would overflow fp8e4's
            # 240 max -> inf -> NaN after the 0-mask. The constant bias
            # cancels exactly in the row-sum normalization.
            bias = nbias[:] if fp8 else 0.0
            if u0:
                st_v = st[:].rearrange("p (c w) -> p c w", w=RCHUNK)[:, :, u0:]
                pt_v = pt[:].rearrange("p (c w) -> p c w", w=RCHUNK)[:, :, u0:]
                nc.scalar.activation(
                    pt_v, st_v, mybir.ActivationFunctionType.Exp,
                    scale=0.0625, bias=bias,
                )
            else:
                nc.scalar.activation(
                    pt[:], st[:], mybir.ActivationFunctionType.Exp,
                    scale=0.0625, bias=bias,
                )
            if gb >= 0:
                # diagonal band: only the 128-row window starting at u0 of
                # each tile mixes causal/non-causal entries (rows below are
                # never read thanks to the u0 skip, rows above are fully
                # causal), so the 0/1 mask covers just that window.
                pt_w = pt[:].rearrange(
                    "p (c w) -> p c w", w=RCHUNK)[:, :, u0:u0 + 128]
                mk_w = msk[:, gb * 128:(gb + G) * 128].rearrange(
                    "p (c w) -> p c w", w=128)
                _mask_mul(nc, r, gb, pt_w, mk_w)
            if dbg is not None and r == 1 and g in (0, 16, 24):
                key = {0: "ptA_sb", 16: "ptB_sb", 24: "ptC_sb"}[g]
                tl = const.tile([128, 1024], F8, tag=key, name=key)
                dbg[key] = tl
                nc.gpsimd.tensor_copy(tl[:], pt[:])
            is_last = (g + G >= ncols)
            pending.append((
                lambda pt=pt, g=g, u0=u0, f=emit_pv: f(pt, g, u0),
                epilogue if is_last else None,
            ))
    while pending:
        drain_one()
    if dbg is not None:
        nc.sync.dma_start(dbg["ptA"][:, :], dbg["ptA_sb"][:])
        nc.sync.dma_start(dbg["ptB"][:, :], dbg["ptB_sb"][:])
        nc.sync.dma_start(dbg["ptC"][:, :], dbg["ptC_sb"][:])
        nc.sync.dma_start(dbg["pv0"][:, :], dbg["pv0_sb"][:])
        nc.sync.dma_start(dbg["pv1"][:, :], dbg["pv1_sb"][:])
        nc.sync.dma_start(dbg["k8o"][:, :], k8_sb[:])
        nc.sync.dma_start(dbg["q8o"][:, :], q8_sb[:])
        nc.sync.dma_start(dbg["vt8o"][:, :], vt8_sb[:])
        nc.sync.dma_start(dbg["m8o"][:, :], m8_sb[:])


def _mask_mul(nc, r, gb, pt_v, mk_v):
    # chunks 1-2 run while DVE is saturated with K/V copies -> gpsimd,
    # except the last band groups (gb >= 8) whose PVs drain into the next
    # chunk: the slow gpsimd there would stall the in-order PE at the
    # boundary. chunk 0 (bf16, 2x mode) and chunk 3 (DVE idle) -> DVE.
    if r in (1, 2, 3) and gb < 8:
        nc.gpsimd.tensor_mul(pt_v, pt_v, mk_v)
    else:
        nc.vector.tensor_mul(pt_v, pt_v, mk_v)


def _host_inputs(x, Wq, Wk, Wv):
    xT = np.ascontiguousarray(x.T)                       # [256, 8192] f32
    x8 = np.ascontiguousarray(
        xT.reshape(2, 128, S).transpose(1, 0, 2).reshape(128, 2 * S)
    ).astype(NPF8)
    xT16 = np.ascontiguousarray(xT[:, :BCOLS]).astype(NPBF16).reshape(2, 128, BCOLS)
    wqb = np.ascontiguousarray(Wq.T).astype(NPBF16).reshape(2, 128, DIM)
    wkb = np.ascontiguousarray(Wk.T).astype(NPBF16).reshape(2, 128, DIM)
    wvb = np.ascontiguousarray(Wv.T).astype(NPBF16).reshape(2, 128, DE)
    wk8 = np.ascontiguousarray(
        Wk.T.reshape(2, 128, DIM).transpose(1, 0, 2).reshape(128, 2 * DIM)
    ).astype(NPF8)
    wv8 = np.ascontiguousarray(
        Wv.T.reshape(2, 128, DE).transpose(1, 0, 2).reshape(128, 2 * DE)
    ).astype(NPF8)
    k_idx = np.arange(128)[:, None, None]
    jp = np.arange(BAND)[None, :, None]
    u = 128 * (jp >= 8) + np.arange(128)[None, None, :]
    in_maps = []
    wq8 = np.ascontiguousarray(
        Wq.T.reshape(2, 128, DIM).transpose(1, 0, 2).reshape(128, 2 * DIM)
    ).astype(NPF8)
    for c in range(NCORES):
        xqf = np.ascontiguousarray(x[c::NCORES].T)
        xq = np.ascontiguousarray(
            xqf[:, :RCHUNK]).astype(NPBF16).reshape(2, 128, RCHUNK)
        xq8 = np.ascontiguousarray(
            xqf.reshape(2, 128, RPC).transpose(1, 0, 2).reshape(128, 2 * RPC)
        ).astype(NPF8)
        m = (128 * jp + k_idx <= 8 * u + c)
        m = np.ascontiguousarray(m.reshape(128, BAND * 128))
        in_maps.append({
            "x8": x8, "xT": xT16, "xqT": xq, "xq8": xq8, "wq8": wq8,
            "wqT": wqb, "wkT": wkb,
            "wvT": wvb, "wk8": wk8, "wv8": wv8,
            "m16": m.astype(NPBF16), "m8": m.astype(NPF8),
        })
    return in_maps


def kernel(x, Wq, Wk, Wv, _trace=False, _trace_kwargs=None):
    if "nc" not in _cached:
        _cached["nc"] = _build_nc()
    nc = _cached["nc"]
    in_maps = _host_inputs(
        np.asarray(x, np.float32), np.asarray(Wq, np.float32),
        np.asarray(Wk, np.float32), np.asarray(Wv, np.float32),
    )
    kw = dict(_trace_kwargs or {})
    res = run_bass_kernel_spmd(
        nc, in_maps, core_ids=list(range(NCORES)), trace=_trace, **kw
    )
    out = np.empty((S, DE), np.float32)
    for c in range(NCORES):
        out[c::NCORES] = res.results[c]["out"]
    _cached["last_results"] = res
    return out


# revision 70
# speedup vs baseline: 1.1178x; 1.0129x over previous
"""Causal attention head (S=8192, De=dim=256) on 8 trn2 NeuronCores.

Math (reference):
    Q = Wq @ x.T; K = Wk @ x.T; V = Wv @ x.T
    S = (Q.T @ K) / sqrt(256); causal mask (upper tri -> -inf)
    out = softmax(S, axis=1) @ V.T          # [8192, 256]

Sharding: core c owns rows c::8 (stride-8 interleave) -> every core's
row block has a near-identical causal prefix profile, so the SPMD kernel
is identical across cores; all per-core variation is input data.

Per-core kernel, fp8e4 DoubleRow edition. The PE's fp8 DoubleRow mode
contracts 2x128 partitions per pass at 0.5 cycles/row (4x bf16 for a
256-deep contraction), so the large matmuls run in fp8; the few-key
early causal rows (chunk 0 = local rows 0..255) keep an exact bf16
score+PV path since score noise doesn't average out there.
  - K: col tiles 0..15 bf16-generated (kept as k16 for chunk-0 ST,
    gpsimd-quantized into k8), tiles 16..63 fp8 DoubleRow into k8.
  - V: tiles 0..15 bf16 into vt16 (chunk-0 PV) + gpsimd-converted into
    vt8; tiles 16..63 fp8 DoubleRow straight into vt8.
  - Q bf16 -> q8 (all rows) and q16 (chunk-0 rows).
  - scores: S.T tile = K_j.T @ Q_r as ONE DoubleRow inst per 128-col
    tile for chunks 1-3, bf16 two-inst contraction for chunk 0.
  - exp on ACT: fp8 out with exponent bias -2 for chunks 1-3 (score
    outliers reach ~95, exp(s/16) would overflow fp8e4's 240 max; the
    bias cancels in the row-sum normalization), bf16 out for chunk 0.
  - causal band masks: 0/1 multiply over just the 128-row mixed window
    per tile; gpsimd for early band groups, DVE (2x bf16) otherwise.
  - PV: chunk 0 in bf16 (exact V), chunks 1-3 DoubleRow over col-tile
    pairs, moving dim split (0:128 | 128:257) to stay under the 512
    moving-row ISA limit; ones column yields row sums for free.
  - K/V generation is emitted as "filler" groups interleaved into the
    attention chunks (just-in-time via ensure_*), so their PSUM->SBUF
    copies overlap the exp-bound phase; the softmax pipeline is 6 deep
    and carried across chunk boundaries.
No softmax max-subtraction needed in fp32: |scores/16| <= ~6.
"""

import sys

sys.path.insert(0, "/opt/trn_rl_repo")

from contextlib import ExitStack

import ml_dtypes
import numpy as np

import concourse.bass as bass
import concourse.mybir as mybir
import concourse.tile as tile
from concourse import bacc
from concourse.bass_utils import run_bass_kernel_spmd

BF16 = mybir.dt.bfloat16
F8 = mybir.dt.float8e4
F32 = mybir.dt.float32
NPBF16 = ml_dtypes.bfloat16
NPF8 = ml_dtypes.float8_e4m3
DR = mybir.MatmulPerfMode.DoubleRow

S, DIM, DE = 8192, 256, 256
NCORES = 8
RPC = S // NCORES          # 1024 rows per core
RCHUNK = 256               # rows per S.T matmul (moving free dim)
NRC = RPC // RCHUNK        # 4 row chunks per core
CT = 128                   # col tile (PE partition)
NCT = S // CT              # 64 col tiles total
BAND = 2048 // CT          # 16 col tiles per causal band of a row chunk
VW = DE + 1                # V.T chunk width incl. ones column
BCOLS = 2048               # cols covered by the bf16 K/V path (tiles < 16)

_cached = {}


def _build_nc(repeat=0, debug=False):
    nc = bacc.Bacc("TRN2", target_bir_lowering=False, debug=False,
                   num_devices=NCORES)
    x8d = nc.dram_tensor("x8", [128, 2 * S], F8, kind="ExternalInput")
    xTd = nc.dram_tensor("xT", [2, 128, BCOLS], BF16, kind="ExternalInput")
    xqT = nc.dram_tensor("xqT", [2, 128, RCHUNK], BF16, kind="ExternalInput")
    xq8d = nc.dram_tensor("xq8", [128, 2 * RPC], F8, kind="ExternalInput")
    wq8d = nc.dram_tensor("wq8", [128, 2 * DIM], F8, kind="ExternalInput")
    wqT = nc.dram_tensor("wqT", [2, 128, DIM], BF16, kind="ExternalInput")
    wkT = nc.dram_tensor("wkT", [2, 128, DIM], BF16, kind="ExternalInput")
    wvT = nc.dram_tensor("wvT", [2, 128, DE], BF16, kind="ExternalInput")
    wk8d = nc.dram_tensor("wk8", [128, 2 * DIM], F8, kind="ExternalInput")
    wv8d = nc.dram_tensor("wv8", [128, 2 * DE], F8, kind="ExternalInput")
    m16d = nc.dram_tensor("m16", [128, BAND * 128], BF16, kind="ExternalInput")
    m8d = nc.dram_tensor("m8", [128, BAND * 128], F8, kind="ExternalInput")
    outd = nc.dram_tensor("out", [RPC, DE], F32, kind="ExternalOutput")
    dbg = None
    if debug:
        dbg = {
            "k8o": nc.dram_tensor("k8o", [128, 2 * S], F8, kind="ExternalOutput"),
            "q8o": nc.dram_tensor("q8o", [128, 2 * RPC], F8, kind="ExternalOutput"),
            "vt8o": nc.dram_tensor("vt8o", [128, NCT * VW], F8, kind="ExternalOutput"),
            "m8o": nc.dram_tensor("m8o", [128, BAND * 128], F8, kind="ExternalOutput"),
            "ptA": nc.dram_tensor("ptA", [128, 1024], F8, kind="ExternalOutput"),
            "ptB": nc.dram_tensor("ptB", [128, 1024], F8, kind="ExternalOutput"),
            "ptC": nc.dram_tensor("ptC", [128, 1024], F8, kind="ExternalOutput"),
            "pv0": nc.dram_tensor("pv0", [128, VW], F32, kind="ExternalOutput"),
            "pv1": nc.dram_tensor("pv1", [128, VW], F32, kind="ExternalOutput"),
        }

    with tile.TileContext(nc) as tc, ExitStack() as ctx:
        const = ctx.enter_context(tc.tile_pool(name="const", bufs=1))
        ps_st = ctx.enter_context(tc.tile_pool(name="ps_st", bufs=3, space="PSUM"))
        ps_pv = ctx.enter_context(tc.tile_pool(name="ps_pv", bufs=1, space="PSUM"))
        pt_pool = ctx.enter_context(tc.tile_pool(name="pt", bufs=9))
        ep_pool = ctx.enter_context(tc.tile_pool(name="ep", bufs=4))

        def body(_iv=None):
            _emit(nc, tc, const, ps_st, ps_pv, pt_pool, ep_pool,
                  x8d, xTd, xqT, xq8d, wq8d, wqT, wkT, wvT, wk8d, wv8d,
                  m16d, m8d, outd, dbg)

        if repeat:
            with tc.For_i(0, repeat, 1) as _iv:
                body(_iv)
        else:
            body()

    nc.compile()
    return nc


def _emit(nc, tc, const, ps_st, ps_pv, pt_pool, ep_pool,
          x8d, xTd, xqT, xq8d, wq8d, wqT, wkT, wvT, wk8d, wv8d,
          m16d, m8d, outd, dbg=None):
    # ---- constants / staged inputs in SBUF ----
    x8_sb = const.tile([128, 2 * S], F8, tag="x8")
    xt_sb = [const.tile([128, BCOLS], BF16, tag=f"xt{i}", name=f"xt{i}") for i in range(2)]
    xq_sb = [const.tile([128, RCHUNK], BF16, tag=f"xq{i}", name=f"xq{i}") for i in range(2)]
    xq8_sb = const.tile([128, 2 * RPC], F8, tag="xq8")
    wq8_sb = const.tile([128, 2 * DIM], F8, tag="wq8")
    wq_sb = const.tile([128, 2 * DIM], BF16, tag="wq")
    wk_sb = const.tile([128, 2 * DIM], BF16, tag="wk")
    wv_sb = const.tile([128, 2 * DE], BF16, tag="wv")
    wk8_sb = const.tile([128, 2 * DIM], F8, tag="wk8")
    wv8_sb = const.tile([128, 2 * DE], F8, tag="wv8")
    m16_sb = const.tile([128, BAND * 128], BF16, tag="m16")
    m8_sb = const.tile([128, BAND * 128], F8, tag="m8")
    k8_sb = const.tile([128, 2 * S], F8, tag="k8")
    q8_sb = const.tile([128, 2 * RPC], F8, tag="q8")
    k16_sb = [const.tile([128, BCOLS], BF16, tag=f"k16_{i}", name=f"k16_{i}")
              for i in range(2)]
    q16_sb = [const.tile([128, RCHUNK], BF16, tag=f"q16_{i}", name=f"q16_{i}")
              for i in range(2)]
    vt8_sb = const.tile([128, NCT * VW], F8, tag="vt8")
    vt16_sb = const.tile([128, BAND * VW], BF16, tag="vt16")

    x8_3 = x8_sb[:].rearrange("p (i c) -> p i c", i=2)     # [128,2,S]
    wk8_3 = wk8_sb[:].rearrange("p (i d) -> p i d", i=2)   # [128,2,256]
    wv8_3 = wv8_sb[:].rearrange("p (i e) -> p i e", i=2)   # [128,2,256]
    k8_3 = k8_sb[:].rearrange("p (i c) -> p i c", i=2)     # [128,2,S]
    q8_3 = q8_sb[:].rearrange("p (i r) -> p i r", i=2)     # [128,2,RPC]
    xq8_3 = xq8_sb[:].rearrange("p (i r) -> p i r", i=2)
    wq8_3 = wq8_sb[:].rearrange("p (i d) -> p i d", i=2)

    # input staging: weights ride the otherwise-idle DVE queue, x.T (bf16)
    # in fine chunks on ACT (K-bf16 consumes it first), x8 immediately on
    # SP in consumption order, xq+masks via gpsimd SWDGE.
    # xq + wq lead the SP queue: Q-gen is the serial prefix of the whole
    # kernel, and the SWDGE path starts ~2us slower than HWDGE
    for i in range(2):
        nc.sync.dma_start(xq_sb[i][:], xqT[i, :, :])
        nc.sync.dma_start(wq_sb[:, i * DIM:(i + 1) * DIM], wqT[i, :, :])
    nc.sync.dma_start(xq8_sb[:], xq8d[:, :])
    nc.sync.dma_start(wq8_sb[:], wq8d[:, :])
    for i in range(2):
        nc.gpsimd.dma_start(wk_sb[:, i * DIM:(i + 1) * DIM], wkT[i, :, :])
        nc.gpsimd.dma_start(wv_sb[:, i * DE:(i + 1) * DE], wvT[i, :, :])
    nc.gpsimd.dma_start(wv8_sb[:], wv8d[:, :])
    nc.gpsimd.dma_start(wk8_sb[:], wk8d[:, :])
    nc.gpsimd.dma_start(m8_sb[:], m8d[:, :])
    nc.gpsimd.dma_start(m16_sb[:], m16d[:, :])
    # first xt half on ACT (fast start for k16 p=0); second half on SP
    # so ACT's sequencer is free when the exp stream begins
    for i in range(2):
        nc.scalar.dma_start(xt_sb[i][:, 0:1024], xTd[i, :, 0:1024])
    for i in range(2):
        nc.sync.dma_start(xt_sb[i][:, 1024:BCOLS], xTd[i, :, 1024:BCOLS])
    XCH = 2048
    for o in range(0, S, XCH):
        for i in range(2):
            nc.sync.dma_start(x8_sb[:, i * S + o: i * S + o + XCH],
                              x8d[:, i * S + o: i * S + o + XCH])
    # per-partition bias vector for the fp8 exp (see below)
    nbias = const.tile([128, 1], F32, tag="nbias")
    nc.vector.memset(nbias[:], -2.0)
    # dummy activation right away: pulls the 1.3us Exp table load into the
    # initial DMA-wait window instead of the first real exp
    warm = const.tile([128, 1], F32, tag="warm")
    nc.scalar.activation(warm[:], nbias[:],
                         mybir.ActivationFunctionType.Exp)
    # ones columns for V.T: only col 256 of each VW-chunk needs the 1.0
    # (on DVE: strided single-element writes, trivially cheap)
    nc.vector.memset(
        vt8_sb[:].rearrange("p (c w) -> p c w", w=VW)[:, :, DE:DE + 1], 1.0)
    nc.vector.memset(
        vt16_sb[:].rearrange("p (c w) -> p c w", w=VW)[:, :, DE:DE + 1], 1.0)

    # round-robin of PSUM->SBUF copy engines; ACT also runs all exps so it
    # gets a lighter share.
    cp_state = [0]

    def cp(dst, src):
        cp_state[0] += 1
        # the first ~12 copies happen before the exp stream ramps up, so
        # ACT can share them evenly; after that ACT is exp-bound and only
        # takes every 8th
        if cp_state[0] <= 18 and cp_state[0] % 2 == 0:
            nc.scalar.copy(dst, src)
        else:
            nc.vector.tensor_copy(dst, src)

    # ---- Q, chunk-0 rows only, in bf16 (the serial prefix) ----
    for d in range(2):
        ps = ps_st.tile([128, 1024], F32, tag="st", name="psq")
        for kd in range(2):
            nc.tensor.matmul(
                ps[:, 0:RCHUNK],
                wq_sb[:, kd * DIM + d * 128: kd * DIM + d * 128 + 128],
                xq_sb[kd][:],
                start=(kd == 0), stop=(kd == 1),
            )
        cp(q16_sb[d][:], ps[:, 0:RCHUNK])

    def _q8_rest():
        # q8 rows 256..1023 via fp8 DoubleRow (chunks 1-3 average >=2048
        # keys, so the fp8 generation noise washes out; rows 0..255 of q8
        # are never read - chunk 0 uses q16)
        for d in range(2):
            ps = ps_st.tile([128, 1024], F32, tag="st", name="psq8")
            for n in range(RCHUNK, RPC, RCHUNK):
                nc.tensor.matmul(
                    ps[:, n:n + RCHUNK],
                    wq8_3[:, :, d * 128:(d + 1) * 128],
                    xq8_3[:, :, n:n + RCHUNK],
                    start=True, stop=True, perf_mode=DR,
                )
            cp(q8_sb[:, d * RPC + RCHUNK:(d + 1) * RPC], ps[:, RCHUNK:RPC])

    # ---- filler closures, interleaved into the attention chunks so their
    # PSUM->SBUF copies (DVE/ACT) overlap the exp-bound phase: bf16 V.T
    # tiles 0..15 into vt16 (chunk-0 PV), fp8 DoubleRow vt8 (all 64
    # tiles), and fp8 K tiles 16..63.
    def _k16_group(d, p):
        # K tiles 0..15 in bf16 (x.T cols < 2048), kept in bf16 for
        # chunk-0's exact ST and gpsimd-quantized to k8 for the fp8 chunks
        def go():
            ps = ps_st.tile([128, 1024], F32, tag="st", name="psk")
            for nn in range(0, 1024, 512):
                for kd in range(2):
                    nc.tensor.matmul(
                        ps[:, nn:nn + 512],
                        wk_sb[:, kd * DIM + d * 128: kd * DIM + d * 128 + 128],
                        xt_sb[kd][:, p + nn:p + nn + 512],
                        start=(kd == 0), stop=(kd == 1),
                    )
            cp(k16_sb[d][:, p:p + 1024], ps[:])
            nc.gpsimd.tensor_copy(k8_sb[:, d * S + p:d * S + p + 1024],
                                  k16_sb[d][:, p:p + 1024])
        return go

    def _v16_group(g4):
        def go():
            ps = ps_st.tile([128, 1024], F32, tag="st", name="psv")
            for jj in range(4):
                j = g4 + jj
                for kd in range(2):
                    nc.tensor.matmul(
                        ps[:, jj * DE:(jj + 1) * DE],
                        xt_sb[kd][:, j * CT:(j + 1) * CT],
                        wv_sb[:, kd * DE:(kd + 1) * DE],
                        start=(kd == 0), stop=(kd == 1),
                    )
            vt_view = vt16_sb[:, g4 * VW:(g4 + 4) * VW].rearrange(
                "p (c w) -> p c w", w=VW)[:, :, 0:DE]
            cp(vt_view, ps[:].rearrange("p (c w) -> p c w", w=DE))
            nc.gpsimd.tensor_copy(vt8_sb[:, g4 * VW:(g4 + 4) * VW],
                                  vt16_sb[:, g4 * VW:(g4 + 4) * VW])
        return go

    def _v8_group(g4):
        def go():
            ps = ps_st.tile([128, 1024], F32, tag="st", name="psv8")
            for jj in range(4):
                j = g4 + jj
                nc.tensor.matmul(
                    ps[:, jj * DE:(jj + 1) * DE],
                    x8_3[:, :, j * CT:(j + 1) * CT],
                    wv8_3,
                    start=True, stop=True, perf_mode=DR,
                )
            vt_view = vt8_sb[:, g4 * VW:(g4 + 4) * VW].rearrange(
                "p (c w) -> p c w", w=VW)[:, :, 0:DE]
            cp(vt_view, ps[:].rearrange("p (c w) -> p c w", w=DE))
        return go

    def _k8_group(d, c0):
        def go():
            ps = ps_st.tile([128, 1024], F32, tag="st", name="psk8")
            for nn in range(0, 1024, 256):
                nc.tensor.matmul(
                    ps[:, nn:nn + 256],
                    wk8_3[:, :, d * 128:(d + 1) * 128],
                    x8_3[:, :, c0 + nn:c0 + nn + 256],
                    start=True, stop=True, perf_mode=DR,
                )
            cp(k8_sb[:, d * S + c0:d * S + c0 + 1024], ps[:])
        return go

    k16_fill = [(p, _k16_group(d, p))
                for p in range(0, BCOLS, 1024) for d in range(2)]
    q8_fill = [_q8_rest]
    v16_fill = [(g4, _v16_group(g4)) for g4 in range(0, BAND, 4)]
    v8_fill = [(g4, _v8_group(g4)) for g4 in range(BAND, NCT, 4)]
    k8_fill = [(c0, _k8_group(d, c0))
               for c0 in range(BCOLS, S, 1024) for d in range(2)]

    def ensure_k16(upto_col):
        while k16_fill and k16_fill[0][0] < upto_col:
            k16_fill.pop(0)[1]()

    def ensure_v16(upto_tile):
        while v16_fill and v16_fill[0][0] < upto_tile:
            v16_fill.pop(0)[1]()

    def ensure_v8(upto_tile):
        while v8_fill and v8_fill[0][0] < upto_tile:
            v8_fill.pop(0)[1]()

    def ensure_k8(upto_col):
        while k8_fill and k8_fill[0][0] < upto_col:
            k8_fill.pop(0)[1]()

    # proportional pacing: spread the filler groups over the 40 ST groups
    # so their PSUM->SBUF copies never pile up on DVE; vt16 first (chunk-0
    # PV drains earliest), then vt8 slightly ahead of k8 (PV trails ST)
    n_fill = (len(k16_fill) + len(v16_fill) + len(v8_fill)
              + len(k8_fill) + len(q8_fill))
    pace = [0]

    def pace_fillers():
        pace[0] += 1
        target = n_fill * pace[0] // 40
        while (len(k16_fill) + len(v16_fill) + len(v8_fill)
               + len(k8_fill) + len(q8_fill)) > n_fill - target:
            if q8_fill:
                q8_fill.pop(0)()
            elif k16_fill:
                k16_fill.pop(0)[1]()
            elif v16_fill:
                v16_fill.pop(0)[1]()
            elif v8_fill and (len(v8_fill) * 2 >= len(k8_fill) or not k8_fill):
                v8_fill.pop(0)[1]()
            elif k8_fill:
                k8_fill.pop(0)[1]()
            else:
                break

    # ---- attention: per row chunk, stream causal col tiles ----
    # col tiles in groups of G: one 2-bank PSUM tile holds G S.T tiles side
    # by side -> a single ACT exp (and a single band mask multiply) covers
    # the whole group, amortizing ACT overhead.
    #
    # software pipeline, depth 4, carried ACROSS chunk boundaries: PV for
    # group g is emitted after the ST matmuls of group g+4, and a chunk's
    # last PVs (plus its epilogue) drain while the next chunk's ST/exp
    # stream is already running.
    G = 4
    from collections import deque
    pending = deque()  # (emit_pv_fn, tail_fn_or_None)

    def drain_one():
        fn, tail = pending.popleft()
        fn()
        if tail is not None:
            tail()

    for r in range(NRC):
        fp8 = r > 0
        ncols = BAND * (r + 1)
        # h=0 (rows 0..127) accumulation ends at col tile 16r+7 (later
        # tiles are fully masked there); h=1 runs to the last tile.
        last_j = {0: min(BAND * r + 7, ncols - 1), 1: ncols - 1}
        pvbox = []

        def get_pv(pvbox=pvbox):
            # lazy: allocated at the first PV drain, which happens after
            # the previous chunk's epilogue has been emitted (bufs=1 ring)
            if not pvbox:
                pvbox.append([
                    ps_pv.tile([128, VW], F32, tag=f"pv{h}", name=f"pv{h}")
                    for h in range(2)
                ])
            return pvbox[0]

        if fp8:
            # DoubleRow PV over col-tile pairs; moving dim split at 128.
            def emit_pv(pt, g, u0, get_pv=get_pv, last_j=last_j):
                ensure_v16(min(g + G, BAND))
                ensure_v8(g + 3 * G)
                pv = get_pv()
                pt3 = pt[:].rearrange("p (c u) -> p c u", u=RCHUNK)
                for t in range(0, G, 2):
                    j = g + t
                    lhs = pt3[:, t:t + 2]  # [128, 2, 256] both tiles
                    rhs = vt8_sb[:, j * VW:(j + 2) * VW].rearrange(
                        "p (c w) -> p c w", w=VW)
                    for h in ((1,) if u0 else (0, 1)):
                        # PSUM start=1 marks the whole 2KB zero region
                        # pending-zero; each write to a pending byte zeroes
                        # then writes. So ONLY the very first inst starts:
                        # the w1 split's first write rides the same mark. A
                        # second start would re-mark w0's bytes and wipe its
                        # pair-0 contribution on the next accumulation.
                        first = (j == 0)
                        last = (j + 1 == last_j[h])
                        nc.tensor.matmul(
                            pv[h][:, 0:128],
                            lhs[:, :, h * 128:h * 128 + 128],
                            rhs[:, :, 0:128],
                            start=first, stop=False, perf_mode=DR,
                            skip_group_check=True,
                        )
                        nc.tensor.matmul(
                            pv[h][:, 128:VW],
                            lhs[:, :, h * 128:h * 128 + 128],
                            rhs[:, :, 128:VW],
                            start=False, stop=last, perf_mode=DR,
                            skip_group_check=True,
                        )
        else:
            def emit_pv(pt, g, u0, get_pv=get_pv, last_j=last_j):
                ensure_v16(min(g + 3 * G, BAND))
                pv = get_pv()
                for t in range(G):
                    j = g + t
                    for h in ((1,) if u0 else (0, 1)):
                        nc.tensor.matmul(
                            pv[h][:],
                            pt[:, t * RCHUNK + h * 128: t * RCHUNK + h * 128 + 128],
                            vt16_sb[:, j * VW:(j + 1) * VW],
                            start=(j == 0), stop=(j == last_j[h]),
                        )

        def epilogue(get_pv=get_pv, r=r):
            pv = get_pv()
            if dbg is not None and r == 1:
                for h in range(2):
                    tl = const.tile([128, VW], F32, tag=f"pv{h}_sb",
                                    name=f"pv{h}_sb")
                    dbg[f"pv{h}_sb"] = tl
                    nc.scalar.copy(tl[:], pv[h][:])
            for h in range(2):
                linv = ep_pool.tile([128, 1], F32, tag="linv")
                nc.vector.reciprocal(linv[:], pv[h][:, DE:DE + 1])
                osb = ep_pool.tile([128, DE], F32, tag="osb")
                nc.vector.tensor_scalar_mul(osb[:], pv[h][:, 0:DE], linv[:])
                rows = r * RCHUNK + h * 128
                nc.sync.dma_start(outd[rows:rows + 128, :], osb[:])

        for g in range(0, ncols, G):
            gb = g - BAND * r
            if r == 0:
                ensure_k16(CT * (g + 2 * G))
            else:
                while q8_fill:  # chunks 1-3 read q8
                    q8_fill.pop(0)()
                # prefetch margin: the filler's PSUM->SBUF copy takes ~1.2us,
                # so pull k8 coverage ~2 groups ahead of the ST that reads it
                ensure_k8(CT * (g + 3 * G))
            # when every col tile in the group has j' >= 8, rows 0..127
            # of the chunk are entirely non-causal: compute only the
            # high 128 rows (u0=128) and skip the h=0 PV matmuls.
            u0 = 128 if gb >= 8 else 0
            st = ps_st.tile([128, G * RCHUNK], F32, tag="st")
            for t in range(G):
                j = g + t
                if fp8:
                    nc.tensor.matmul(
                        st[:, t * RCHUNK + u0:(t + 1) * RCHUNK],
                        k8_3[:, :, j * CT:(j + 1) * CT],
                        q8_3[:, :, r * RCHUNK + u0:(r + 1) * RCHUNK],
                        start=True, stop=True, perf_mode=DR,
                    )
                else:
                    # chunk 0 holds the few-key causal rows, most sensitive
                    # to score noise: exact bf16 K/Q there
                    for kd in range(2):
                        nc.tensor.matmul(
                            st[:, t * RCHUNK + u0:(t + 1) * RCHUNK],
                            k16_sb[kd][:, j * CT:(j + 1) * CT],
                            q16_sb[kd][:, u0:RCHUNK],
                            start=(kd == 0), stop=(kd == 1),
                        )
            pace_fillers()
            # ramp the pipeline down toward the end of the last chunk so
            # the tail drain after the final ST group is short
            depth = 6 if not (r == 3 and g >= ncols - 4 * G) else 3
            while len(pending) >= depth:
                drain_one()
            pdt = F8 if fp8 else BF16
            msk = m8_sb if fp8 else m16_sb
            pt = pt_pool.tile([128, G * RCHUNK], pdt, tag="pt8" if fp8 else "pt16")
            # fp8 chunks store P' = exp(s/16 - 2): score outliers reach
            # ~16*6 (heavy |q||k| tails), and exp would overflow fp8e4's
            # 240 max -> inf -> NaN after the 0-mask. The constant bias
            # cancels exactly in the row-sum normalization.
            bias = nbias[:] if fp8 else 0.0
            if u0:
                st_v = st[:].rearrange("p (c w) -> p c w", w=RCHUNK)[:, :, u0:]
                pt_v = pt[:].rearrange("p (c w) -> p c w", w=RCHUNK)[:, :, u0:]
                nc.scalar.activation(
                    pt_v, st_v, mybir.ActivationFunctionType.Exp,
                    scale=0.0625, bias=bias,
                )
            else:
                nc.scalar.activation(
                    pt[:], st[:], mybir.ActivationFunctionType.Exp,
                    scale=0.0625, bias=bias,
                )
            if gb >= 0:
                # diagonal band: only the 128-row window starting at u0 of
                # each tile mixes causal/non-causal entries (rows below are
                # never read thanks to the u0 skip, rows above are fully
                # causal), so the 0/1 mask covers just that window.
                pt_w = pt[:].rearrange(
                    "p (c w) -> p c w", w=RCHUNK)[:, :, u0:u0 + 128]
                mk_w = msk[:, gb * 128:(gb + G) * 128].rearrange(
                    "p (c w) -> p c w", w=128)
                _mask_mul(nc, r, gb, pt_w, mk_w)
            if dbg is not None and r == 1 and g in (0, 16, 24):
                key = {0: "ptA_sb", 16: "ptB_sb", 24: "ptC_sb"}[g]
                tl = const.tile([128, 1024], F8, tag=key, name=key)
                dbg[key] = tl
                nc.gpsimd.tensor_copy(tl[:], pt[:])
            is_last = (g + G >= ncols)
            pending.append((
                lambda pt=pt, g=g, u0=u0, f=emit_pv: f(pt, g, u0),
                epilogue if is_last else None,
            ))
    while pending:
        drain_one()
    if dbg is not None:
        nc.sync.dma_start(dbg["ptA"][:, :], dbg["ptA_sb"][:])
        nc.sync.dma_start(dbg["ptB"][:, :], dbg["ptB_sb"][:])
        nc.sync.dma_start(dbg["ptC"][:, :], dbg["ptC_sb"][:])
        nc.sync.dma_start(dbg["pv0"][:, :], dbg["pv0_sb"][:])
        nc.sync.dma_start(dbg["pv1"][:, :], dbg["pv1_sb"][:])
        nc.sync.dma_start(dbg["k8o"][:, :], k8_sb[:])
        nc.sync.dma_start(dbg["q8o"][:, :], q8_sb[:])
        nc.sync.dma_start(dbg["vt8o"][:, :], vt8_sb[:])
        nc.sync.dma_start(dbg["m8o"][:, :], m8_sb[:])


def _mask_mul(nc, r, gb, pt_v, mk_v):
    # chunks 1-2 run while DVE is saturated with K/V copies -> gpsimd,
    # except the last band groups (gb >= 8) whose PVs drain into the next
    # chunk: the slow gpsimd there would stall the in-order PE at the
    # boundary. chunk 0 (bf16, 2x mode) and chunk 3 (DVE idle) -> DVE.
    if r in (1, 2, 3) and gb < 8:
        nc.gpsimd.tensor_mul(pt_v, pt_v, mk_v)
    else:
        nc.vector.tensor_mul(pt_v, pt_v, mk_v)


def _host_inputs(x, Wq, Wk, Wv):
    xT = np.ascontiguousarray(x.T)                       # [256, 8192] f32
    x8 = np.ascontiguousarray(
        xT.reshape(2, 128, S).transpose(1, 0, 2).reshape(128, 2 * S)
    ).astype(NPF8)
    xT16 = np.ascontiguousarray(xT[:, :BCOLS]).astype(NPBF16).reshape(2, 128, BCOLS)
    wqb = np.ascontiguousarray(Wq.T).astype(NPBF16).reshape(2, 128, DIM)
    wkb = np.ascontiguousarray(Wk.T).astype(NPBF16).reshape(2, 128, DIM)
    wvb = np.ascontiguousarray(Wv.T).astype(NPBF16).reshape(2, 128, DE)
    wk8 = np.ascontiguousarray(
        Wk.T.reshape(2, 128, DIM).transpose(1, 0, 2).reshape(128, 2 * DIM)
    ).astype(NPF8)
    wv8 = np.ascontiguousarray(
        Wv.T.reshape(2, 128, DE).transpose(1, 0, 2).reshape(128, 2 * DE)
    ).astype(NPF8)
    k_idx = np.arange(128)[:, None, None]
    jp = np.arange(BAND)[None, :, None]
    u = 128 * (jp >= 8) + np.arange(128)[None, None, :]
    in_maps = []
    wq8 = np.ascontiguousarray(
        Wq.T.reshape(2, 128, DIM).transpose(1, 0, 2).reshape(128, 2 * DIM)
    ).astype(NPF8)
    for c in range(NCORES):
        xqf = np.ascontiguousarray(x[c::NCORES].T)
        xq = np.ascontiguousarray(
            xqf[:, :RCHUNK]).astype(NPBF16).reshape(2, 128, RCHUNK)
        xq8 = np.ascontiguousarray(
            xqf.reshape(2, 128, RPC).transpose(1, 0, 2).reshape(128, 2 * RPC)
        ).astype(NPF8)
        m = (128 * jp + k_idx <= 8 * u + c)
        m = np.ascontiguousarray(m.reshape(128, BAND * 128))
        in_maps.append({
            "x8": x8, "xT": xT16, "xqT": xq, "xq8": xq8, "wq8": wq8,
            "wqT": wqb, "wkT": wkb,
            "wvT": wvb, "wk8": wk8, "wv8": wv8,
            "m16": m.astype(NPBF16), "m8": m.astype(NPF8),
        })
    return in_maps


def kernel(x, Wq, Wk, Wv, _trace=False, _trace_kwargs=None):
    if "nc" not in _cached:
        _cached["nc"] = _build_nc()
    nc = _cached["nc"]
    in_maps = _host_inputs(
        np.asarray(x, np.float32), np.asarray(Wq, np.float32),
        np.asarray(Wk, np.float32), np.asarray(Wv, np.float32),
    )
    kw = dict(_trace_kwargs or {})
    res = run_bass_kernel_spmd(
        nc, in_maps, core_ids=list(range(NCORES)), trace=_trace, **kw
    )
    out = np.empty((S, DE), np.float32)
    for c in range(NCORES):
        out[c::NCORES] = res.results[c]["out"]
    _cached["last_results"] = res
    return out


# revision 73
# speedup vs baseline: 1.1275x; 1.0087x over previous
"""Causal attention head (S=8192, De=dim=256) on 8 trn2 NeuronCores.

Math (reference):
    Q = Wq @ x.T; K = Wk @ x.T; V = Wv @ x.T
    S = (Q.T @ K) / sqrt(256); causal mask (upper tri -> -inf)
    out = softmax(S, axis=1) @ V.T          # [8192, 256]

Sharding: core c owns rows c::8 (stride-8 interleave) -> every core's
row block has a near-identical causal prefix profile, so the SPMD kernel
is identical across cores; all per-core variation is input data.

Per-core kernel, fp8e4 DoubleRow edition. The PE's fp8 DoubleRow mode
contracts 2x128 partitions per pass at 0.5 cycles/row (4x bf16 for a
256-deep contraction), so the large matmuls run in fp8; the few-key
early causal rows (chunk 0 = local rows 0..255) keep an exact bf16
score+PV path since score noise doesn't average out there.
  - K: col tiles 0..15 bf16-generated (kept as k16 for chunk-0 ST,
    gpsimd-quantized into k8), tiles 16..63 fp8 DoubleRow into k8.
  - V: tiles 0..15 bf16 into vt16 (chunk-0 PV) + gpsimd-converted into
    vt8; tiles 16..63 fp8 DoubleRow straight into vt8.
  - Q bf16 -> q8 (all rows) and q16 (chunk-0 rows).
  - scores: S.T tile = K_j.T @ Q_r as ONE DoubleRow inst per 128-col
    tile for chunks 1-3, bf16 two-inst contraction for chunk 0.
  - exp on ACT: fp8 out with exponent bias -2 for chunks 1-3 (score
    outliers reach ~95, exp(s/16) would overflow fp8e4's 240 max; the
    bias cancels in the row-sum normalization), bf16 out for chunk 0.
  - causal band masks: 0/1 multiply over just the 128-row mixed window
    per tile; gpsimd for early band groups, DVE (2x bf16) otherwise.
  - PV: chunk 0 in bf16 (exact V), chunks 1-3 DoubleRow over col-tile
    pairs, moving dim split (0:128 | 128:257) to stay under the 512
    moving-row ISA limit; ones column yields row sums for free.
  - K/V generation is emitted as "filler" groups interleaved into the
    attention chunks (just-in-time via ensure_*), so their PSUM->SBUF
    copies overlap the exp-bound phase; the softmax pipeline is 6 deep
    and carried across chunk boundaries.
No softmax max-subtraction needed in fp32: |scores/16| <= ~6.
"""

import sys

sys.path.insert(0, "/opt/trn_rl_repo")

from contextlib import ExitStack

import ml_dtypes
import numpy as np

import concourse.bass as bass
import concourse.mybir as mybir
import concourse.tile as tile
from concourse import bacc
from concourse.bass_utils import run_bass_kernel_spmd

BF16 = mybir.dt.bfloat16
F8 = mybir.dt.float8e4
F32 = mybir.dt.float32
NPBF16 = ml_dtypes.bfloat16
NPF8 = ml_dtypes.float8_e4m3
DR = mybir.MatmulPerfMode.DoubleRow

S, DIM, DE = 8192, 256, 256
NCORES = 8
RPC = S // NCORES          # 1024 rows per core
RCHUNK = 256               # rows per S.T matmul (moving free dim)
NRC = RPC // RCHUNK        # 4 row chunks per core
CT = 128                   # col tile (PE partition)
NCT = S // CT              # 64 col tiles total
BAND = 2048 // CT          # 16 col tiles per causal band of a row chunk
VW = DE + 1                # V.T chunk width incl. ones column
BCOLS = 2048               # cols covered by the bf16 K/V path (tiles < 16)

_cached = {}


def _build_nc(repeat=0, debug=False):
    nc = bacc.Bacc("TRN2", target_bir_lowering=False, debug=False,
                   num_devices=NCORES)
    x8d = nc.dram_tensor("x8", [128, 2 * S], F8, kind="ExternalInput")
    xTd = nc.dram_tensor("xT", [2, 128, BCOLS], BF16, kind="ExternalInput")
    xqT = nc.dram_tensor("xqT", [2, 128, RCHUNK], BF16, kind="ExternalInput")
    xq8d = nc.dram_tensor("xq8", [128, 2 * RPC], F8, kind="ExternalInput")
    wq8d = nc.dram_tensor("wq8", [128, 2 * DIM], F8, kind="ExternalInput")
    wqT = nc.dram_tensor("wqT", [2, 128, DIM], BF16, kind="ExternalInput")
    wkT = nc.dram_tensor("wkT", [2, 128, DIM], BF16, kind="ExternalInput")
    wvT = nc.dram_tensor("wvT", [2, 128, DE], BF16, kind="ExternalInput")
    wk8d = nc.dram_tensor("wk8", [128, 2 * DIM], F8, kind="ExternalInput")
    wv8d = nc.dram_tensor("wv8", [128, 2 * DE], F8, kind="ExternalInput")
    m16d = nc.dram_tensor("m16", [128, BAND * 128], BF16, kind="ExternalInput")
    m8d = nc.dram_tensor("m8", [128, BAND * 128], F8, kind="ExternalInput")
    outd = nc.dram_tensor("out", [RPC, DE], F32, kind="ExternalOutput")
    dbg = None
    if debug:
        dbg = {
            "k8o": nc.dram_tensor("k8o", [128, 2 * S], F8, kind="ExternalOutput"),
            "q8o": nc.dram_tensor("q8o", [128, 2 * RPC], F8, kind="ExternalOutput"),
            "vt8o": nc.dram_tensor("vt8o", [128, NCT * VW], F8, kind="ExternalOutput"),
            "m8o": nc.dram_tensor("m8o", [128, BAND * 128], F8, kind="ExternalOutput"),
            "ptA": nc.dram_tensor("ptA", [128, 1024], F8, kind="ExternalOutput"),
            "ptB": nc.dram_tensor("ptB", [128, 1024], F8, kind="ExternalOutput"),
            "ptC": nc.dram_tensor("ptC", [128, 1024], F8, kind="ExternalOutput"),
            "pv0": nc.dram_tensor("pv0", [128, VW], F32, kind="ExternalOutput"),
            "pv1": nc.dram_tensor("pv1", [128, VW], F32, kind="ExternalOutput"),
        }

    with tile.TileContext(nc) as tc, ExitStack() as ctx:
        const = ctx.enter_context(tc.tile_pool(name="const", bufs=1))
        ps_st = ctx.enter_context(tc.tile_pool(name="ps_st", bufs=3, space="PSUM"))
        ps_pv = ctx.enter_context(tc.tile_pool(name="ps_pv", bufs=1, space="PSUM"))
        pt_pool = ctx.enter_context(tc.tile_pool(name="pt", bufs=9))
        ep_pool = ctx.enter_context(tc.tile_pool(name="ep", bufs=4))

        def body(_iv=None):
            _emit(nc, tc, const, ps_st, ps_pv, pt_pool, ep_pool,
                  x8d, xTd, xqT, xq8d, wq8d, wqT, wkT, wvT, wk8d, wv8d,
                  m16d, m8d, outd, dbg)

        if repeat:
            with tc.For_i(0, repeat, 1) as _iv:
                body(_iv)
        else:
            body()

    nc.compile()
    return nc


def _emit(nc, tc, const, ps_st, ps_pv, pt_pool, ep_pool,
          x8d, xTd, xqT, xq8d, wq8d, wqT, wkT, wvT, wk8d, wv8d,
          m16d, m8d, outd, dbg=None):
    # ---- constants / staged inputs in SBUF ----
    x8_sb = const.tile([128, 2 * S], F8, tag="x8")
    xt_sb = [const.tile([128, BCOLS], BF16, tag=f"xt{i}", name=f"xt{i}") for i in range(2)]
    xq_sb = [const.tile([128, RCHUNK], BF16, tag=f"xq{i}", name=f"xq{i}") for i in range(2)]
    xq8_sb = const.tile([128, 2 * RPC], F8, tag="xq8")
    wq8_sb = const.tile([128, 2 * DIM], F8, tag="wq8")
    wq_sb = const.tile([128, 2 * DIM], BF16, tag="wq")
    wk_sb = const.tile([128, 2 * DIM], BF16, tag="wk")
    wv_sb = const.tile([128, 2 * DE], BF16, tag="wv")
    wk8_sb = const.tile([128, 2 * DIM], F8, tag="wk8")
    wv8_sb = const.tile([128, 2 * DE], F8, tag="wv8")
    m16_sb = const.tile([128, BAND * 128], BF16, tag="m16")
    m8_sb = const.tile([128, BAND * 128], F8, tag="m8")
    k8_sb = const.tile([128, 2 * S], F8, tag="k8")
    q8_sb = const.tile([128, 2 * RPC], F8, tag="q8")
    k16_sb = [const.tile([128, BCOLS], BF16, tag=f"k16_{i}", name=f"k16_{i}")
              for i in range(2)]
    q16_sb = [const.tile([128, RCHUNK], BF16, tag=f"q16_{i}", name=f"q16_{i}")
              for i in range(2)]
    vt8_sb = const.tile([128, NCT * VW], F8, tag="vt8")
    vt16_sb = const.tile([128, BAND * VW], BF16, tag="vt16")

    x8_3 = x8_sb[:].rearrange("p (i c) -> p i c", i=2)     # [128,2,S]
    wk8_3 = wk8_sb[:].rearrange("p (i d) -> p i d", i=2)   # [128,2,256]
    wv8_3 = wv8_sb[:].rearrange("p (i e) -> p i e", i=2)   # [128,2,256]
    k8_3 = k8_sb[:].rearrange("p (i c) -> p i c", i=2)     # [128,2,S]
    q8_3 = q8_sb[:].rearrange("p (i r) -> p i r", i=2)     # [128,2,RPC]
    xq8_3 = xq8_sb[:].rearrange("p (i r) -> p i r", i=2)
    wq8_3 = wq8_sb[:].rearrange("p (i d) -> p i d", i=2)

    # input staging: weights ride the otherwise-idle DVE queue, x.T (bf16)
    # in fine chunks on ACT (K-bf16 consumes it first), x8 immediately on
    # SP in consumption order, xq+masks via gpsimd SWDGE.
    # xq + wq lead the SP queue: Q-gen is the serial prefix of the whole
    # kernel, and the SWDGE path starts ~2us slower than HWDGE
    for i in range(2):
        nc.sync.dma_start(xq_sb[i][:], xqT[i, :, :])
        nc.sync.dma_start(wq_sb[:, i * DIM:(i + 1) * DIM], wqT[i, :, :])
    nc.sync.dma_start(xq8_sb[:], xq8d[:, :])
    nc.sync.dma_start(wq8_sb[:], wq8d[:, :])
    for i in range(2):
        nc.gpsimd.dma_start(wk_sb[:, i * DIM:(i + 1) * DIM], wkT[i, :, :])
        nc.gpsimd.dma_start(wv_sb[:, i * DE:(i + 1) * DE], wvT[i, :, :])
    nc.gpsimd.dma_start(wv8_sb[:], wv8d[:, :])
    nc.gpsimd.dma_start(wk8_sb[:], wk8d[:, :])
    nc.gpsimd.dma_start(m8_sb[:], m8d[:, :])
    nc.gpsimd.dma_start(m16_sb[:], m16d[:, :])
    # first xt half on ACT (fast start for k16 p=0); second half on SP
    # so ACT's sequencer is free when the exp stream begins
    for i in range(2):
        nc.scalar.dma_start(xt_sb[i][:, 0:1024], xTd[i, :, 0:1024])
    for i in range(2):
        nc.sync.dma_start(xt_sb[i][:, 1024:BCOLS], xTd[i, :, 1024:BCOLS])
    XCH = 2048
    for o in range(0, S, XCH):
        for i in range(2):
            nc.sync.dma_start(x8_sb[:, i * S + o: i * S + o + XCH],
                              x8d[:, i * S + o: i * S + o + XCH])
    # per-partition bias vector for the fp8 exp (see below)
    nbias = const.tile([128, 1], F32, tag="nbias")
    nc.vector.memset(nbias[:], -2.0)
    # dummy activation right away: pulls the 1.3us Exp table load into the
    # initial DMA-wait window instead of the first real exp
    warm = const.tile([128, 1], F32, tag="warm")
    nc.scalar.activation(warm[:], nbias[:],
                         mybir.ActivationFunctionType.Exp)
    # ones columns for V.T: only col 256 of each VW-chunk needs the 1.0
    # (on DVE: strided single-element writes, trivially cheap)
    nc.vector.memset(
        vt8_sb[:].rearrange("p (c w) -> p c w", w=VW)[:, :, DE:DE + 1], 1.0)
    nc.vector.memset(
        vt16_sb[:].rearrange("p (c w) -> p c w", w=VW)[:, :, DE:DE + 1], 1.0)

    # round-robin of PSUM->SBUF copy engines; ACT also runs all exps so it
    # gets a lighter share.
    cp_state = [0]

    def cp(dst, src):
        cp_state[0] += 1
        # the first ~12 copies happen before the exp stream ramps up, so
        # ACT can share them evenly; after that ACT is exp-bound and only
        # takes every 8th
        if cp_state[0] <= 18 and cp_state[0] % 2 == 0:
            nc.scalar.copy(dst, src)
        else:
            nc.vector.tensor_copy(dst, src)

    # ---- Q, chunk-0 rows only, in bf16 (the serial prefix) ----
    for d in range(2):
        ps = ps_st.tile([128, 1024], F32, tag="st", name="psq")
        for kd in range(2):
            nc.tensor.matmul(
                ps[:, 0:RCHUNK],
                wq_sb[:, kd * DIM + d * 128: kd * DIM + d * 128 + 128],
                xq_sb[kd][:],
                start=(kd == 0), stop=(kd == 1),
            )
        cp(q16_sb[d][:], ps[:, 0:RCHUNK])

    def _q8_rest():
        # q8 rows 256..1023 via fp8 DoubleRow (chunks 1-3 average >=2048
        # keys, so the fp8 generation noise washes out; rows 0..255 of q8
        # are never read - chunk 0 uses q16)
        for d in range(2):
            ps = ps_st.tile([128, 1024], F32, tag="st", name="psq8")
            for n in range(RCHUNK, RPC, RCHUNK):
                nc.tensor.matmul(
                    ps[:, n:n + RCHUNK],
                    wq8_3[:, :, d * 128:(d + 1) * 128],
                    xq8_3[:, :, n:n + RCHUNK],
                    start=True, stop=True, perf_mode=DR,
                )
            cp(q8_sb[:, d * RPC + RCHUNK:(d + 1) * RPC], ps[:, RCHUNK:RPC])

    # ---- filler closures, interleaved into the attention chunks so their
    # PSUM->SBUF copies (DVE/ACT) overlap the exp-bound phase: bf16 V.T
    # tiles 0..15 into vt16 (chunk-0 PV), fp8 DoubleRow vt8 (all 64
    # tiles), and fp8 K tiles 16..63.
    def _k16_group(d, p):
        # K tiles 0..15 in bf16 (x.T cols < 2048), kept in bf16 for
        # chunk-0's exact ST and gpsimd-quantized to k8 for the fp8 chunks
        def go():
            ps = ps_st.tile([128, 1024], F32, tag="st", name="psk")
            for nn in range(0, 1024, 512):
                for kd in range(2):
                    nc.tensor.matmul(
                        ps[:, nn:nn + 512],
                        wk_sb[:, kd * DIM + d * 128: kd * DIM + d * 128 + 128],
                        xt_sb[kd][:, p + nn:p + nn + 512],
                        start=(kd == 0), stop=(kd == 1),
                    )
            cp(k16_sb[d][:, p:p + 1024], ps[:])
            nc.gpsimd.tensor_copy(k8_sb[:, d * S + p:d * S + p + 1024],
                                  k16_sb[d][:, p:p + 1024])
        return go

    def _v16_group(g4):
        def go():
            ps = ps_st.tile([128, 1024], F32, tag="st", name="psv")
            for jj in range(4):
                j = g4 + jj
                for kd in range(2):
                    nc.tensor.matmul(
                        ps[:, jj * DE:(jj + 1) * DE],
                        xt_sb[kd][:, j * CT:(j + 1) * CT],
                        wv_sb[:, kd * DE:(kd + 1) * DE],
                        start=(kd == 0), stop=(kd == 1),
                    )
            vt_view = vt16_sb[:, g4 * VW:(g4 + 4) * VW].rearrange(
                "p (c w) -> p c w", w=VW)[:, :, 0:DE]
            cp(vt_view, ps[:].rearrange("p (c w) -> p c w", w=DE))
            nc.gpsimd.tensor_copy(vt8_sb[:, g4 * VW:(g4 + 4) * VW],
                                  vt16_sb[:, g4 * VW:(g4 + 4) * VW])
        return go

    def _v8_group(g4):
        def go():
            ps = ps_st.tile([128, 1024], F32, tag="st", name="psv8")
            for jj in range(4):
                j = g4 + jj
                nc.tensor.matmul(
                    ps[:, jj * DE:(jj + 1) * DE],
                    x8_3[:, :, j * CT:(j + 1) * CT],
                    wv8_3,
                    start=True, stop=True, perf_mode=DR,
                )
            vt_view = vt8_sb[:, g4 * VW:(g4 + 4) * VW].rearrange(
                "p (c w) -> p c w", w=VW)[:, :, 0:DE]
            cp(vt_view, ps[:].rearrange("p (c w) -> p c w", w=DE))
        return go

    def _k8_group(d, c0):
        def go():
            ps = ps_st.tile([128, 1024], F32, tag="st", name="psk8")
            for nn in range(0, 1024, 256):
                nc.tensor.matmul(
                    ps[:, nn:nn + 256],
                    wk8_3[:, :, d * 128:(d + 1) * 128],
                    x8_3[:, :, c0 + nn:c0 + nn + 256],
                    start=True, stop=True, perf_mode=DR,
                )
            cp(k8_sb[:, d * S + c0:d * S + c0 + 1024], ps[:])
        return go

    k16_fill = [(p, _k16_group(d, p))
                for p in range(0, BCOLS, 1024) for d in range(2)]
    q8_fill = [_q8_rest]
    v16_fill = [(g4, _v16_group(g4)) for g4 in range(0, BAND, 4)]
    v8_fill = [(g4, _v8_group(g4)) for g4 in range(BAND, NCT, 4)]
    k8_fill = [(c0, _k8_group(d, c0))
               for c0 in range(BCOLS, S, 1024) for d in range(2)]

    def ensure_k16(upto_col):
        while k16_fill and k16_fill[0][0] < upto_col:
            k16_fill.pop(0)[1]()

    def ensure_v16(upto_tile):
        while v16_fill and v16_fill[0][0] < upto_tile:
            v16_fill.pop(0)[1]()

    def ensure_v8(upto_tile):
        while v8_fill and v8_fill[0][0] < upto_tile:
            v8_fill.pop(0)[1]()

    def ensure_k8(upto_col):
        while k8_fill and k8_fill[0][0] < upto_col:
            k8_fill.pop(0)[1]()

    # proportional pacing: spread the filler groups over the 40 ST groups
    # so their PSUM->SBUF copies never pile up on DVE; vt16 first (chunk-0
    # PV drains earliest), then vt8 slightly ahead of k8 (PV trails ST)
    n_fill = (len(k16_fill) + len(v16_fill) + len(v8_fill)
              + len(k8_fill) + len(q8_fill))
    pace = [0]

    def pace_fillers():
        pace[0] += 1
        target = n_fill * pace[0] // 40
        while (len(k16_fill) + len(v16_fill) + len(v8_fill)
               + len(k8_fill) + len(q8_fill)) > n_fill - target:
            if q8_fill:
                q8_fill.pop(0)()
            elif k16_fill:
                k16_fill.pop(0)[1]()
            elif v16_fill:
                v16_fill.pop(0)[1]()
            elif v8_fill and (len(v8_fill) * 2 >= len(k8_fill) or not k8_fill):
                v8_fill.pop(0)[1]()
            elif k8_fill:
                k8_fill.pop(0)[1]()
            else:
                break

    # ---- attention: per row chunk, stream causal col tiles ----
    # col tiles in groups of G: one 2-bank PSUM tile holds G S.T tiles side
    # by side -> a single ACT exp (and a single band mask multiply) covers
    # the whole group, amortizing ACT overhead.
    #
    # software pipeline, depth 4, carried ACROSS chunk boundaries: PV for
    # group g is emitted after the ST matmuls of group g+4, and a chunk's
    # last PVs (plus its epilogue) drain while the next chunk's ST/exp
    # stream is already running.
    G = 4
    from collections import deque
    pending = deque()  # (emit_pv_fn, tail_fn_or_None)

    def drain_one():
        fn, tail = pending.popleft()
        fn()
        if tail is not None:
            tail()

    for r in range(NRC):
        fp8 = r > 0
        ncols = BAND * (r + 1)
        # h=0 (rows 0..127) accumulation ends at col tile 16r+7 (later
        # tiles are fully masked there); h=1 runs to the last tile.
        last_j = {0: min(BAND * r + 7, ncols - 1), 1: ncols - 1}
        pvbox = []

        def get_pv(pvbox=pvbox):
            # lazy: allocated at the first PV drain, which happens after
            # the previous chunk's epilogue has been emitted (bufs=1 ring)
            if not pvbox:
                pvbox.append([
                    ps_pv.tile([128, VW], F32, tag=f"pv{h}", name=f"pv{h}")
                    for h in range(2)
                ])
            return pvbox[0]

        if fp8:
            # DoubleRow PV over col-tile pairs; moving dim split at 128.
            def emit_pv(pt, g, u0, get_pv=get_pv, last_j=last_j):
                ensure_v16(min(g + G, BAND))
                ensure_v8(g + 3 * G)
                pv = get_pv()
                pt3 = pt[:].rearrange("p (c u) -> p c u", u=RCHUNK)
                for t in range(0, G, 2):
                    j = g + t
                    lhs = pt3[:, t:t + 2]  # [128, 2, 256] both tiles
                    rhs = vt8_sb[:, j * VW:(j + 2) * VW].rearrange(
                        "p (c w) -> p c w", w=VW)
                    for h in ((1,) if u0 else (0, 1)):
                        # PSUM start=1 marks the whole 2KB zero region
                        # pending-zero; each write to a pending byte zeroes
                        # then writes. So ONLY the very first inst starts:
                        # the w1 split's first write rides the same mark. A
                        # second start would re-mark w0's bytes and wipe its
                        # pair-0 contribution on the next accumulation.
                        first = (j == 0)
                        last = (j + 1 == last_j[h])
                        nc.tensor.matmul(
                            pv[h][:, 0:128],
                            lhs[:, :, h * 128:h * 128 + 128],
                            rhs[:, :, 0:128],
                            start=first, stop=False, perf_mode=DR,
                            skip_group_check=True,
                        )
                        nc.tensor.matmul(
                            pv[h][:, 128:VW],
                            lhs[:, :, h * 128:h * 128 + 128],
                            rhs[:, :, 128:VW],
                            start=False, stop=last, perf_mode=DR,
                            skip_group_check=True,
                        )
        else:
            def emit_pv(pt, g, u0, get_pv=get_pv, last_j=last_j):
                ensure_v16(min(g + 3 * G, BAND))
                pv = get_pv()
                for t in range(G):
                    j = g + t
                    for h in ((1,) if u0 else (0, 1)):
                        nc.tensor.matmul(
                            pv[h][:],
                            pt[:, t * RCHUNK + h * 128: t * RCHUNK + h * 128 + 128],
                            vt16_sb[:, j * VW:(j + 1) * VW],
                            start=(j == 0), stop=(j == last_j[h]),
                        )

        def epilogue(get_pv=get_pv, r=r):
            pv = get_pv()
            if dbg is not None and r == 1:
                for h in range(2):
                    tl = const.tile([128, VW], F32, tag=f"pv{h}_sb",
                                    name=f"pv{h}_sb")
                    dbg[f"pv{h}_sb"] = tl
                    nc.scalar.copy(tl[:], pv[h][:])
            for h in range(2):
                linv = ep_pool.tile([128, 1], F32, tag="linv")
                nc.vector.reciprocal(linv[:], pv[h][:, DE:DE + 1])
                osb = ep_pool.tile([128, DE], F32, tag="osb")
                nc.vector.tensor_scalar_mul(osb[:], pv[h][:, 0:DE], linv[:])
                rows = r * RCHUNK + h * 128
                nc.sync.dma_start(outd[rows:rows + 128, :], osb[:])

        for g in range(0, ncols, G):
            gb = g - BAND * r
            if r == 0:
                ensure_k16(CT * (g + 3 * G))
            else:
                while q8_fill:  # chunks 1-3 read q8
                    q8_fill.pop(0)()
                # prefetch margin: the filler's PSUM->SBUF copy takes ~1.2us,
                # so pull k8 coverage ~2 groups ahead of the ST that reads it
                ensure_k8(CT * (g + 3 * G))
            # when every col tile in the group has j' >= 8, rows 0..127
            # of the chunk are entirely non-causal: compute only the
            # high 128 rows (u0=128) and skip the h=0 PV matmuls.
            u0 = 128 if gb >= 8 else 0
            st = ps_st.tile([128, G * RCHUNK], F32, tag="st")
            for t in range(G):
                j = g + t
                if fp8:
                    nc.tensor.matmul(
                        st[:, t * RCHUNK + u0:(t + 1) * RCHUNK],
                        k8_3[:, :, j * CT:(j + 1) * CT],
                        q8_3[:, :, r * RCHUNK + u0:(r + 1) * RCHUNK],
                        start=True, stop=True, perf_mode=DR,
                    )
                else:
                    # chunk 0 holds the few-key causal rows, most sensitive
                    # to score noise: exact bf16 K/Q there
                    for kd in range(2):
                        nc.tensor.matmul(
                            st[:, t * RCHUNK + u0:(t + 1) * RCHUNK],
                            k16_sb[kd][:, j * CT:(j + 1) * CT],
                            q16_sb[kd][:, u0:RCHUNK],
                            start=(kd == 0), stop=(kd == 1),
                        )
            pace_fillers()
            # ramp the pipeline down toward the end of the last chunk so
            # the tail drain after the final ST group is short
            depth = 6 if not (r == 3 and g >= ncols - 4 * G) else 3
            while len(pending) >= depth:
                drain_one()
            pdt = F8 if fp8 else BF16
            msk = m8_sb if fp8 else m16_sb
            pt = pt_pool.tile([128, G * RCHUNK], pdt, tag="pt8" if fp8 else "pt16")
            # fp8 chunks store P' = exp(s/16 - 2): score outliers reach
            # ~16*6 (heavy |q||k| tails), and exp would overflow fp8e4's
            # 240 max -> inf -> NaN after the 0-mask. The constant bias
            # cancels exactly in the row-sum normalization.
            bias = nbias[:] if fp8 else 0.0
            if u0:
                st_v = st[:].rearrange("p (c w) -> p c w", w=RCHUNK)[:, :, u0:]
                pt_v = pt[:].rearrange("p (c w) -> p c w", w=RCHUNK)[:, :, u0:]
                nc.scalar.activation(
                    pt_v, st_v, mybir.ActivationFunctionType.Exp,
                    scale=0.0625, bias=bias,
                )
            else:
                nc.scalar.activation(
                    pt[:], st[:], mybir.ActivationFunctionType.Exp,
                    scale=0.0625, bias=bias,
                )
            if gb >= 0:
                # diagonal band: only the 128-row window starting at u0 of
                # each tile mixes causal/non-causal entries (rows below are
                # never read thanks to the u0 skip, rows above are fully
                # causal), so the 0/1 mask covers just that window.
                pt_w = pt[:].rearrange(
                    "p (c w) -> p c w", w=RCHUNK)[:, :, u0:u0 + 128]
                mk_w = msk[:, gb * 128:(gb + G) * 128].rearrange(
                    "p (c w) -> p c w", w=128)
                _mask_mul(nc, r, gb, pt_w, mk_w)
            if dbg is not None and r == 1 and g in (0, 16, 24):
                key = {0: "ptA_sb", 16: "ptB_sb", 24: "ptC_sb"}[g]
                tl = const.tile([128, 1024], F8, tag=key, name=key)
                dbg[key] = tl
                nc.gpsimd.tensor_copy(tl[:], pt[:])
            is_last = (g + G >= ncols)
            pending.append((
                lambda pt=pt, g=g, u0=u0, f=emit_pv: f(pt, g, u0),
                epilogue if is_last else None,
            ))
    while pending:
        drain_one()
    if dbg is not None:
        nc.sync.dma_start(dbg["ptA"][:, :], dbg["ptA_sb"][:])
        nc.sync.dma_start(dbg["ptB"][:, :], dbg["ptB_sb"][:])
        nc.sync.dma_start(dbg["ptC"][:, :], dbg["ptC_sb"][:])
        nc.sync.dma_start(dbg["pv0"][:, :], dbg["pv0_sb"][:])
        nc.sync.dma_start(dbg["pv1"][:, :], dbg["pv1_sb"][:])
        nc.sync.dma_start(dbg["k8o"][:, :], k8_sb[:])
        nc.sync.dma_start(dbg["q8o"][:, :], q8_sb[:])
        nc.sync.dma_start(dbg["vt8o"][:, :], vt8_sb[:])
        nc.sync.dma_start(dbg["m8o"][:, :], m8_sb[:])


def _mask_mul(nc, r, gb, pt_v, mk_v):
    # chunks 1-2 run while DVE is saturated with K/V copies -> gpsimd,
    # except the last band groups (gb >= 8) whose PVs drain into the next
    # chunk: the slow gpsimd there would stall the in-order PE at the
    # boundary. chunk 0 (bf16, 2x mode) and chunk 3 (DVE idle) -> DVE.
    if r in (1, 2, 3) and gb < 8:
        nc.gpsimd.tensor_mul(pt_v, pt_v, mk_v)
    else:
        nc.vector.tensor_mul(pt_v, pt_v, mk_v)


def _host_inputs(x, Wq, Wk, Wv):
    xT = np.ascontiguousarray(x.T)                       # [256, 8192] f32
    x8 = np.ascontiguousarray(
        xT.reshape(2, 128, S).transpose(1, 0, 2).reshape(128, 2 * S)
    ).astype(NPF8)
    xT16 = np.ascontiguousarray(xT[:, :BCOLS]).astype(NPBF16).reshape(2, 128, BCOLS)
    wqb = np.ascontiguousarray(Wq.T).astype(NPBF16).reshape(2, 128, DIM)
    wkb = np.ascontiguousarray(Wk.T).astype(NPBF16).reshape(2, 128, DIM)
    wvb = np.ascontiguousarray(Wv.T).astype(NPBF16).reshape(2, 128, DE)
    wk8 = np.ascontiguousarray(
        Wk.T.reshape(2, 128, DIM).transpose(1, 0, 2).reshape(128, 2 * DIM)
    ).astype(NPF8)
    wv8 = np.ascontiguousarray(
        Wv.T.reshape(2, 128, DE).transpose(1, 0, 2).reshape(128, 2 * DE)
    ).astype(NPF8)
    k_idx = np.arange(128)[:, None, None]
    jp = np.arange(BAND)[None, :, None]
    u = 128 * (jp >= 8) + np.arange(128)[None, None, :]
    in_maps = []
    wq8 = np.ascontiguousarray(
        Wq.T.reshape(2, 128, DIM).transpose(1, 0, 2).reshape(128, 2 * DIM)
    ).astype(NPF8)
    for c in range(NCORES):
        xqf = np.ascontiguousarray(x[c::NCORES].T)
        xq = np.ascontiguousarray(
            xqf[:, :RCHUNK]).astype(NPBF16).reshape(2, 128, RCHUNK)
        xq8 = np.ascontiguousarray(
            xqf.reshape(2, 128, RPC).transpose(1, 0, 2).reshape(128, 2 * RPC)
        ).astype(NPF8)
        m = (128 * jp + k_idx <= 8 * u + c)
        m = np.ascontiguousarray(m.reshape(128, BAND * 128))
        in_maps.append({
            "x8": x8, "xT": xT16, "xqT": xq, "xq8": xq8, "wq8": wq8,
            "wqT": wqb, "wkT": wkb,
            "wvT": wvb, "wk8": wk8, "wv8": wv8,
            "m16": m.astype(NPBF16), "m8": m.astype(NPF8),
        })
    return in_maps


def kernel(x, Wq, Wk, Wv, _trace=False, _trace_kwargs=None):
    if "nc" not in _cached:
        _cached["nc"] = _build_nc()
    nc = _cached["nc"]
    in_maps = _host_inputs(
        np.asarray(x, np.float32), np.asarray(Wq, np.float32),
        np.asarray(Wk, np.float32), np.asarray(Wv, np.float32),
    )
    kw = dict(_trace_kwargs or {})
    res = run_bass_kernel_spmd(
        nc, in_maps, core_ids=list(range(NCORES)), trace=_trace, **kw
    )
    out = np.empty((S, DE), np.float32)
    for c in range(NCORES):
        out[c::NCORES] = res.results[c]["out"]
    _cached["last_results"] = res
    return out
